# revision 2
# baseline (speedup 1.0000x reference)
"""GridNetBlock_Att Trainium2 kernel (Bass/Tile, 8 NeuronCores).

Core c handles sample s=c//2, T-half h=c%2 (rows [500h, 500h+500)).
Pre-attention is T-split per core (host supplies x with a reflect-padded
t-halo of 7 rows, which makes the SPMD program identical on all cores);
tiny pair AllGathers combine global stats, and one pair AllGather
exchanges the K/V halves before full-sequence attention.

Pre-attention tiles are "2-stack": 128 partitions = 2 consecutive
row-blocks x 64 channels.  LN-over-channels per psum chunk:
    w1  = (I - BO/64) @ x        (PE; BO = block-ones)    = x - mu
    sqw = Square(w1)             (ACT, psum->sbuf)
    s1  = BO @ sqw               (PE)                     = 64*var
    inv = AbsRecipSqrt(s1/64)    (ACT)                    = 1/sqrt(var)
    z   = w1 * inv               (DVE, psum x sbuf -> bf16)
"""
import time

import numpy as np

EPS = 1e-5
B, C, T, F = 4, 64, 1000, 129
H, E, D = 4, 4, 16
GROUP, KT = 4, 3
DILS = (3, 5, 7)
EF, DF = E * F, D * F      # 516, 2064
TH = T // 2                 # 500 local rows
PADR = 7
RP = TH + 2 * PADR          # 514
FP = F + 14                 # 143
TF = float(T * F)
RW = 3                      # rows per psum chunk
ISCALE = float(1.0 / np.sqrt(EF))
MB = 125                    # attention row block

_CACHE = {}
LAST_EXEC_NS = -1


def _tiles(total, w):
    out, t0 = [], 0
    while t0 < total:
        ww = min(2 * w, total - t0) // 2
        out.append((t0, ww))
        t0 += 2 * ww
    return out


def _chunks(w, rw=RW):
    return [(ci * rw, min(rw, w - ci * rw)) for ci in range((w + rw - 1) // rw)]


# ---------------------------------------------------------------------------
# host-side weight folding
# ---------------------------------------------------------------------------

def _fold_shapes():
    sh = {
        "bo_f": ((128, 128), False),
        "w1m_bf": ((128, 128), True),
        "ind2f": ((2, 128), False), "ident_bf": ((128, 128), True),
        "fold64": ((128, 64), False), "redq": ((128, 2), False),
        "grp4": ((4, 128), False),
        "wbbd": ((128, 128), True), "cbb": ((128, 1), False),
        "gn_g": ((128, 1), False), "gn_b": ((128, 1), False),
        "a0vec": ((128, 1), False),
        "wqkbd": ((128, 64), True), "wvbd": ((128, 128), True),
        "qkb": ((64, 1), False), "qka": ((64, 1), False),
        "vb": ((128, 1), False), "va": ((128, 1), False),
        "grp16": ((64, 16), False), "bc16": ((16, 64), False),
        "grp8": ((128, 8), False), "bc8": ((8, 128), False),
        "pwbd": ((128, 128), True), "pconst": ((128, F), False),
        "pb": ((128, 1), False), "pa": ((128, 1), False),
        "plng": ((128, F), False), "plnb": ((128, F), False),
    }
    for i in range(6):
        for nm in ("gs", "hc", "kg", "cs", "cb", "gc"):
            sh[f"{nm}_{i}"] = ((128, 1), False)
        sh[f"lwT_{i}"] = ((64, 12), False)
        sh[f"gtf_{i}"] = ((64, 1), False)
        sh[f"c64_{i}"] = ((64, 1), False)
    for i in range(3):
        sh[f"wgbd_{i}"] = ((128, 128), True)
    return sh


# loaded from DRAM on demand, not staged in SBUF constants
def _dram_only_shapes():
    return {
        "qlngT": (H, 640), "klngT": (H, 640),
        "qlnbT": (H, 640), "klnbT": (H, 640),
        "vlngT": (H, DF),
    }


def _fold_weights(w):
    f32 = np.float32
    g = {}
    ar = lambda a: np.ascontiguousarray(a, f32)
    dup = lambda v: np.tile(ar(v).reshape(64), 2).reshape(128, 1)

    bo = np.zeros((128, 128), f32)
    bo[:64, :64] = 1.0
    bo[64:, 64:] = 1.0
    g["bo_f"] = bo
    g["w1m_bf"] = np.eye(128, dtype=f32) - bo / 64.0
    ind2 = np.zeros((2, 128), f32)
    ind2[0, :64] = 1.0
    ind2[1, 64:] = 1.0
    g["ind2f"] = ind2
    g["ident_bf"] = np.eye(128, dtype=f32)
    fold2 = np.zeros((128, 64), f32)
    for p in range(128):
        fold2[p, p % 64] = 1.0
    g["fold64"] = fold2
    redq = np.zeros((128, 2), f32)
    redq[:64, 0] = 1.0
    redq[64:, 1] = 1.0
    g["redq"] = redq
    gi4 = np.zeros((4, 128), f32)
    for p in range(128):
        gi4[(p % 64) // 16, p] = 1.0
    g["grp4"] = gi4

    for i in range(6):
        gg = w["br_g"][i].astype(f32)
        cc = w["br_b"][i].astype(f32)
        ia = w["lisa_in"][i].astype(f32)
        ll = w["lisa_ll"][i].astype(f32)
        lh = w["lisa_lh"][i].astype(f32)
        s = (ia + 1.0) * ll
        gap_div = float(F) if i < 3 else float(T)
        g[f"gs_{i}"] = dup(gg * s)
        g[f"hc_{i}"] = dup(gg * (lh + 1.0))
        g[f"kg_{i}"] = dup((-ia * ll * gg) / gap_div)
        g[f"gc_{i}"] = dup(-ia * ll * cc)
        g[f"cs_{i}"] = dup(cc * s)
        g[f"cb_{i}"] = dup(cc * (lh + 1.0))
        g[f"lwT_{i}"] = ar(w["lisa_w"][i].T)
        g[f"gtf_{i}"] = ar((gg / TF).reshape(64, 1))
        g[f"c64_{i}"] = ar(cc.reshape(64, 1))

    cw_ = w["convb_w"].astype(f32)
    gam = w["mix_gamma"].astype(f32)
    bet = w["mix_beta"].astype(f32)

    def bd(m, no):
        z = np.zeros((128, 2 * no), f32)
        z[:64, :no] = m.T
        z[64:, no:] = m.T
        return z

    for i in range(3):
        g[f"wgbd_{i}"] = bd(cw_ * gam[i][None, :], 64)
    g["wbbd"] = bd(cw_ * bet.sum(0)[None, :], 64)
    g["cbb"] = dup(w["convb_b"])
    g["gn_g"] = dup(w["gn_g"])
    g["gn_b"] = dup(w["gn_b"])
    g["a0vec"] = np.full((128, 1), float(w["convb_a"]), f32)

    wqk = np.concatenate([w["q_w"].astype(f32).reshape(H * E, C),
                          w["k_w"].astype(f32).reshape(H * E, C)], 0)
    g["wqkbd"] = bd(wqk, 32)
    g["wvbd"] = bd(w["v_w"].astype(f32).reshape(H * D, C), 64)
    qkb = np.concatenate([w["q_b"].reshape(-1), w["k_b"].reshape(-1)])
    g["qkb"] = np.tile(ar(qkb), 2).reshape(64, 1)
    qka = np.concatenate([np.repeat(w["q_a"], E), np.repeat(w["k_a"], E)])
    g["qka"] = np.tile(ar(qka), 2).reshape(64, 1)
    g["vb"] = np.tile(ar(w["v_b"].reshape(-1)), 2).reshape(128, 1)
    g["va"] = np.tile(ar(np.repeat(w["v_a"], D)), 2).reshape(128, 1)
    g16 = np.zeros((64, 16), f32)
    for p in range(64):
        q, j = divmod(p, 32)
        g16[p, q * 8 + (j // 16) * 4 + (j % 16) // 4] = 1.0
    g["grp16"] = g16
    g["bc16"] = ar(g16.T)
    g8 = np.zeros((128, 8), f32)
    for p in range(128):
        q, j = divmod(p, 64)
        g8[p, q * 4 + j // 16] = 1.0
    g["grp8"] = g8
    g["bc8"] = ar(g8.T)
    for nm, src in (("qlngT", "q_lng"), ("klngT", "k_lng"),
                    ("qlnbT", "q_lnb"), ("klnbT", "k_lnb")):
        m = np.zeros((H, 640), f32)
        for h in range(H):
            m[h, :EF] = w[src][h].reshape(EF)
        g[nm] = m
    g["vlngT"] = ar(w["v_lng"].reshape(H, DF))
    pw = w["proj_w"].astype(f32)
    g["pwbd"] = bd(pw, 64)
    pconst = pw @ w["v_lnb"].reshape(H * D, F).astype(f32)
    g["pconst"] = np.tile(pconst, (2, 1)).reshape(128, F)
    g["pb"] = dup(w["proj_b"])
    g["pa"] = np.full((128, 1), float(w["proj_a"]), f32)
    g["plng"] = np.tile(w["proj_lng"].astype(f32), (2, 1)).reshape(128, F)
    g["plnb"] = np.tile(w["proj_lnb"].astype(f32), (2, 1)).reshape(128, F)
    return g


# ---------------------------------------------------------------------------
# device program
# ---------------------------------------------------------------------------

def _build(dbg=(), phases=6):
    import concourse.bass as bass
    import concourse.bacc as bacc
    import concourse.mybir as mybir
    from concourse import tile
    from contextlib import ExitStack

    f32 = mybir.dt.float32
    bf = mybir.dt.bfloat16
    AF = mybir.ActivationFunctionType
    OP = mybir.AluOpType
    AX = mybir.AxisListType

    nc = bacc.Bacc("TRN2", target_bir_lowering=False, debug=False,
                   num_devices=8)

    def AP(tensor, offset, dims):
        return bass.AP(tensor=tensor, offset=offset,
                       ap=[list(d) for d in dims])

    shapes = _fold_shapes()
    dshapes = _dram_only_shapes()
    x_d = nc.dram_tensor("x_d", [C, RP, F], bf, kind="ExternalInput")
    fw = {n: nc.dram_tensor(n, list(s), bf if b else f32,
                            kind="ExternalInput")
          for n, (s, b) in shapes.items()}
    for n, s in dshapes.items():
        fw[n] = nc.dram_tensor(n, list(s), f32, kind="ExternalInput")

    def idram(name, shape, dt_):
        kind = "ExternalOutput" if name in dbg else "Internal"
        return nc.dram_tensor(name, list(shape), dt_, kind=kind)

    n2_d = idram("n2_d", [3, C, RP, F], bf)
    y_d = idram("y_d", [C, TH, F], bf)
    out_d = idram("out_d", [C, TH, F], bf)
    qkvh_d = idram("qkvh_d", [96, TH, F], bf)
    qkvf_d = idram("qkvf_d", [2, 96, TH, F], bf)
    o_d = idram("o_d", [TH, H, D, F], bf)
    b1_d = idram("b1_d", [3, C, RP, F], bf) if "b1_d" in dbg else None
    dsm_d = idram("dsm_d", [16, 128], f32) if "dsm_d" in dbg else None
    ag1i = nc.dram_tensor("ag1i", [1, 128], f32)
    ag1o = nc.dram_tensor("ag1o", [2, 128], f32)
    ag2i = nc.dram_tensor("ag2i", [1, 3 * 128 * F], f32)
    ag2o = nc.dram_tensor("ag2o", [2, 3 * 128 * F], f32)
    ag3i = nc.dram_tensor("ag3i", [1, 256], f32)
    ag3o = nc.dram_tensor("ag3o", [2, 256], f32)
    fin = nc.dram_tensor("fin", [C, TH, F], f32, kind="ExternalOutput")

    groups = [[0, 1], [2, 3], [4, 5], [6, 7]]

    with nc.allow_low_precision(reason="bf16 pipeline, tol 2e-2"), \
         tile.TileContext(nc) as tc:
        ctx = ExitStack()
        cst = ctx.enter_context(tc.tile_pool(name="cst", bufs=1))
        per = ctx.enter_context(tc.tile_pool(name="per", bufs=1))

        def load_const(name):
            sh, isbf = shapes[name]
            t = cst.tile(list(sh), bf if isbf else f32, name=f"c_{name}",
                         tag=f"c_{name}")
            nc.sync.dma_start(t[:], fw[name].ap())
            return t

        cw = {n: load_const(n) for n in shapes}

        def x2_load(pool, t0, w, nm):
            xt = pool.tile([128, w, F], bf, name=nm, tag=nm)
            src = AP(x_d, t0 * F,
                     [[w * F, 2], [RP * F, C], [F, w], [1, F]])
            nc.sync.dma_start(xt[:], src)
            return xt

        def ln_chunk(sb, ps, src_flat, n, nm):
            w1 = ps.tile([128, RW * FP], f32, name=f"{nm}w1", tag="Lw1")
            s1 = ps.tile([128, RW * FP], f32, name=f"{nm}s1", tag="Ls1")
            nc.tensor.matmul(w1[:, :n], cw["w1m_bf"][:], src_flat,
                             start=True, stop=True)
            sqw = sb.tile([128, RW * FP], f32, name=f"{nm}sqw", tag="Lsq")
            nc.scalar.activation(sqw[:, :n], w1[:, :n], AF.Square)
            nc.tensor.matmul(s1[:, :n], cw["bo_f"][:], sqw[:, :n],
                             start=True, stop=True)
            sd = sb.tile([128, RW * FP], f32, name=f"{nm}sd", tag="Lsd")
            nc.scalar.activation(sd[:, :n], s1[:, :n], AF.Sqrt,
                                 scale=1.0 / 64.0)
            inv = sb.tile([128, RW * FP], f32, name=f"{nm}inv", tag="Linv")
            nc.vector.reciprocal(inv[:, :n], sd[:, :n])
            return w1, inv

        # persistent accumulators
        macc = per.tile([128, 1], f32, name="macc")
        nc.vector.memset(macc[:], 0.0)
        g2acc = [per.tile([128, F], f32, name=f"g2acc_{i}") for i in range(3)]
        for i in range(3):
            nc.vector.memset(g2acc[i][:], 0.0)
        ysacc = per.tile([128, 1], f32, name="ysacc")
        yqacc = per.tile([128, 1], f32, name="yqacc")
        nc.vector.memset(ysacc[:], 0.0)
        nc.vector.memset(yqacc[:], 0.0)

        # ============================ P1 ================================
        with tc.tile_pool(name="p1sb", bufs=3) as sb, \
             tc.tile_pool(name="p1ps", bufs=2, space="PSUM") as ps:
            for (t0, w) in _tiles(TH, 24):
                x2 = x2_load(sb, PADR + t0, w, "p1x")
                for (r0, rr) in _chunks(w):
                    n = rr * F
                    xc = x2[:, r0:r0 + rr, :].rearrange("p a b -> p (a b)")
                    w1, inv = ln_chunk(sb, ps, xc, n, "p1")
                    junk = sb.tile([128, RW * F], bf, name="p1junk",
                                   tag="p1junk")
                    acc = sb.tile([128, 1], f32, name="p1acc", tag="p1acc")
                    nc.vector.scalar_tensor_tensor(
                        junk[:, :n], w1[:, :n], 1.0, inv[:, :n],
                        OP.mult, OP.mult, accum_out=acc[:])
                    nc.vector.tensor_tensor(macc[:], macc[:], acc[:], OP.add)
        nc.sync.dma_start(AP(ag1i, 0, [[1, 128]]), macc[:])
        nc.gpsimd.collective_compute(
            "AllGather", OP.bypass, replica_groups=groups,
            ins=[ag1i.ap()], outs=[ag1o.ap()])
        m_a = per.tile([128, 2], f32, name="m_a")
        nc.sync.dma_start(m_a[:], AP(ag1o, 0, [[1, 128], [128, 2]]))
        mtot = per.tile([128, 1], f32, name="mtot")
        nc.vector.tensor_tensor(mtot[:], m_a[:, 0:1], m_a[:, 1:2], OP.add)

        # filt folds, horizontal stages
        ataps, kcv = [], []
        with tc.tile_pool(name="ffsb", bufs=2) as sb, \
             tc.tile_pool(name="ffps", bufs=2, space="PSUM") as ps:
            m64p = ps.tile([64, 1], f32, name="m64p")
            nc.tensor.matmul(m64p[:], cw["fold64"][:], mtot[:],
                             start=True, stop=True)
            m64 = per.tile([64, 1], f32, name="m64")
            nc.vector.tensor_copy(m64[:], m64p[:])
            for i in range(3):
                a_t, kc_t = _filt_fold(nc, sb, ps, per, cw, m64, i,
                                       f32, bf, AF, OP, AX)
                ataps.append(a_t)
                kcv.append(kc_t)
        if dsm_d is not None:
            nc.sync.dma_start(AP(dsm_d, 0, [[1, 128]]), mtot[:])
            for i in range(3):
                for k in range(KT):
                    nc.sync.dma_start(
                        AP(dsm_d, (1 + i * 3 + k) * 128, [[1, 128]]),
                        ataps[i][:, k:k + 1])
                nc.sync.dma_start(AP(dsm_d, (10 + i) * 128, [[1, 128]]),
                                  kcv[i][:, 0:1])
        dgh = []
        for i in range(3):
            for k in range(KT):
                dt_ = per.tile([128, 128], bf, name=f"dgh_{i}_{k}")
                nc.vector.tensor_scalar_mul(dt_[:], cw["ident_bf"][:],
                                            ataps[i][:, k:k + 1])
                dgh.append(dt_)

        if phases >= 2:
            # ============================ P2 ================================
            with tc.tile_pool(name="p2sb", bufs=3) as sb, \
                 tc.tile_pool(name="p2big", bufs=2) as big, \
                 tc.tile_pool(name="p2ps", bufs=2, space="PSUM") as ps, \
                 tc.tile_pool(name="p2pst", bufs=2, space="PSUM") as pst:
                for (t0, w) in _tiles(RP, 20):
                    x2 = x2_load(sb, t0, w, "p2x")
                    zzt = big.tile([128, 8 + w * FP + 8], bf, name="p2zzt",
                                   tag="p2zzt")
                    zz = zzt[:, 8:8 + w * FP].rearrange("p (a b) -> p a b", b=FP)
                    nc.vector.memset(zzt[:, 0:8], 0.0)
                    nc.vector.memset(zzt[:, 8 + w * FP:8 + w * FP + 8], 0.0)
                    for (r0, rr) in _chunks(w):
                        n = rr * F
                        xc = x2[:, r0:r0 + rr, :].rearrange("p a b -> p (a b)")
                        w1, inv = ln_chunk(sb, ps, xc, n, "p2a")
                        nc.vector.tensor_tensor(
                            zz[:, r0:r0 + rr, 7:7 + F],
                            w1[:, :n].rearrange("p (a b) -> p a b", b=F),
                            inv[:, :n].rearrange("p (a b) -> p a b", b=F),
                            OP.mult)
                    nc.vector.tensor_copy(zz[:, :, 0:7], zz[:, :, 14:7:-1])
                    nc.vector.tensor_copy(zz[:, :, 136:143], zz[:, :, 134:127:-1])
                    gpf = sb.tile([128, w], f32, name="p2gpf", tag="p2gpf")
                    nc.vector.tensor_reduce(gpf[:], zz[:, :, 7:7 + F], AX.X,
                                            OP.add)
                    zzf = zzt[:]
                    for i in range(3):
                        d = DILS[i]
                        grow = sb.tile([128, w], f32, name="p2grow", tag="p2grow")
                        nc.vector.tensor_scalar(grow[:], gpf[:],
                                                cw[f"kg_{i}"][:, 0:1],
                                                kcv[i][:, 0:1], OP.mult, OP.add)
                        b1 = big.tile([128, w, FP], bf, name="p2b1", tag="p2b1")
                        for (r0, rr) in _chunks(w):
                            n = rr * FP
                            bps = pst.tile([128, RW * FP], f32, name="p2bps",
                                           tag="p2bps")
                            for k in range(KT):
                                off = 8 + r0 * FP + (k - 1) * d
                                nc.tensor.matmul(bps[:, :n], dgh[i * KT + k][:],
                                                 zzf[:, off:off + n],
                                                 start=(k == 0), stop=(k == 2))
                            nc.vector.tensor_tensor(
                                b1[:, r0:r0 + rr, :],
                                bps[:, :n].rearrange("p (a b) -> p a b", b=FP),
                                grow[:, r0:r0 + rr].unsqueeze(2)
                                .broadcast_to([128, rr, FP]),
                                OP.add)
                        n2b = big.tile([128, w, F], bf, name="p2n2b",
                                       tag="p2n2b")
                        for (r0, rr) in _chunks(w):
                            n = rr * F
                            bc_ = b1[:, r0:r0 + rr, 7:7 + F]
                            w1, inv = ln_chunk(sb, ps, bc_, n, "p2b")
                            nc.vector.tensor_tensor(
                                n2b[:, r0:r0 + rr, :].rearrange(
                                    "p a b -> p (a b)"),
                                w1[:, :n], inv[:, :n], OP.mult)
                        dst = AP(n2_d, i * C * RP * F + t0 * F,
                                 [[w * F, 2], [RP * F, C], [F, w], [1, F]])
                        nc.sync.dma_start(dst, n2b[:])
                        # gap2 partials over strictly-local rows [PADR, PADR+TH)
                        rng = []
                        for q in range(2):
                            a = max(PADR - (t0 + q * w), 0)
                            bq = min(PADR + TH - (t0 + q * w), w)
                            rng.append((a, bq))
                        if rng[0] == (0, w) and rng[1] == (0, w):
                            red = sb.tile([128, F], f32, name="p2red",
                                          tag="p2red")
                            nc.vector.tensor_reduce(
                                red[:], n2b[:].transpose([0, 2, 1]),
                                AX.X, OP.add)
                            nc.vector.tensor_tensor(g2acc[i][:], g2acc[i][:],
                                                    red[:], OP.add)
                        else:
                            for q in range(2):
                                a, bq = rng[q]
                                if bq <= a:
                                    continue
                                p0, p1 = q * 64, q * 64 + 64
                                redh = sb.tile([128, F], f32, name="p2redh",
                                               tag="p2red")
                                nc.vector.tensor_reduce(
                                    redh[p0:p1],
                                    n2b[p0:p1, a:bq, :]
                                    .transpose([0, 2, 1]),
                                    AX.X, OP.add)
                                nc.vector.tensor_tensor(g2acc[i][p0:p1],
                                                        g2acc[i][p0:p1],
                                                        redh[p0:p1], OP.add)
            for i in range(3):
                nc.sync.dma_start(AP(ag2i, i * 128 * F, [[1, 128 * F]]),
                                  g2acc[i][:])
            nc.gpsimd.collective_compute(
                "AllGather", OP.bypass, replica_groups=groups,
                ins=[ag2i.ap()], outs=[ag2o.ap()])

            # filt folds vertical + gterm2
            gt2 = []
            with tc.tile_pool(name="f2sb", bufs=2) as sb, \
                 tc.tile_pool(name="f2ps", bufs=2, space="PSUM") as ps:
                for i in range(3):
                    ga = sb.tile([128, F], f32, name=f"f2ga_{i}", tag="f2ga")
                    gb = sb.tile([128, F], f32, name=f"f2gb_{i}", tag="f2gb")
                    nc.sync.dma_start(ga[:], AP(ag2o, i * 128 * F,
                                                [[F, 128], [1, F]]))
                    nc.sync.dma_start(gb[:], AP(ag2o, 3 * 128 * F + i * 128 * F,
                                                [[F, 128], [1, F]]))
                    gf = per.tile([128, F], f32, name=f"g2full_{i}")
                    nc.vector.tensor_tensor(gf[:], ga[:], gb[:], OP.add)
                    nsum = sb.tile([128, 1], f32, name=f"f2ns_{i}", tag="f2ns")
                    nc.vector.tensor_reduce(nsum[:], gf[:], AX.X, OP.add)
                    n64p = ps.tile([64, 1], f32, name=f"f2n64_{i}", tag="f2n64")
                    nc.tensor.matmul(n64p[:], cw["fold64"][:], nsum[:],
                                     start=True, stop=True)
                    n64 = sb.tile([64, 1], f32, name=f"f2n64s_{i}", tag="f2n64s")
                    nc.vector.tensor_copy(n64[:], n64p[:])
                    a_t, kc_t = _filt_fold(nc, sb, ps, per, cw, n64, i + 3,
                                           f32, bf, AF, OP, AX)
                    gt = per.tile([128, F], f32, name=f"gt2_{i}")
                    nc.vector.tensor_scalar(gt[:], gf[:],
                                            cw[f"kg_{i + 3}"][:, 0:1],
                                            kc_t[:, 0:1], OP.mult, OP.add)
                    gt2.append(gt)
                    ataps.append(a_t)
            dgv = []
            for i in range(3):
                for k in range(KT):
                    dt_ = per.tile([128, 128], bf, name=f"dgv_{i}_{k}")
                    nc.vector.tensor_scalar_mul(dt_[:], cw["ident_bf"][:],
                                                ataps[3 + i][:, k:k + 1])
                    dgv.append(dt_)

        if phases >= 3:
            # ============================ P3 ================================
            with tc.tile_pool(name="p3sb", bufs=3) as sb, \
                 tc.tile_pool(name="p3b2", bufs=2) as b2p, \
                 tc.tile_pool(name="p3ps", bufs=2, space="PSUM") as ps:
                for (t0, w) in _tiles(TH, 20):
                    x2 = x2_load(sb, PADR + t0, w, "p3x")
                    b2s = []
                    for i in range(3):
                        d = DILS[i]
                        n2w = b2p.tile([128, w + 14, F], bf, name=f"p3n2w_{i}",
                                       tag=f"p3n2w_{i}")
                        src = AP(n2_d, i * C * RP * F + t0 * F,
                                 [[w * F, 2], [RP * F, C], [F, w + 14], [1, F]])
                        nc.sync.dma_start(n2w[:], src)
                        n2f = n2w[:].rearrange("p a b -> p (a b)")
                        b2 = b2p.tile([128, w, F], bf, name=f"p3b2_{i}",
                                      tag=f"p3b2_{i}")
                        for (r0, rr) in _chunks(w):
                            n = rr * F
                            bps = ps.tile([128, RW * F], f32, name="p3bps",
                                          tag="p3bps")
                            for k in range(KT):
                                off = (PADR + r0 + (k - 1) * d) * F
                                nc.tensor.matmul(bps[:, :n], dgv[i * KT + k][:],
                                                 n2f[:, off:off + n],
                                                 start=(k == 0), stop=(k == 2))
                            nc.vector.tensor_tensor(
                                b2[:, r0:r0 + rr, :],
                                bps[:, :n].rearrange("p (a b) -> p a b", b=F),
                                gt2[i][:].unsqueeze(1).broadcast_to([128, rr, F]),
                                OP.add)
                        b2s.append(b2)
                    yt = sb.tile([128, w, F], bf, name="p3y", tag="p3y")
                    for (r0, rr) in _chunks(w):
                        n = rr * F
                        yps = ps.tile([128, RW * F], f32, name="p3yps",
                                      tag="p3yps")
                        nc.tensor.matmul(
                            yps[:, :n], cw["wbbd"][:],
                            x2[:, r0:r0 + rr, :].rearrange("p a b -> p (a b)"),
                            start=True, stop=False)
                        for i in range(3):
                            nc.tensor.matmul(
                                yps[:, :n], cw[f"wgbd_{i}"][:],
                                b2s[i][:, r0:r0 + rr, :]
                                .rearrange("p a b -> p (a b)"),
                                start=False, stop=(i == 2))
                        acc = sb.tile([128, 1], f32, name="p3acc", tag="p3acc")
                        nc.scalar.activation(
                            yt[:, r0:r0 + rr, :].rearrange("p a b -> p (a b)"),
                            yps[:, :n], AF.Copy, accum_out=acc[:])
                        nc.vector.tensor_tensor(ysacc[:], ysacc[:], acc[:],
                                                OP.add)
                        sqy = sb.tile([128, RW * F], f32, name="p3sqy",
                                      tag="p3sqy")
                        acq = sb.tile([128, 1], f32, name="p3acq", tag="p3acq")
                        nc.scalar.activation(
                            sqy[:, :n],
                            yt[:, r0:r0 + rr, :].rearrange("p a b -> p (a b)"),
                            AF.Square, accum_out=acq[:])
                        nc.vector.tensor_tensor(yqacc[:], yqacc[:], acq[:],
                                                OP.add)
                    dst = AP(y_d, t0 * F,
                             [[w * F, 2], [TH * F, C], [F, w], [1, F]])
                    nc.sync.dma_start(dst, yt[:])
            pk = per.tile([128, 2], f32, name="pk")
            nc.vector.tensor_copy(pk[:, 0:1], ysacc[:])
            nc.vector.tensor_copy(pk[:, 1:2], yqacc[:])
            nc.sync.dma_start(AP(ag3i, 0, [[1, 256]]), pk[:])
            nc.gpsimd.collective_compute(
                "AllGather", OP.bypass, replica_groups=groups,
                ins=[ag3i.ap()], outs=[ag3o.ap()])

            # GN scalars (y in y_d excludes convb_b; fold it analytically)
            gnS = per.tile([128, 1], f32, name="gnS")
            gnB = per.tile([128, 1], f32, name="gnB")
            with tc.tile_pool(name="gnsb", bufs=2) as sb, \
                 tc.tile_pool(name="gnps", bufs=2, space="PSUM") as ps:
                pa_ = sb.tile([128, 4], f32, name="gn_pa")
                nc.sync.dma_start(pa_[:, 0:2], AP(ag3o, 0, [[2, 128], [1, 2]]))
                nc.sync.dma_start(pa_[:, 2:4], AP(ag3o, 256, [[2, 128], [1, 2]]))
                sy = sb.tile([128, 1], f32, name="gn_sy")
                sq = sb.tile([128, 1], f32, name="gn_sq")
                nc.vector.tensor_tensor(sy[:], pa_[:, 0:1], pa_[:, 2:3], OP.add)
                nc.vector.tensor_tensor(sq[:], pa_[:, 1:2], pa_[:, 3:4], OP.add)
                NcF = float(TH * F)
                t1 = sb.tile([128, 1], f32, name="gn_t1")
                nc.vector.scalar_tensor_tensor(t1[:], sy[:], 2.0,
                                               cw["cbb"][:], OP.mult, OP.mult)
                nc.vector.tensor_tensor(sq[:], sq[:], t1[:], OP.add)
                cb2 = sb.tile([128, 1], f32, name="gn_cb2")
                nc.scalar.activation(cb2[:], cw["cbb"][:], AF.Square,
                                     scale=1.0)
                nc.vector.tensor_scalar(cb2[:], cb2[:], NcF, None, OP.mult)
                nc.vector.tensor_tensor(sq[:], sq[:], cb2[:], OP.add)
                nc.vector.scalar_tensor_tensor(t1[:], cw["cbb"][:], NcF, sy[:],
                                               OP.mult, OP.add)
                on1 = sb.tile([128, 1], f32, name="gn_on1")
                nc.vector.memset(on1[:], 1.0)
                tots = ps.tile([1, 2], f32, name="gn_tots")
                nc.tensor.matmul(tots[0:1, 0:1], on1[:], t1[:],
                                 start=True, stop=True)
                nc.tensor.matmul(tots[0:1, 1:2], on1[:], sq[:],
                                 start=True, stop=True)
                Ntot = float(C * T * F)
                mg = sb.tile([1, 1], f32, name="gn_mg")
                nc.vector.tensor_scalar(mg[:], tots[0:1, 0:1], 1.0 / Ntot, None,
                                        OP.mult)
                m2g = sb.tile([1, 1], f32, name="gn_m2g")
                nc.scalar.activation(m2g[:], mg[:], AF.Square)
                vg = sb.tile([1, 1], f32, name="gn_vg")
                nc.vector.scalar_tensor_tensor(vg[:], tots[0:1, 1:2], 1.0 / Ntot,
                                               m2g[:], OP.mult, OP.subtract)
                nc.vector.tensor_scalar(vg[:], vg[:], EPS, None, OP.add)
                ig = sb.tile([1, 1], f32, name="gn_ig")
                nc.scalar.activation(ig[:], vg[:], AF.Sqrt)
                nc.vector.reciprocal(ig[:], ig[:])
                igb = sb.tile([128, 1], f32, name="gn_igb")
                mgb = sb.tile([128, 1], f32, name="gn_mgb")
                nc.gpsimd.partition_broadcast(igb[:], ig[:])
                nc.gpsimd.partition_broadcast(mgb[:], mg[:])
                nc.vector.tensor_tensor(gnS[:], igb[:], cw["gn_g"][:], OP.mult)
                nc.vector.tensor_tensor(gnB[:], cw["cbb"][:], mgb[:],
                                        OP.subtract)
                nc.vector.tensor_tensor(gnB[:], gnB[:], gnS[:], OP.mult)
                nc.vector.tensor_tensor(gnB[:], gnB[:], cw["gn_b"][:], OP.add)

        if phases >= 4:
            # ============================ P4 ================================
            with tc.tile_pool(name="p4sb", bufs=2) as sb, \
                 tc.tile_pool(name="p4ps", bufs=2, space="PSUM") as ps, \
                 tc.tile_pool(name="p4st", bufs=1, space="PSUM") as pst:
                for (t0, w) in _tiles(TH, 20):
                    y2 = sb.tile([128, w, F], bf, name="p4y", tag="p4y")
                    nc.sync.dma_start(
                        y2[:], AP(y_d, t0 * F,
                                  [[w * F, 2], [TH * F, C], [F, w], [1, F]]))
                    ot = sb.tile([128, w, F], bf, name="p4o", tag="p4o")
                    nc.scalar.activation(ot[:], y2[:], AF.Prelu,
                                         bias=gnB[:, 0:1], scale=gnS[:, 0:1],
                                         alpha=cw["a0vec"][:, 0:1])
                    nc.sync.dma_start(
                        AP(out_d, t0 * F,
                           [[w * F, 2], [TH * F, C], [F, w], [1, F]]), ot[:])
                    qk = sb.tile([64, w, F], bf, name="p4qk", tag="p4qk")
                    vt = sb.tile([128, w, F], bf, name="p4v", tag="p4v")
                    for (r0, rr) in _chunks(w):
                        n = rr * F
                        oc = ot[:, r0:r0 + rr, :].rearrange("p a b -> p (a b)")
                        qps = ps.tile([64, RW * F], f32, name="p4qps",
                                      tag="p4qps")
                        vps = ps.tile([128, RW * F], f32, name="p4vps",
                                      tag="p4vps")
                        nc.tensor.matmul(qps[:, :n], cw["wqkbd"][:], oc,
                                         start=True, stop=True)
                        nc.tensor.matmul(vps[:, :n], cw["wvbd"][:], oc,
                                         start=True, stop=True)
                        nc.scalar.activation(
                            qk[:, r0:r0 + rr, :].rearrange("p a b -> p (a b)"),
                            qps[:, :n], AF.Prelu, bias=cw["qkb"][:, 0:1],
                            alpha=cw["qka"][:, 0:1])
                        nc.scalar.activation(
                            vt[:, r0:r0 + rr, :].rearrange("p a b -> p (a b)"),
                            vps[:, :n], AF.Prelu, bias=cw["vb"][:, 0:1],
                            alpha=cw["va"][:, 0:1])
                    qs = sb.tile([64, w], f32, name="p4qs", tag="p4qs")
                    vs = sb.tile([128, w], f32, name="p4vs", tag="p4vs")
                    nc.vector.tensor_reduce(qs[:], qk[:], AX.X, OP.add)
                    nc.vector.tensor_reduce(vs[:], vt[:], AX.X, OP.add)
                    qq = sb.tile([64, w, F], f32, name="p4qq", tag="p4qq")
                    vv = sb.tile([128, w, F], f32, name="p4vv", tag="p4vv")
                    nc.scalar.activation(qq[:], qk[:], AF.Square)
                    nc.scalar.activation(vv[:], vt[:], AF.Square)
                    qsq = sb.tile([64, w], f32, name="p4qsq", tag="p4qsq")
                    vsq = sb.tile([128, w], f32, name="p4vsq", tag="p4vsq")
                    nc.vector.tensor_reduce(qsq[:], qq[:], AX.X, OP.add)
                    nc.vector.tensor_reduce(vsq[:], vv[:], AX.X, OP.add)
                    stq = pst.tile([16, 2 * w], f32, name="p4stq", tag="p4stq")
                    stv = pst.tile([8, 2 * w], f32, name="p4stv", tag="p4stv")
                    nc.tensor.matmul(stq[:, 0:w], cw["grp16"][:], qs[:],
                                     start=True, stop=True)
                    nc.tensor.matmul(stq[:, w:2 * w], cw["grp16"][:], qsq[:],
                                     start=True, stop=True)
                    nc.tensor.matmul(stv[:, 0:w], cw["grp8"][:], vs[:],
                                     start=True, stop=True)
                    nc.tensor.matmul(stv[:, w:2 * w], cw["grp8"][:], vsq[:],
                                     start=True, stop=True)
                    mivs = {}
                    for (st, npart, ncnt, nm) in ((stq, 16, 4 * F, "q"),
                                                  (stv, 8, 16 * F, "v")):
                        mu = sb.tile([npart, w], f32, name=f"p4mu{nm}",
                                     tag=f"p4mu{nm}")
                        nc.vector.tensor_scalar(mu[:], st[:, 0:w], 1.0 / ncnt,
                                                None, OP.mult)
                        m2_ = sb.tile([npart, w], f32, name=f"p4m2{nm}",
                                      tag=f"p4m2{nm}")
                        nc.scalar.activation(m2_[:], mu[:], AF.Square)
                        var = sb.tile([npart, w], f32, name=f"p4var{nm}",
                                      tag=f"p4var{nm}")
                        nc.vector.scalar_tensor_tensor(var[:], st[:, w:2 * w],
                                                       1.0 / ncnt, m2_[:],
                                                       OP.mult, OP.subtract)
                        nc.vector.tensor_scalar(var[:], var[:], EPS, None,
                                                OP.add)
                        iv = sb.tile([npart, w], f32, name=f"p4iv{nm}",
                                     tag=f"p4iv{nm}")
                        nc.scalar.activation(iv[:], var[:], AF.Sqrt)
                        nc.vector.reciprocal(iv[:], iv[:])
                        mivs[nm] = (mu, iv)
                    qb_ = pst.tile([64, 2 * w], f32, name="p4qb", tag="p4qb")
                    vb_ = pst.tile([128, 2 * w], f32, name="p4vb", tag="p4vb")
                    nc.tensor.matmul(qb_[:, 0:w], cw["bc16"][:], mivs["q"][0][:],
                                     start=True, stop=True)
                    nc.tensor.matmul(qb_[:, w:2 * w], cw["bc16"][:],
                                     mivs["q"][1][:], start=True, stop=True)
                    nc.tensor.matmul(vb_[:, 0:w], cw["bc8"][:], mivs["v"][0][:],
                                     start=True, stop=True)
                    nc.tensor.matmul(vb_[:, w:2 * w], cw["bc8"][:],
                                     mivs["v"][1][:], start=True, stop=True)
                    qkn = sb.tile([64, w, F], bf, name="p4qkn", tag="p4qkn")
                    vn = sb.tile([128, w, F], bf, name="p4vn", tag="p4vn")
                    nc.vector.tensor_tensor(
                        qkn[:], qk[:],
                        qb_[:, 0:w].unsqueeze(2).broadcast_to([64, w, F]),
                        OP.subtract)
                    nc.vector.tensor_tensor(
                        qkn[:], qkn[:],
                        qb_[:, w:2 * w].unsqueeze(2).broadcast_to([64, w, F]),
                        OP.mult)
                    nc.vector.tensor_tensor(
                        vn[:], vt[:],
                        vb_[:, 0:w].unsqueeze(2).broadcast_to([128, w, F]),
                        OP.subtract)
                    nc.vector.tensor_tensor(
                        vn[:], vn[:],
                        vb_[:, w:2 * w].unsqueeze(2).broadcast_to([128, w, F]),
                        OP.mult)
                    nc.sync.dma_start(
                        AP(qkvh_d, t0 * F,
                           [[w * F, 2], [TH * F, 32], [F, w], [1, F]]), qkn[:])
                    nc.sync.dma_start(
                        AP(qkvh_d, 32 * TH * F + t0 * F,
                           [[w * F, 2], [TH * F, 64], [F, w], [1, F]]), vn[:])
            nc.gpsimd.collective_compute(
                "AllGather", OP.bypass, replica_groups=groups,
                ins=[qkvh_d.ap()], outs=[qkvf_d.ap()])

        if phases >= 5:
            # ============================ P5 ================================
            KCH = [(0, 128), (128, 128), (256, 128), (384, 128), (512, 4)]
            NCH = [(0, 512), (512, 512), (1024, 512), (1536, 512), (2048, 16)]
            import contextlib as _cl
            for h in range(H):
                hx = _cl.ExitStack()
                sb = hx.enter_context(tc.tile_pool(name=f"a{h}sb", bufs=3))
                res = hx.enter_context(tc.tile_pool(name=f"a{h}res", bufs=1))
                ps = hx.enter_context(tc.tile_pool(name=f"a{h}ps", bufs=2,
                                                   space="PSUM"))
                pss = hx.enter_context(tc.tile_pool(name=f"a{h}pss", bufs=1,
                                                    space="PSUM"))
                pso = hx.enter_context(tc.tile_pool(name=f"a{h}pso", bufs=2,
                                                    space="PSUM"))
                ktr = [res.tile([kn, T], bf, name=f"ktr{h}_{ci}")
                       for ci, (k0, kn) in enumerate(KCH)]
                qtr = [res.tile([kn, TH], bf, name=f"qtr{h}_{ci}")
                       for ci, (k0, kn) in enumerate(KCH)]
                vres = [res.tile([MB, DF], bf, name=f"vres{h}_{b}")
                        for b in range(8)]
                for b in range(8):
                    seg, tl = divmod(b, 4)
                    km = sb.tile([MB, EF], bf, name=f"km{h}", tag="km")
                    nc.sync.dma_start(
                        km[:], AP(qkvf_d,
                                  seg * 96 * TH * F + (16 + h * 4) * TH * F
                                  + tl * MB * F,
                                  [[F, MB], [TH * F, E], [1, F]]))
                    for ci, (k0, kn) in enumerate(KCH):
                        tp = ps.tile([128, MB], bf, name=f"tp{h}", tag="tp")
                        nc.tensor.transpose(tp[:kn, :], km[:, k0:k0 + kn],
                                            cw["ident_bf"][0:MB, 0:MB])
                        nc.vector.tensor_copy(ktr[ci][:, b * MB:(b + 1) * MB],
                                              tp[:kn, :])
                    nc.sync.dma_start(
                        vres[b][:], AP(qkvf_d,
                                       seg * 96 * TH * F + (32 + h * 16) * TH * F
                                       + tl * MB * F,
                                       [[F, MB], [TH * F, D], [1, F]]))
                for b in range(4):
                    km = sb.tile([MB, EF], bf, name=f"qm{h}", tag="km")
                    nc.sync.dma_start(
                        km[:], AP(qkvh_d, (h * 4) * TH * F + b * MB * F,
                                  [[F, MB], [TH * F, E], [1, F]]))
                    for ci, (k0, kn) in enumerate(KCH):
                        tp = ps.tile([128, MB], bf, name=f"tp{h}", tag="tp")
                        nc.tensor.transpose(tp[:kn, :], km[:, k0:k0 + kn],
                                            cw["ident_bf"][0:MB, 0:MB])
                        nc.vector.tensor_copy(qtr[ci][:, b * MB:(b + 1) * MB],
                                              tp[:kn, :])
                # lng/lnb per-partition post-transpose
                for ci, (k0, kn) in enumerate(KCH):
                    for (lg, lb, tt) in ((("klngT"), ("klnbT"), ktr),
                                         (("qlngT"), ("qlnbT"), qtr)):
                        vg_ = sb.tile([128, 1], f32, name=f"lg{h}", tag="lg")
                        vb2 = sb.tile([128, 1], f32, name=f"lb{h}", tag="lb")
                        nc.sync.dma_start(vg_[:kn, :],
                                          AP(fw[lg], h * 640 + k0, [[1, kn]]))
                        nc.sync.dma_start(vb2[:kn, :],
                                          AP(fw[lb], h * 640 + k0, [[1, kn]]))
                        nc.vector.tensor_scalar(tt[ci][:], tt[ci][:],
                                                vg_[:kn, 0:1], vb2[:kn, 0:1],
                                                OP.mult, OP.add)
                lngb = res.tile([MB, DF], f32, name=f"lngb{h}")
                lrow = sb.tile([1, DF], f32, name=f"lrow{h}", tag="lrow")
                nc.sync.dma_start(lrow[:], AP(fw["vlngT"], h * DF, [[DF, 1],
                                                                   [1, DF]]))
                nc.gpsimd.partition_broadcast(lngb[:], lrow[:])
                for mt in range(4):
                    spt = pss.tile([MB, 1024], f32, name=f"spt{h}", tag="spt")
                    for ci, (k0, kn) in enumerate(KCH):
                        lhs = qtr[ci][:, mt * MB:(mt + 1) * MB]
                        nc.tensor.matmul(spt[:, 0:500], lhs, ktr[ci][:, 0:500],
                                         start=(ci == 0), stop=(ci == 4))
                        nc.tensor.matmul(spt[:, 512:1012], lhs,
                                         ktr[ci][:, 500:1000],
                                         start=(ci == 0), stop=(ci == 4))
                    sview = spt[:].rearrange("p (a b) -> p a b", b=512)[:, :, 0:500]
                    mx = sb.tile([MB, 1], f32, name=f"mx{h}", tag="mx")
                    nc.vector.tensor_reduce(mx[:], sview, AX.XY, OP.max)
                    bias = sb.tile([MB, 1], f32, name=f"bias{h}", tag="bias")
                    nc.vector.tensor_scalar(bias[:], mx[:], -ISCALE, None,
                                            OP.mult)
                    pexp = sb.tile([MB, 1000], bf, name=f"pexp{h}", tag="pexp")
                    lsum = sb.tile([MB, 1], f32, name=f"lsum{h}", tag="lsum")
                    nc.scalar.activation(pexp[:], sview, AF.Exp,
                                         bias=bias[:, 0:1], scale=ISCALE,
                                         accum_out=lsum[:])
                    linv = sb.tile([MB, 1], f32, name=f"linv{h}", tag="linv")
                    nc.vector.reciprocal(linv[:], lsum[:])
                    ptr = []
                    for b in range(8):
                        tp2 = ps.tile([MB, MB], bf, name=f"tp2{h}", tag="tp2")
                        nc.tensor.transpose(tp2[:], pexp[:, b * MB:(b + 1) * MB],
                                            cw["ident_bf"][0:MB, 0:MB])
                        pb_ = sb.tile([MB, MB], bf, name=f"ptr{h}_{b}",
                                      tag=f"ptr{b}")
                        nc.vector.tensor_copy(pb_[:], tp2[:])
                        ptr.append(pb_)
                    ob = sb.tile([MB, DF], bf, name=f"ob{h}", tag="ob")
                    for (n0, nn) in NCH:
                        op_ = pso.tile([MB, 512], f32, name=f"op{h}", tag="op")
                        for b in range(8):
                            nc.tensor.matmul(op_[:, :nn], ptr[b][:],
                                             vres[b][:, n0:n0 + nn],
                                             start=(b == 0), stop=(b == 7))
                        nc.vector.scalar_tensor_tensor(
                            ob[:, n0:n0 + nn], op_[:, :nn], linv[:, 0:1],
                            lngb[:, n0:n0 + nn], OP.mult, OP.mult)
                    nc.sync.dma_start(
                        AP(o_d, mt * MB * H * D * F + h * D * F,
                           [[H * D * F, MB], [1, D * F]]), ob[:])
                hx.close()

        if phases >= 6:
            # ============================ P6 ================================
            with tc.tile_pool(name="p6sb", bufs=2) as sb, \
                 tc.tile_pool(name="p6ps", bufs=2, space="PSUM") as ps, \
                 tc.tile_pool(name="p6st", bufs=1, space="PSUM") as pst:
                for (t0, w) in _tiles(TH, 20):
                    o2 = sb.tile([128, w, F], bf, name="p6o", tag="p6o")
                    for q in range(2):
                        nc.sync.dma_start(
                            o2[q * 64:(q + 1) * 64],
                            AP(o_d, (t0 + q * w) * H * D * F,
                               [[F, 64], [H * D * F, w], [1, F]]))
                    u2 = sb.tile([128, w, F], f32, name="p6u2", tag="p6u2")
                    for (r0, rr) in _chunks(w):
                        n = rr * F
                        pps = ps.tile([128, RW * F], f32, name="p6pps",
                                      tag="p6pps")
                        nc.tensor.matmul(
                            pps[:, :n], cw["pwbd"][:],
                            o2[:, r0:r0 + rr, :].rearrange("p a b -> p (a b)"),
                            start=True, stop=True)
                        tmp = sb.tile([128, RW, F], f32, name="p6tmp",
                                      tag="p6tmp")
                        nc.vector.tensor_tensor(
                            tmp[:, :rr, :],
                            pps[:, :n].rearrange("p (a b) -> p a b", b=F),
                            cw["pconst"][:].unsqueeze(1)
                            .broadcast_to([128, rr, F]),
                            OP.add)
                        nc.scalar.activation(
                            u2[:, r0:r0 + rr, :].rearrange("p a b -> p (a b)"),
                            tmp[:, :rr, :].rearrange("p a b -> p (a b)"),
                            AF.Prelu, bias=cw["pb"][:, 0:1],
                            alpha=cw["pa"][:, 0:1])
                    us = sb.tile([128, w], f32, name="p6us", tag="p6us")
                    nc.vector.tensor_reduce(us[:], u2[:], AX.X, OP.add)
                    uq = sb.tile([128, w, F], f32, name="p6uq", tag="p6uq")
                    nc.scalar.activation(uq[:], u2[:], AF.Square)
                    usq = sb.tile([128, w], f32, name="p6usq", tag="p6usq")
                    nc.vector.tensor_reduce(usq[:], uq[:], AX.X, OP.add)
                    st2 = pst.tile([2, 2 * w], f32, name="p6st2", tag="p6st2")
                    nc.tensor.matmul(st2[:, 0:w], cw["redq"][:], us[:],
                                     start=True, stop=True)
                    nc.tensor.matmul(st2[:, w:2 * w], cw["redq"][:], usq[:],
                                     start=True, stop=True)
                    ncnt = float(64 * F)
                    mu2 = sb.tile([2, w], f32, name="p6mu2", tag="p6mu2")
                    nc.vector.tensor_scalar(mu2[:], st2[:, 0:w], 1.0 / ncnt,
                                            None, OP.mult)
                    m22 = sb.tile([2, w], f32, name="p6m22", tag="p6m22")
                    nc.scalar.activation(m22[:], mu2[:], AF.Square)
                    var2 = sb.tile([2, w], f32, name="p6var2", tag="p6var2")
                    nc.vector.scalar_tensor_tensor(var2[:], st2[:, w:2 * w],
                                                   1.0 / ncnt, m22[:],
                                                   OP.mult, OP.subtract)
                    nc.vector.tensor_scalar(var2[:], var2[:], EPS, None, OP.add)
                    iv2 = sb.tile([2, w], f32, name="p6iv2", tag="p6iv2")
                    nc.scalar.activation(iv2[:], var2[:], AF.Sqrt)
                    nc.vector.reciprocal(iv2[:], iv2[:])
                    mb2 = pst.tile([128, 2 * w], f32, name="p6mb2", tag="p6mb2")
                    nc.tensor.matmul(mb2[:, 0:w], cw["ind2f"][:], mu2[:],
                                     start=True, stop=True)
                    nc.tensor.matmul(mb2[:, w:2 * w], cw["ind2f"][:], iv2[:],
                                     start=True, stop=True)
                    nc.vector.tensor_tensor(
                        u2[:], u2[:],
                        mb2[:, 0:w].unsqueeze(2).broadcast_to([128, w, F]),
                        OP.subtract)
                    nc.vector.tensor_tensor(
                        u2[:], u2[:],
                        mb2[:, w:2 * w].unsqueeze(2).broadcast_to([128, w, F]),
                        OP.mult)
                    nc.vector.tensor_tensor(
                        u2[:], u2[:],
                        cw["plng"][:].unsqueeze(1).broadcast_to([128, w, F]),
                        OP.mult)
                    rt = sb.tile([128, w, F], bf, name="p6rt", tag="p6rt")
                    nc.sync.dma_start(
                        rt[:], AP(out_d, t0 * F,
                                  [[w * F, 2], [TH * F, C], [F, w], [1, F]]))
                    r1 = sb.tile([128, w, F], f32, name="p6r1", tag="p6r1")
                    nc.vector.tensor_tensor(
                        r1[:], rt[:],
                        cw["plnb"][:].unsqueeze(1).broadcast_to([128, w, F]),
                        OP.add)
                    fint = sb.tile([128, w, F], f32, name="p6fin", tag="p6fin")
                    nc.vector.tensor_tensor(fint[:], u2[:], r1[:], OP.add)
                    nc.sync.dma_start(
                        AP(fin, t0 * F,
                           [[w * F, 2], [TH * F, C], [F, w], [1, F]]), fint[:])
        if phases < 6:
            with tc.tile_pool(name="dummy", bufs=1) as dp:
                zt = dp.tile([C, 16], mybir.dt.float32, name="zfin")
                nc.vector.memset(zt[:], 0.0)
                nc.sync.dma_start(AP(fin, 0, [[TH * F, C], [1, 16]]), zt[:])
        ctx.close()
    nc.compile()
    return nc


def _filt_fold(nc, sb, ps, per, cw, mean64, i, f32, bf, AF, OP, AX):
    u1 = sb.tile([64, 1], f32, name=f"u1_{i}", tag="ffu1")
    nc.vector.tensor_scalar(u1[:], mean64[:], cw[f"gtf_{i}"][:, 0:1],
                            cw[f"c64_{i}"][:, 0:1], OP.mult, OP.add)
    ftp = ps.tile([1, 12], f32, name=f"ftp_{i}", tag="ffftp")
    nc.tensor.matmul(ftp[:], u1[:], cw[f"lwT_{i}"][:], start=True, stop=True)
    ft = sb.tile([1, 12], f32, name=f"ft_{i}", tag="ffft")
    nc.scalar.activation(ft[:], ftp[:], AF.Tanh)
    ft4 = sb.tile([4, 3], f32, name=f"ft4_{i}", tag="ffft4")
    nc.sync.dma_start(ft4[:], ft[:].rearrange("o (g k) -> (o g) k", g=4))
    wcp = ps.tile([128, 3], f32, name=f"wcp_{i}", tag="ffwcp")
    nc.tensor.matmul(wcp[:], cw["grp4"][:], ft4[:], start=True, stop=True)
    atap = per.tile([128, 3], f32, name=f"atap_{i}")
    nc.vector.tensor_scalar_mul(atap[:], wcp[:], cw[f"gs_{i}"][:, 0:1])
    nc.vector.tensor_tensor(atap[:, 1:2], atap[:, 1:2],
                            cw[f"hc_{i}"][:, 0:1], OP.add)
    wcs = sb.tile([128, 1], f32, name=f"wcs_{i}", tag="ffwcs")
    nc.vector.tensor_reduce(wcs[:], wcp[:], AX.X, OP.add)
    kc = per.tile([128, 1], f32, name=f"kc_{i}")
    nc.vector.tensor_scalar(kc[:], wcs[:], cw[f"cs_{i}"][:, 0:1],
                            cw[f"cb_{i}"][:, 0:1], OP.mult, OP.add)
    nc.vector.tensor_tensor(kc[:], kc[:], cw[f"gc_{i}"][:, 0:1], OP.add)
    return atap, kc


# ---------------------------------------------------------------------------
# host entry
# ---------------------------------------------------------------------------

def _prep_inputs(inputs, fold):
    import ml_dtypes
    x = np.asarray(inputs["x"], np.float32)
    in_maps = []
    for c in range(8):
        s, hf = divmod(c, 2)
        xs = np.pad(x[s], ((0, 0), (PADR, PADR), (0, 0)), mode="reflect")
        xc = xs[:, hf * TH:hf * TH + RP, :]
        m = {"x_d": np.ascontiguousarray(xc).astype(ml_dtypes.bfloat16)}
        for n, (sh, isbf) in _fold_shapes().items():
            v = fold[n]
            m[n] = v.astype(ml_dtypes.bfloat16) if isbf else v
        for n in _dram_only_shapes():
            m[n] = fold[n]
        in_maps.append(m)
    return in_maps


def kernel(**inputs):
    import os
    from concourse.bass_utils import run_bass_kernel_spmd
    global LAST_EXEC_NS
    if "nc" not in _CACHE:
        _CACHE["nc"] = _build(dbg=_CACHE.get("dbg", ()),
                              phases=int(os.environ.get("KPHASES", "6")))
    nc = _CACHE["nc"]
    fold = _fold_weights(inputs)
    in_maps = _prep_inputs(inputs, fold)
    kw = {}
    if os.environ.get("KTRACE"):
        tdir = os.environ.get("KTRACE_DIR",
                              os.path.join(os.getcwd(), "work", "trace"))
        os.makedirs(tdir, exist_ok=True)
        tc_ = os.environ.get("KTRACE_CORES", "0")
        kw = dict(trace=True, tmpdir=tdir,
                  trace_cores=[int(c) for c in tc_.split(",")])
    res = run_bass_kernel_spmd(nc, in_maps, core_ids=list(range(8)), **kw)
    _CACHE["last"] = res
    if getattr(res, "exec_time_ns", None):
        LAST_EXEC_NS = res.exec_time_ns
    out = np.zeros((B, C, T, F), np.float32)
    for c in range(8):
        s, hf = divmod(c, 2)
        out[s][:, hf * TH:(hf + 1) * TH, :] = res.results[c]["fin"]
    return out



# revision 28
# speedup vs baseline: 1.2614x; 1.2614x over previous
"""GridNetBlock_Att Trainium2 kernel (Bass/Tile, 8 NeuronCores).

Core c handles sample s=c//2, T-half h=c%2 (rows [500h, 500h+500)).
Pre-attention is T-split per core (host supplies x with a reflect-padded
t-halo of 7 rows, which makes the SPMD program identical on all cores);
tiny pair AllGathers combine global stats, and one pair AllGather
exchanges the K/V halves before full-sequence attention.

Pre-attention tiles are "2-stack": 128 partitions = 2 consecutive
row-blocks x 64 channels.  LN-over-channels per psum chunk:
    w1  = (I - BO/64) @ x        (PE; BO = block-ones)    = x - mu
    sqw = Square(w1)             (ACT, psum->sbuf)
    s1  = BO @ sqw               (PE)                     = 64*var
    inv = AbsRecipSqrt(s1/64)    (ACT)                    = 1/sqrt(var)
    z   = w1 * inv               (DVE, psum x sbuf -> bf16)
"""
import time

import numpy as np

EPS = 1e-5
B, C, T, F = 4, 64, 1000, 129
H, E, D = 4, 4, 16
GROUP, KT = 4, 3
DILS = (3, 5, 7)
EF, DF = E * F, D * F      # 516, 2064
TH = T // 2                 # 500 local rows
PADR = 7
RP = TH + 2 * PADR          # 514
FP = F + 14                 # 143
TF = float(T * F)
RW = 3                      # rows per psum chunk
ISCALE = float(1.0 / np.sqrt(EF))
MB = 125                    # attention row block

_CACHE = {}
LAST_EXEC_NS = -1


def _tiles(total, w):
    out, t0 = [], 0
    while t0 < total:
        ww = min(2 * w, total - t0) // 2
        out.append((t0, ww))
        t0 += 2 * ww
    return out


def _chunks(w, rw=RW):
    return [(ci * rw, min(rw, w - ci * rw)) for ci in range((w + rw - 1) // rw)]


# ---------------------------------------------------------------------------
# host-side weight folding
# ---------------------------------------------------------------------------

def _fold_shapes():
    sh = {
        "bo_f": ((128, 128), False),
        "w1m_bf": ((128, 128), True),
        "ind2f": ((2, 128), False), "ident_bf": ((128, 128), True),
        "fold64": ((128, 64), False), "redq": ((128, 2), False),
        "grp4": ((4, 128), False),
        "wbbd": ((128, 128), True), "cbb": ((128, 1), False),
        "gn_g": ((128, 1), False), "gn_b": ((128, 1), False),
        "a0vec": ((128, 1), False),
        "wqkbd": ((128, 64), True), "wvbd": ((128, 128), True),
        "qkb": ((64, 1), False), "qka": ((64, 1), False),
        "vb": ((128, 1), False), "va": ((128, 1), False),
        "grp16": ((64, 16), False), "bc16": ((16, 64), False),
        "grp8": ((128, 8), False), "bc8": ((8, 128), False),
        "pwbd": ((128, 128), True), "pconst": ((128, F), False),
        "pb": ((128, 1), False), "pa": ((128, 1), False),
        "plng": ((128, F), False), "plnb": ((128, F), False),
    }
    for i in range(6):
        for nm in ("gs", "hc", "kg", "cs", "cb", "gc"):
            sh[f"{nm}_{i}"] = ((128, 1), False)
        sh[f"lwT_{i}"] = ((64, 12), False)
        sh[f"gtf_{i}"] = ((64, 1), False)
        sh[f"c64_{i}"] = ((64, 1), False)
    for i in range(3):
        sh[f"wgbd_{i}"] = ((128, 128), True)
    return sh


# loaded from DRAM on demand, not staged in SBUF constants
def _dram_only_shapes():
    return {
        "qlngT": (H, 640), "klngT": (H, 640),
        "qlnbT": (H, 640), "klnbT": (H, 640),
        "vlngT": (H, DF),
    }


def _fold_weights(w):
    f32 = np.float32
    g = {}
    ar = lambda a: np.ascontiguousarray(a, f32)
    dup = lambda v: np.tile(ar(v).reshape(64), 2).reshape(128, 1)

    bo = np.zeros((128, 128), f32)
    bo[:64, :64] = 1.0
    bo[64:, 64:] = 1.0
    g["bo_f"] = bo
    g["w1m_bf"] = np.eye(128, dtype=f32) - bo / 64.0
    ind2 = np.zeros((2, 128), f32)
    ind2[0, :64] = 1.0
    ind2[1, 64:] = 1.0
    g["ind2f"] = ind2
    g["ident_bf"] = np.eye(128, dtype=f32)
    fold2 = np.zeros((128, 64), f32)
    for p in range(128):
        fold2[p, p % 64] = 1.0
    g["fold64"] = fold2
    redq = np.zeros((128, 2), f32)
    redq[:64, 0] = 1.0
    redq[64:, 1] = 1.0
    g["redq"] = redq
    gi4 = np.zeros((4, 128), f32)
    for p in range(128):
        gi4[(p % 64) // 16, p] = 1.0
    g["grp4"] = gi4

    for i in range(6):
        gg = w["br_g"][i].astype(f32)
        cc = w["br_b"][i].astype(f32)
        ia = w["lisa_in"][i].astype(f32)
        ll = w["lisa_ll"][i].astype(f32)
        lh = w["lisa_lh"][i].astype(f32)
        s = (ia + 1.0) * ll
        gap_div = float(F) if i < 3 else float(T)
        g[f"gs_{i}"] = dup(gg * s)
        g[f"hc_{i}"] = dup(gg * (lh + 1.0))
        g[f"kg_{i}"] = dup((-ia * ll * gg) / gap_div)
        g[f"gc_{i}"] = dup(-ia * ll * cc)
        g[f"cs_{i}"] = dup(cc * s)
        g[f"cb_{i}"] = dup(cc * (lh + 1.0))
        g[f"lwT_{i}"] = ar(w["lisa_w"][i].T)
        g[f"gtf_{i}"] = ar((gg / TF).reshape(64, 1))
        g[f"c64_{i}"] = ar(cc.reshape(64, 1))

    cw_ = w["convb_w"].astype(f32)
    gam = w["mix_gamma"].astype(f32)
    bet = w["mix_beta"].astype(f32)

    def bd(m, no):
        z = np.zeros((128, 2 * no), f32)
        z[:64, :no] = m.T
        z[64:, no:] = m.T
        return z

    for i in range(3):
        g[f"wgbd_{i}"] = bd(cw_ * gam[i][None, :], 64)
    g["wbbd"] = bd(cw_ * bet.sum(0)[None, :], 64)
    g["cbb"] = dup(w["convb_b"])
    g["gn_g"] = dup(w["gn_g"])
    g["gn_b"] = dup(w["gn_b"])
    g["a0vec"] = np.full((128, 1), float(w["convb_a"]), f32)

    wqk = np.concatenate([w["q_w"].astype(f32).reshape(H * E, C),
                          w["k_w"].astype(f32).reshape(H * E, C)], 0)
    g["wqkbd"] = bd(wqk, 32)
    g["wvbd"] = bd(w["v_w"].astype(f32).reshape(H * D, C), 64)
    qkb = np.concatenate([w["q_b"].reshape(-1), w["k_b"].reshape(-1)])
    g["qkb"] = np.tile(ar(qkb), 2).reshape(64, 1)
    qka = np.concatenate([np.repeat(w["q_a"], E), np.repeat(w["k_a"], E)])
    g["qka"] = np.tile(ar(qka), 2).reshape(64, 1)
    g["vb"] = np.tile(ar(w["v_b"].reshape(-1)), 2).reshape(128, 1)
    g["va"] = np.tile(ar(np.repeat(w["v_a"], D)), 2).reshape(128, 1)
    g16 = np.zeros((64, 16), f32)
    for p in range(64):
        q, j = divmod(p, 32)
        g16[p, q * 8 + (j // 16) * 4 + (j % 16) // 4] = 1.0
    g["grp16"] = g16
    g["bc16"] = ar(g16.T)
    g8 = np.zeros((128, 8), f32)
    for p in range(128):
        q, j = divmod(p, 64)
        g8[p, q * 4 + j // 16] = 1.0
    g["grp8"] = g8
    g["bc8"] = ar(g8.T)
    for nm, src in (("qlngT", "q_lng"), ("klngT", "k_lng"),
                    ("qlnbT", "q_lnb"), ("klnbT", "k_lnb")):
        m = np.zeros((H, 640), f32)
        for h in range(H):
            m[h, :EF] = w[src][h].reshape(EF)
        g[nm] = m
    g["vlngT"] = ar(w["v_lng"].reshape(H, DF))
    pw = w["proj_w"].astype(f32)
    g["pwbd"] = bd(pw, 64)
    pconst = pw @ w["v_lnb"].reshape(H * D, F).astype(f32)
    g["pconst"] = np.tile(pconst, (2, 1)).reshape(128, F)
    g["pb"] = dup(w["proj_b"])
    g["pa"] = np.full((128, 1), float(w["proj_a"]), f32)
    g["plng"] = np.tile(w["proj_lng"].astype(f32), (2, 1)).reshape(128, F)
    g["plnb"] = np.tile(w["proj_lnb"].astype(f32), (2, 1)).reshape(128, F)
    return g


# ---------------------------------------------------------------------------
# device program
# ---------------------------------------------------------------------------

def _build(dbg=(), phases=6):
    import concourse.bass as bass
    import concourse.bacc as bacc
    import concourse.mybir as mybir
    from concourse import tile
    from contextlib import ExitStack

    f32 = mybir.dt.float32
    bf = mybir.dt.bfloat16
    AF = mybir.ActivationFunctionType
    OP = mybir.AluOpType
    AX = mybir.AxisListType

    nc = bacc.Bacc("TRN2", target_bir_lowering=False, debug=False,
                   num_devices=8)

    def AP(tensor, offset, dims):
        return bass.AP(tensor=tensor, offset=offset,
                       ap=[list(d) for d in dims])

    shapes = _fold_shapes()
    dshapes = _dram_only_shapes()
    x_d = nc.dram_tensor("x_d", [C, RP, F], bf, kind="ExternalInput")
    fw = {n: nc.dram_tensor(n, list(s), bf if b else f32,
                            kind="ExternalInput")
          for n, (s, b) in shapes.items()}
    for n, s in dshapes.items():
        fw[n] = nc.dram_tensor(n, list(s), f32, kind="ExternalInput")

    def idram(name, shape, dt_):
        kind = "ExternalOutput" if name in dbg else "Internal"
        return nc.dram_tensor(name, list(shape), dt_, kind=kind)

    n2_d = idram("n2_d", [3, C, RP, F], bf)
    y_d = idram("y_d", [C, TH, F], bf)
    out_d = idram("out_d", [C, TH, F], bf)
    qkvh_d = idram("qkvh_d", [96, TH, F], bf)
    qkvf_d = idram("qkvf_d", [2, 80, TH, F], bf)
    o_d = idram("o_d", [TH, H, D, F], bf)
    b1_d = idram("b1_d", [3, C, RP, F], bf) if "b1_d" in dbg else None
    dsm_d = idram("dsm_d", [16, 128], f32) if "dsm_d" in dbg else None
    ag1i = nc.dram_tensor("ag1i", [1, 128], f32)
    ag1o = nc.dram_tensor("ag1o", [2, 128], f32)
    ag2i = nc.dram_tensor("ag2i", [1, 3 * 128 * F], f32)
    ag2o = nc.dram_tensor("ag2o", [2, 3 * 128 * F], f32)
    ag3i = nc.dram_tensor("ag3i", [1, 256], f32)
    ag3o = nc.dram_tensor("ag3o", [2, 256], f32)
    fin = nc.dram_tensor("fin", [C, TH, F], bf, kind="ExternalOutput")

    groups = [[0, 1], [2, 3], [4, 5], [6, 7]]

    with nc.allow_low_precision(reason="bf16 pipeline, tol 2e-2"), \
         tile.TileContext(nc) as tc:
        ctx = ExitStack()
        cst = ctx.enter_context(tc.tile_pool(name="cst", bufs=1))
        per = ctx.enter_context(tc.tile_pool(name="per", bufs=1))

        def load_const(name):
            sh, isbf = shapes[name]
            t = cst.tile(list(sh), bf if isbf else f32, name=f"c_{name}",
                         tag=f"c_{name}")
            nc.sync.dma_start(t[:], fw[name].ap())
            return t

        cw = {n: load_const(n) for n in shapes}

        def x2_load(pool, t0, w, nm):
            xt = pool.tile([128, w, F], bf, name=nm, tag=nm)
            src = AP(x_d, t0 * F,
                     [[w * F, 2], [RP * F, C], [F, w], [1, F]])
            nc.sync.dma_start(xt[:], src)
            return xt

        def ln_chunk(sb, ps, src_flat, n, nm):
            w1 = ps.tile([128, RW * FP], f32, name=f"{nm}w1", tag="Lw1")
            s1 = ps.tile([128, RW * FP], f32, name=f"{nm}s1", tag="Ls1")
            nc.tensor.matmul(w1[:, :n], cw["w1m_bf"][:], src_flat,
                             start=True, stop=True)
            sqw = sb.tile([128, RW * FP], f32, name=f"{nm}sqw", tag="Lsq")
            nc.scalar.activation(sqw[:, :n], w1[:, :n], AF.Square)
            nc.tensor.matmul(s1[:, :n], cw["bo_f"][:], sqw[:, :n],
                             start=True, stop=True)
            inv = sb.tile([128, RW * FP], f32, name=f"{nm}inv", tag="Linv")
            nc.scalar.activation(inv[:, :n], s1[:, :n], AF.Abs_reciprocal_sqrt,
                                 scale=1.0 / 64.0)
            return w1, inv

        # persistent accumulators
        macc = per.tile([128, 1], f32, name="macc")
        nc.vector.memset(macc[:], 0.0)
        g2acc = [per.tile([128, F], f32, name=f"g2acc_{i}") for i in range(3)]
        for i in range(3):
            nc.vector.memset(g2acc[i][:], 0.0)
        ysacc = per.tile([128, 1], f32, name="ysacc")
        yqacc = per.tile([128, 1], f32, name="yqacc")
        nc.vector.memset(ysacc[:], 0.0)
        nc.vector.memset(yqacc[:], 0.0)

        # ============================ P1 ================================
        with tc.tile_pool(name="p1sb", bufs=3) as sb, \
             tc.tile_pool(name="p1ps", bufs=2, space="PSUM") as ps:
            for (t0, w) in _tiles(TH, 24):
                x2 = x2_load(sb, PADR + t0, w, "p1x")
                for (r0, rr) in _chunks(w):
                    n = rr * F
                    xc = x2[:, r0:r0 + rr, :].rearrange("p a b -> p (a b)")
                    w1, inv = ln_chunk(sb, ps, xc, n, "p1")
                    junk = sb.tile([128, RW * F], bf, name="p1junk",
                                   tag="p1junk")
                    acc = sb.tile([128, 1], f32, name="p1acc", tag="p1acc")
                    nc.vector.scalar_tensor_tensor(
                        junk[:, :n], w1[:, :n], 1.0, inv[:, :n],
                        OP.mult, OP.mult, accum_out=acc[:])
                    nc.vector.tensor_tensor(macc[:], macc[:], acc[:], OP.add)
        nc.sync.dma_start(AP(ag1i, 0, [[1, 128]]), macc[:])
        nc.gpsimd.collective_compute(
            "AllGather", OP.bypass, replica_groups=groups,
            ins=[ag1i.ap()], outs=[ag1o.ap()])
        m_a = per.tile([128, 2], f32, name="m_a")
        nc.sync.dma_start(m_a[:], AP(ag1o, 0, [[1, 128], [128, 2]]))
        mtot = per.tile([128, 1], f32, name="mtot")
        nc.vector.tensor_tensor(mtot[:], m_a[:, 0:1], m_a[:, 1:2], OP.add)

        # filt folds, horizontal stages
        ataps, kcv = [], []
        with tc.tile_pool(name="ffsb", bufs=2) as sb, \
             tc.tile_pool(name="ffps", bufs=2, space="PSUM") as ps:
            m64p = ps.tile([64, 1], f32, name="m64p")
            nc.tensor.matmul(m64p[:], cw["fold64"][:], mtot[:],
                             start=True, stop=True)
            m64 = per.tile([64, 1], f32, name="m64")
            nc.vector.tensor_copy(m64[:], m64p[:])
            for i in range(3):
                a_t, kc_t = _filt_fold(nc, sb, ps, per, cw, m64, i,
                                       f32, bf, AF, OP, AX)
                ataps.append(a_t)
                kcv.append(kc_t)
        if dsm_d is not None:
            nc.sync.dma_start(AP(dsm_d, 0, [[1, 128]]), mtot[:])
            for i in range(3):
                for k in range(KT):
                    nc.sync.dma_start(
                        AP(dsm_d, (1 + i * 3 + k) * 128, [[1, 128]]),
                        ataps[i][:, k:k + 1])
                nc.sync.dma_start(AP(dsm_d, (10 + i) * 128, [[1, 128]]),
                                  kcv[i][:, 0:1])
        dgh = []
        for i in range(3):
            for k in range(KT):
                dt_ = per.tile([128, 128], bf, name=f"dgh_{i}_{k}")
                nc.vector.tensor_scalar_mul(dt_[:], cw["ident_bf"][:],
                                            ataps[i][:, k:k + 1])
                dgh.append(dt_)

        if phases >= 2:
            # ============================ P2 ================================
            with tc.tile_pool(name="p2sb", bufs=3) as sb, \
                 tc.tile_pool(name="p2big", bufs=2) as big, \
                 tc.tile_pool(name="p2ps", bufs=2, space="PSUM") as ps, \
                 tc.tile_pool(name="p2pst", bufs=2, space="PSUM") as pst:
                for (t0, w) in _tiles(RP, 20):
                    x2 = x2_load(sb, t0, w, "p2x")
                    zzt = big.tile([128, 8 + w * FP + 8], bf, name="p2zzt",
                                   tag="p2zzt")
                    zz = zzt[:, 8:8 + w * FP].rearrange("p (a b) -> p a b", b=FP)
                    nc.vector.memset(zzt[:, 0:8], 0.0)
                    nc.vector.memset(zzt[:, 8 + w * FP:8 + w * FP + 8], 0.0)
                    for (r0, rr) in _chunks(w):
                        n = rr * F
                        xc = x2[:, r0:r0 + rr, :].rearrange("p a b -> p (a b)")
                        w1, inv = ln_chunk(sb, ps, xc, n, "p2a")
                        nc.vector.tensor_tensor(
                            zz[:, r0:r0 + rr, 7:7 + F],
                            w1[:, :n].rearrange("p (a b) -> p a b", b=F),
                            inv[:, :n].rearrange("p (a b) -> p a b", b=F),
                            OP.mult)
                    nc.vector.tensor_copy(zz[:, :, 0:7], zz[:, :, 14:7:-1])
                    nc.vector.tensor_copy(zz[:, :, 136:143], zz[:, :, 134:127:-1])
                    gpf = sb.tile([128, w], f32, name="p2gpf", tag="p2gpf")
                    nc.vector.tensor_reduce(gpf[:], zz[:, :, 7:7 + F], AX.X,
                                            OP.add)
                    zzf = zzt[:]
                    for i in range(3):
                        d = DILS[i]
                        grow = sb.tile([128, w], f32, name="p2grow", tag="p2grow")
                        nc.vector.tensor_scalar(grow[:], gpf[:],
                                                cw[f"kg_{i}"][:, 0:1],
                                                kcv[i][:, 0:1], OP.mult, OP.add)
                        b1 = big.tile([128, w, FP], bf, name="p2b1", tag="p2b1")
                        for (r0, rr) in _chunks(w):
                            n = rr * FP
                            bps = pst.tile([128, RW * FP], f32, name="p2bps",
                                           tag="p2bps")
                            for k in range(KT):
                                off = 8 + r0 * FP + (k - 1) * d
                                nc.tensor.matmul(bps[:, :n], dgh[i * KT + k][:],
                                                 zzf[:, off:off + n],
                                                 start=(k == 0), stop=(k == 2))
                            nc.vector.tensor_tensor(
                                b1[:, r0:r0 + rr, :],
                                bps[:, :n].rearrange("p (a b) -> p a b", b=FP),
                                grow[:, r0:r0 + rr].unsqueeze(2)
                                .broadcast_to([128, rr, FP]),
                                OP.add)
                        n2b = big.tile([128, w, F], bf, name="p2n2b",
                                       tag="p2n2b")
                        for (r0, rr) in _chunks(w):
                            n = rr * F
                            bc_ = b1[:, r0:r0 + rr, 7:7 + F]
                            w1, inv = ln_chunk(sb, ps, bc_, n, "p2b")
                            nc.vector.tensor_tensor(
                                n2b[:, r0:r0 + rr, :].rearrange(
                                    "p a b -> p (a b)"),
                                w1[:, :n], inv[:, :n], OP.mult)
                        dst = AP(n2_d, i * C * RP * F + t0 * F,
                                 [[w * F, 2], [RP * F, C], [F, w], [1, F]])
                        nc.scalar.dma_start(dst, n2b[:])
                        # gap2 partials over strictly-local rows [PADR, PADR+TH)
                        rng = []
                        for q in range(2):
                            a = max(PADR - (t0 + q * w), 0)
                            bq = min(PADR + TH - (t0 + q * w), w)
                            rng.append((a, bq))
                        if rng[0] == (0, w) and rng[1] == (0, w):
                            red = sb.tile([128, F], f32, name="p2red",
                                          tag="p2red")
                            nc.vector.tensor_reduce(
                                red[:], n2b[:].transpose([0, 2, 1]),
                                AX.X, OP.add)
                            nc.vector.tensor_tensor(g2acc[i][:], g2acc[i][:],
                                                    red[:], OP.add)
                        else:
                            for q in range(2):
                                a, bq = rng[q]
                                if bq <= a:
                                    continue
                                p0, p1 = q * 64, q * 64 + 64
                                redh = sb.tile([128, F], f32, name="p2redh",
                                               tag="p2red")
                                nc.vector.tensor_reduce(
                                    redh[p0:p1],
                                    n2b[p0:p1, a:bq, :]
                                    .transpose([0, 2, 1]),
                                    AX.X, OP.add)
                                nc.vector.tensor_tensor(g2acc[i][p0:p1],
                                                        g2acc[i][p0:p1],
                                                        redh[p0:p1], OP.add)
            for i in range(3):
                nc.sync.dma_start(AP(ag2i, i * 128 * F, [[1, 128 * F]]),
                                  g2acc[i][:])
            nc.gpsimd.collective_compute(
                "AllGather", OP.bypass, replica_groups=groups,
                ins=[ag2i.ap()], outs=[ag2o.ap()])

            # filt folds vertical + gterm2
            gt2 = []
            with tc.tile_pool(name="f2sb", bufs=2) as sb, \
                 tc.tile_pool(name="f2ps", bufs=2, space="PSUM") as ps:
                for i in range(3):
                    ga = sb.tile([128, F], f32, name=f"f2ga_{i}", tag="f2ga")
                    gb = sb.tile([128, F], f32, name=f"f2gb_{i}", tag="f2gb")
                    nc.sync.dma_start(ga[:], AP(ag2o, i * 128 * F,
                                                [[F, 128], [1, F]]))
                    nc.sync.dma_start(gb[:], AP(ag2o, 3 * 128 * F + i * 128 * F,
                                                [[F, 128], [1, F]]))
                    gf = per.tile([128, F], f32, name=f"g2full_{i}")
                    nc.vector.tensor_tensor(gf[:], ga[:], gb[:], OP.add)
                    nsum = sb.tile([128, 1], f32, name=f"f2ns_{i}", tag="f2ns")
                    nc.vector.tensor_reduce(nsum[:], gf[:], AX.X, OP.add)
                    n64p = ps.tile([64, 1], f32, name=f"f2n64_{i}", tag="f2n64")
                    nc.tensor.matmul(n64p[:], cw["fold64"][:], nsum[:],
                                     start=True, stop=True)
                    n64 = sb.tile([64, 1], f32, name=f"f2n64s_{i}", tag="f2n64s")
                    nc.vector.tensor_copy(n64[:], n64p[:])
                    a_t, kc_t = _filt_fold(nc, sb, ps, per, cw, n64, i + 3,
                                           f32, bf, AF, OP, AX)
                    gt = per.tile([128, F], f32, name=f"gt2_{i}")
                    nc.vector.tensor_scalar(gt[:], gf[:],
                                            cw[f"kg_{i + 3}"][:, 0:1],
                                            kc_t[:, 0:1], OP.mult, OP.add)
                    gt2.append(gt)
                    ataps.append(a_t)
            dgv = []
            for i in range(3):
                for k in range(KT):
                    dt_ = per.tile([128, 128], bf, name=f"dgv_{i}_{k}")
                    nc.vector.tensor_scalar_mul(dt_[:], cw["ident_bf"][:],
                                                ataps[3 + i][:, k:k + 1])
                    dgv.append(dt_)

        if phases >= 3:
            # ============================ P3 ================================
            with tc.tile_pool(name="p3sb", bufs=3) as sb, \
                 tc.tile_pool(name="p3b2", bufs=2) as b2p, \
                 tc.tile_pool(name="p3ps", bufs=2, space="PSUM") as ps:
                for (t0, w) in _tiles(TH, 20):
                    x2 = x2_load(sb, PADR + t0, w, "p3x")
                    b2s = []
                    for i in range(3):
                        d = DILS[i]
                        n2w = b2p.tile([128, w + 14, F], bf, name=f"p3n2w_{i}",
                                       tag=f"p3n2w_{i}")
                        src = AP(n2_d, i * C * RP * F + t0 * F,
                                 [[w * F, 2], [RP * F, C], [F, w + 14], [1, F]])
                        nc.sync.dma_start(n2w[:], src)
                        n2f = n2w[:].rearrange("p a b -> p (a b)")
                        b2 = b2p.tile([128, w, F], bf, name=f"p3b2_{i}",
                                      tag=f"p3b2_{i}")
                        for (r0, rr) in _chunks(w):
                            n = rr * F
                            bps = ps.tile([128, RW * F], f32, name="p3bps",
                                          tag="p3bps")
                            for k in range(KT):
                                off = (PADR + r0 + (k - 1) * d) * F
                                nc.tensor.matmul(bps[:, :n], dgv[i * KT + k][:],
                                                 n2f[:, off:off + n],
                                                 start=(k == 0), stop=(k == 2))
                            nc.vector.tensor_tensor(
                                b2[:, r0:r0 + rr, :],
                                bps[:, :n].rearrange("p (a b) -> p a b", b=F),
                                gt2[i][:].unsqueeze(1).broadcast_to([128, rr, F]),
                                OP.add)
                        b2s.append(b2)
                    yt = sb.tile([128, w, F], bf, name="p3y", tag="p3y")
                    for (r0, rr) in _chunks(w):
                        n = rr * F
                        yps = ps.tile([128, RW * F], f32, name="p3yps",
                                      tag="p3yps")
                        nc.tensor.matmul(
                            yps[:, :n], cw["wbbd"][:],
                            x2[:, r0:r0 + rr, :].rearrange("p a b -> p (a b)"),
                            start=True, stop=False)
                        for i in range(3):
                            nc.tensor.matmul(
                                yps[:, :n], cw[f"wgbd_{i}"][:],
                                b2s[i][:, r0:r0 + rr, :]
                                .rearrange("p a b -> p (a b)"),
                                start=False, stop=(i == 2))
                        acc = sb.tile([128, 1], f32, name="p3acc", tag="p3acc")
                        nc.scalar.activation(
                            yt[:, r0:r0 + rr, :].rearrange("p a b -> p (a b)"),
                            yps[:, :n], AF.Copy, accum_out=acc[:])
                        nc.vector.tensor_tensor(ysacc[:], ysacc[:], acc[:],
                                                OP.add)
                        sqy = sb.tile([128, RW * F], f32, name="p3sqy",
                                      tag="p3sqy")
                        acq = sb.tile([128, 1], f32, name="p3acq", tag="p3acq")
                        nc.scalar.activation(
                            sqy[:, :n],
                            yt[:, r0:r0 + rr, :].rearrange("p a b -> p (a b)"),
                            AF.Square, accum_out=acq[:])
                        nc.vector.tensor_tensor(yqacc[:], yqacc[:], acq[:],
                                                OP.add)
                    dst = AP(y_d, t0 * F,
                             [[w * F, 2], [TH * F, C], [F, w], [1, F]])
                    nc.scalar.dma_start(dst, yt[:])
            pk = per.tile([128, 2], f32, name="pk")
            nc.vector.tensor_copy(pk[:, 0:1], ysacc[:])
            nc.vector.tensor_copy(pk[:, 1:2], yqacc[:])
            nc.sync.dma_start(AP(ag3i, 0, [[1, 256]]), pk[:])
            nc.gpsimd.collective_compute(
                "AllGather", OP.bypass, replica_groups=groups,
                ins=[ag3i.ap()], outs=[ag3o.ap()])

            # GN scalars (y in y_d excludes convb_b; fold it analytically)
            gnS = per.tile([128, 1], f32, name="gnS")
            gnB = per.tile([128, 1], f32, name="gnB")
            with tc.tile_pool(name="gnsb", bufs=2) as sb, \
                 tc.tile_pool(name="gnps", bufs=2, space="PSUM") as ps:
                pa_ = sb.tile([128, 4], f32, name="gn_pa")
                nc.sync.dma_start(pa_[:, 0:2], AP(ag3o, 0, [[2, 128], [1, 2]]))
                nc.sync.dma_start(pa_[:, 2:4], AP(ag3o, 256, [[2, 128], [1, 2]]))
                sy = sb.tile([128, 1], f32, name="gn_sy")
                sq = sb.tile([128, 1], f32, name="gn_sq")
                nc.vector.tensor_tensor(sy[:], pa_[:, 0:1], pa_[:, 2:3], OP.add)
                nc.vector.tensor_tensor(sq[:], pa_[:, 1:2], pa_[:, 3:4], OP.add)
                NcF = float(TH * F)
                t1 = sb.tile([128, 1], f32, name="gn_t1")
                nc.vector.scalar_tensor_tensor(t1[:], sy[:], 2.0,
                                               cw["cbb"][:], OP.mult, OP.mult)
                nc.vector.tensor_tensor(sq[:], sq[:], t1[:], OP.add)
                cb2 = sb.tile([128, 1], f32, name="gn_cb2")
                nc.scalar.activation(cb2[:], cw["cbb"][:], AF.Square,
                                     scale=1.0)
                nc.vector.tensor_scalar(cb2[:], cb2[:], NcF, None, OP.mult)
                nc.vector.tensor_tensor(sq[:], sq[:], cb2[:], OP.add)
                nc.vector.scalar_tensor_tensor(t1[:], cw["cbb"][:], NcF, sy[:],
                                               OP.mult, OP.add)
                on1 = sb.tile([128, 1], f32, name="gn_on1")
                nc.vector.memset(on1[:], 1.0)
                tots = ps.tile([1, 2], f32, name="gn_tots")
                nc.tensor.matmul(tots[0:1, 0:1], on1[:], t1[:],
                                 start=True, stop=True)
                nc.tensor.matmul(tots[0:1, 1:2], on1[:], sq[:],
                                 start=True, stop=True)
                Ntot = float(C * T * F)
                mg = sb.tile([1, 1], f32, name="gn_mg")
                nc.vector.tensor_scalar(mg[:], tots[0:1, 0:1], 1.0 / Ntot, None,
                                        OP.mult)
                m2g = sb.tile([1, 1], f32, name="gn_m2g")
                nc.scalar.activation(m2g[:], mg[:], AF.Square)
                vg = sb.tile([1, 1], f32, name="gn_vg")
                nc.vector.scalar_tensor_tensor(vg[:], tots[0:1, 1:2], 1.0 / Ntot,
                                               m2g[:], OP.mult, OP.subtract)
                nc.vector.tensor_scalar(vg[:], vg[:], EPS, None, OP.add)
                ig = sb.tile([1, 1], f32, name="gn_ig")
                nc.scalar.activation(ig[:], vg[:], AF.Abs_reciprocal_sqrt)
                igb = sb.tile([128, 1], f32, name="gn_igb")
                mgb = sb.tile([128, 1], f32, name="gn_mgb")
                nc.gpsimd.partition_broadcast(igb[:], ig[:])
                nc.gpsimd.partition_broadcast(mgb[:], mg[:])
                nc.vector.tensor_tensor(gnS[:], igb[:], cw["gn_g"][:], OP.mult)
                nc.vector.tensor_tensor(gnB[:], cw["cbb"][:], mgb[:],
                                        OP.subtract)
                nc.vector.tensor_tensor(gnB[:], gnB[:], gnS[:], OP.mult)
                nc.vector.tensor_tensor(gnB[:], gnB[:], cw["gn_b"][:], OP.add)

        if phases >= 4:
            # ============================ P4 ================================
            with tc.tile_pool(name="p4sb", bufs=2) as sb, \
                 tc.tile_pool(name="p4ps", bufs=2, space="PSUM") as ps, \
                 tc.tile_pool(name="p4st", bufs=1, space="PSUM") as pst:
                for (t0, w) in _tiles(TH, 20):
                    y2 = sb.tile([128, w, F], bf, name="p4y", tag="p4y")
                    nc.sync.dma_start(
                        y2[:], AP(y_d, t0 * F,
                                  [[w * F, 2], [TH * F, C], [F, w], [1, F]]))
                    ot = sb.tile([128, w, F], bf, name="p4o", tag="p4o")
                    nc.scalar.activation(ot[:], y2[:], AF.Prelu,
                                         bias=gnB[:, 0:1], scale=gnS[:, 0:1],
                                         alpha=cw["a0vec"][:, 0:1])
                    nc.scalar.dma_start(
                        AP(out_d, t0 * F,
                           [[w * F, 2], [TH * F, C], [F, w], [1, F]]), ot[:])
                    qk = sb.tile([64, w, F], bf, name="p4qk", tag="p4qk")
                    vt = sb.tile([128, w, F], bf, name="p4v", tag="p4v")
                    for (r0, rr) in _chunks(w):
                        n = rr * F
                        oc = ot[:, r0:r0 + rr, :].rearrange("p a b -> p (a b)")
                        qps = ps.tile([64, RW * F], f32, name="p4qps",
                                      tag="p4qps")
                        vps = ps.tile([128, RW * F], f32, name="p4vps",
                                      tag="p4vps")
                        nc.tensor.matmul(qps[:, :n], cw["wqkbd"][:], oc,
                                         start=True, stop=True)
                        nc.tensor.matmul(vps[:, :n], cw["wvbd"][:], oc,
                                         start=True, stop=True)
                        nc.scalar.activation(
                            qk[:, r0:r0 + rr, :].rearrange("p a b -> p (a b)"),
                            qps[:, :n], AF.Prelu, bias=cw["qkb"][:, 0:1],
                            alpha=cw["qka"][:, 0:1])
                        nc.scalar.activation(
                            vt[:, r0:r0 + rr, :].rearrange("p a b -> p (a b)"),
                            vps[:, :n], AF.Prelu, bias=cw["vb"][:, 0:1],
                            alpha=cw["va"][:, 0:1])
                    qs = sb.tile([64, w], f32, name="p4qs", tag="p4qs")
                    vs = sb.tile([128, w], f32, name="p4vs", tag="p4vs")
                    nc.vector.tensor_reduce(qs[:], qk[:], AX.X, OP.add)
                    nc.vector.tensor_reduce(vs[:], vt[:], AX.X, OP.add)
                    qq = sb.tile([64, w, F], f32, name="p4qq", tag="p4qq")
                    vv = sb.tile([128, w, F], f32, name="p4vv", tag="p4vv")
                    nc.scalar.activation(qq[:], qk[:], AF.Square)
                    nc.scalar.activation(vv[:], vt[:], AF.Square)
                    qsq = sb.tile([64, w], f32, name="p4qsq", tag="p4qsq")
                    vsq = sb.tile([128, w], f32, name="p4vsq", tag="p4vsq")
                    nc.vector.tensor_reduce(qsq[:], qq[:], AX.X, OP.add)
                    nc.vector.tensor_reduce(vsq[:], vv[:], AX.X, OP.add)
                    stq = pst.tile([16, 2 * w], f32, name="p4stq", tag="p4stq")
                    stv = pst.tile([8, 2 * w], f32, name="p4stv", tag="p4stv")
                    nc.tensor.matmul(stq[:, 0:w], cw["grp16"][:], qs[:],
                                     start=True, stop=True)
                    nc.tensor.matmul(stq[:, w:2 * w], cw["grp16"][:], qsq[:],
                                     start=True, stop=True)
                    nc.tensor.matmul(stv[:, 0:w], cw["grp8"][:], vs[:],
                                     start=True, stop=True)
                    nc.tensor.matmul(stv[:, w:2 * w], cw["grp8"][:], vsq[:],
                                     start=True, stop=True)
                    mivs = {}
                    for (st, npart, ncnt, nm) in ((stq, 16, 4 * F, "q"),
                                                  (stv, 8, 16 * F, "v")):
                        mu = sb.tile([npart, w], f32, name=f"p4mu{nm}",
                                     tag=f"p4mu{nm}")
                        nc.vector.tensor_scalar(mu[:], st[:, 0:w], 1.0 / ncnt,
                                                None, OP.mult)
                        m2_ = sb.tile([npart, w], f32, name=f"p4m2{nm}",
                                      tag=f"p4m2{nm}")
                        nc.scalar.activation(m2_[:], mu[:], AF.Square)
                        var = sb.tile([npart, w], f32, name=f"p4var{nm}",
                                      tag=f"p4var{nm}")
                        nc.vector.scalar_tensor_tensor(var[:], st[:, w:2 * w],
                                                       1.0 / ncnt, m2_[:],
                                                       OP.mult, OP.subtract)
                        nc.vector.tensor_scalar(var[:], var[:], EPS, None,
                                                OP.add)
                        iv = sb.tile([npart, w], f32, name=f"p4iv{nm}",
                                     tag=f"p4iv{nm}")
                        nc.scalar.activation(iv[:], var[:], AF.Abs_reciprocal_sqrt)
                        mivs[nm] = (mu, iv)
                    qb_ = pst.tile([64, 2 * w], f32, name="p4qb", tag="p4qb")
                    vb_ = pst.tile([128, 2 * w], f32, name="p4vb", tag="p4vb")
                    nc.tensor.matmul(qb_[:, 0:w], cw["bc16"][:], mivs["q"][0][:],
                                     start=True, stop=True)
                    nc.tensor.matmul(qb_[:, w:2 * w], cw["bc16"][:],
                                     mivs["q"][1][:], start=True, stop=True)
                    nc.tensor.matmul(vb_[:, 0:w], cw["bc8"][:], mivs["v"][0][:],
                                     start=True, stop=True)
                    nc.tensor.matmul(vb_[:, w:2 * w], cw["bc8"][:],
                                     mivs["v"][1][:], start=True, stop=True)
                    qkn = sb.tile([64, w, F], bf, name="p4qkn", tag="p4qkn")
                    vn = sb.tile([128, w, F], bf, name="p4vn", tag="p4vn")
                    nc.vector.tensor_tensor(
                        qkn[:], qk[:],
                        qb_[:, 0:w].unsqueeze(2).broadcast_to([64, w, F]),
                        OP.subtract)
                    nc.vector.tensor_tensor(
                        qkn[:], qkn[:],
                        qb_[:, w:2 * w].unsqueeze(2).broadcast_to([64, w, F]),
                        OP.mult)
                    nc.vector.tensor_tensor(
                        vn[:], vt[:],
                        vb_[:, 0:w].unsqueeze(2).broadcast_to([128, w, F]),
                        OP.subtract)
                    nc.vector.tensor_tensor(
                        vn[:], vn[:],
                        vb_[:, w:2 * w].unsqueeze(2).broadcast_to([128, w, F]),
                        OP.mult)
                    nc.scalar.dma_start(
                        AP(qkvh_d, t0 * F,
                           [[w * F, 2], [TH * F, 32], [F, w], [1, F]]), qkn[:])
                    nc.scalar.dma_start(
                        AP(qkvh_d, 32 * TH * F + t0 * F,
                           [[w * F, 2], [TH * F, 64], [F, w], [1, F]]), vn[:])
            nc.gpsimd.collective_compute(
                "AllGather", OP.bypass, replica_groups=groups,
                ins=[AP(qkvh_d, 16 * TH * F, [[1, 80 * TH * F]])],
                outs=[qkvf_d.ap()])

        if phases >= 5:
            # ============================ P5 ================================
            KCH = [(0, 128), (128, 128), (256, 128), (384, 128), (512, 4)]
            NCH = [(0, 512), (512, 512), (1024, 512), (1536, 512), (2048, 16)]
            import contextlib as _cl
            for h in range(H):
                hx = _cl.ExitStack()
                sb = hx.enter_context(tc.tile_pool(name=f"a{h}sb", bufs=3))
                res = hx.enter_context(tc.tile_pool(name=f"a{h}res", bufs=1))
                ps = hx.enter_context(tc.tile_pool(name=f"a{h}ps", bufs=2,
                                                   space="PSUM"))
                pss = hx.enter_context(tc.tile_pool(name=f"a{h}pss", bufs=1,
                                                    space="PSUM"))
                pso = hx.enter_context(tc.tile_pool(name=f"a{h}pso", bufs=2,
                                                    space="PSUM"))
                ktr = [res.tile([kn, T], bf, name=f"ktr{h}_{ci}")
                       for ci, (k0, kn) in enumerate(KCH)]
                qtr = [res.tile([kn, TH], bf, name=f"qtr{h}_{ci}")
                       for ci, (k0, kn) in enumerate(KCH)]
                vres = [res.tile([MB, DF], bf, name=f"vres{h}_{b}")
                        for b in range(8)]
                for b in range(8):
                    seg, tl = divmod(b, 4)
                    km = sb.tile([MB, EF], bf, name=f"km{h}", tag="km")
                    nc.sync.dma_start(
                        km[:], AP(qkvf_d,
                                  seg * 80 * TH * F + (h * 4) * TH * F
                                  + tl * MB * F,
                                  [[F, MB], [TH * F, E], [1, F]]))
                    for ci, (k0, kn) in enumerate(KCH):
                        tp = ps.tile([128, MB], bf, name=f"tp{h}", tag="tp")
                        nc.tensor.transpose(tp[:kn, :], km[:, k0:k0 + kn],
                                            cw["ident_bf"][0:MB, 0:MB])
                        nc.vector.tensor_copy(ktr[ci][:, b * MB:(b + 1) * MB],
                                              tp[:kn, :])
                    nc.sync.dma_start(
                        vres[b][:], AP(qkvf_d,
                                       seg * 80 * TH * F + (16 + h * 16) * TH * F
                                       + tl * MB * F,
                                       [[F, MB], [TH * F, D], [1, F]]))
                for b in range(4):
                    km = sb.tile([MB, EF], bf, name=f"qm{h}", tag="km")
                    nc.sync.dma_start(
                        km[:], AP(qkvh_d, (h * 4) * TH * F + b * MB * F,
                                  [[F, MB], [TH * F, E], [1, F]]))
                    for ci, (k0, kn) in enumerate(KCH):
                        tp = ps.tile([128, MB], bf, name=f"tp{h}", tag="tp")
                        nc.tensor.transpose(tp[:kn, :], km[:, k0:k0 + kn],
                                            cw["ident_bf"][0:MB, 0:MB])
                        nc.vector.tensor_copy(qtr[ci][:, b * MB:(b + 1) * MB],
                                              tp[:kn, :])
                # lng/lnb per-partition post-transpose
                for ci, (k0, kn) in enumerate(KCH):
                    for (lg, lb, tt) in ((("klngT"), ("klnbT"), ktr),
                                         (("qlngT"), ("qlnbT"), qtr)):
                        vg_ = sb.tile([128, 1], f32, name=f"lg{h}", tag="lg")
                        vb2 = sb.tile([128, 1], f32, name=f"lb{h}", tag="lb")
                        nc.sync.dma_start(vg_[:kn, :],
                                          AP(fw[lg], h * 640 + k0, [[1, kn]]))
                        nc.sync.dma_start(vb2[:kn, :],
                                          AP(fw[lb], h * 640 + k0, [[1, kn]]))
                        nc.vector.tensor_scalar(tt[ci][:], tt[ci][:],
                                                vg_[:kn, 0:1], vb2[:kn, 0:1],
                                                OP.mult, OP.add)
                lngb = res.tile([MB, DF], f32, name=f"lngb{h}")
                lrow = sb.tile([1, DF], f32, name=f"lrow{h}", tag="lrow")
                nc.sync.dma_start(lrow[:], AP(fw["vlngT"], h * DF, [[DF, 1],
                                                                   [1, DF]]))
                nc.gpsimd.partition_broadcast(lngb[:], lrow[:])
                for mt in range(4):
                    spt = pss.tile([MB, 1024], f32, name=f"spt{h}", tag="spt")
                    for ci, (k0, kn) in enumerate(KCH):
                        lhs = qtr[ci][:, mt * MB:(mt + 1) * MB]
                        nc.tensor.matmul(spt[:, 0:500], lhs, ktr[ci][:, 0:500],
                                         start=(ci == 0), stop=(ci == 4))
                        nc.tensor.matmul(spt[:, 512:1012], lhs,
                                         ktr[ci][:, 500:1000],
                                         start=(ci == 0), stop=(ci == 4))
                    sview = spt[:].rearrange("p (a b) -> p a b", b=512)[:, :, 0:500]
                    mx = sb.tile([MB, 1], f32, name=f"mx{h}", tag="mx")
                    nc.vector.tensor_reduce(mx[:], sview, AX.XY, OP.max)
                    bias = sb.tile([MB, 1], f32, name=f"bias{h}", tag="bias")
                    nc.vector.tensor_scalar(bias[:], mx[:], -ISCALE, None,
                                            OP.mult)
                    pexp = sb.tile([MB, 1000], bf, name=f"pexp{h}", tag="pexp")
                    lsum = sb.tile([MB, 1], f32, name=f"lsum{h}", tag="lsum")
                    nc.scalar.activation(pexp[:], sview, AF.Exp,
                                         bias=bias[:, 0:1], scale=ISCALE,
                                         accum_out=lsum[:])
                    linv = sb.tile([MB, 1], f32, name=f"linv{h}", tag="linv")
                    nc.vector.reciprocal(linv[:], lsum[:])
                    ptr = []
                    for b in range(8):
                        tp2 = ps.tile([MB, MB], bf, name=f"tp2{h}", tag="tp2")
                        nc.tensor.transpose(tp2[:], pexp[:, b * MB:(b + 1) * MB],
                                            cw["ident_bf"][0:MB, 0:MB])
                        pb_ = sb.tile([MB, MB], bf, name=f"ptr{h}_{b}",
                                      tag=f"ptr{b}")
                        nc.vector.tensor_copy(pb_[:], tp2[:])
                        ptr.append(pb_)
                    ob = sb.tile([MB, DF], bf, name=f"ob{h}", tag="ob")
                    for (n0, nn) in NCH:
                        op_ = pso.tile([MB, 512], f32, name=f"op{h}", tag="op")
                        for b in range(8):
                            nc.tensor.matmul(op_[:, :nn], ptr[b][:],
                                             vres[b][:, n0:n0 + nn],
                                             start=(b == 0), stop=(b == 7))
                        nc.vector.scalar_tensor_tensor(
                            ob[:, n0:n0 + nn], op_[:, :nn], linv[:, 0:1],
                            lngb[:, n0:n0 + nn], OP.mult, OP.mult)
                    nc.scalar.dma_start(
                        AP(o_d, mt * MB * H * D * F + h * D * F,
                           [[H * D * F, MB], [1, D * F]]), ob[:])
                hx.close()

        if phases >= 6:
            # ============================ P6 ================================
            with tc.tile_pool(name="p6sb", bufs=2) as sb, \
                 tc.tile_pool(name="p6ps", bufs=2, space="PSUM") as ps, \
                 tc.tile_pool(name="p6st", bufs=1, space="PSUM") as pst:
                for (t0, w) in _tiles(TH, 20):
                    o2 = sb.tile([128, w, F], bf, name="p6o", tag="p6o")
                    for q, eng in ((0, nc.sync), (1, nc.gpsimd)):
                        eng.dma_start(
                            o2[q * 64:(q + 1) * 64],
                            AP(o_d, (t0 + q * w) * H * D * F,
                               [[F, 64], [H * D * F, w], [1, F]]))
                    u2 = sb.tile([128, w, F], f32, name="p6u2", tag="p6u2")
                    for (r0, rr) in _chunks(w):
                        n = rr * F
                        pps = ps.tile([128, RW * F], f32, name="p6pps",
                                      tag="p6pps")
                        nc.tensor.matmul(
                            pps[:, :n], cw["pwbd"][:],
                            o2[:, r0:r0 + rr, :].rearrange("p a b -> p (a b)"),
                            start=True, stop=True)
                        tmp = sb.tile([128, RW, F], f32, name="p6tmp",
                                      tag="p6tmp")
                        nc.vector.tensor_tensor(
                            tmp[:, :rr, :],
                            pps[:, :n].rearrange("p (a b) -> p a b", b=F),
                            cw["pconst"][:].unsqueeze(1)
                            .broadcast_to([128, rr, F]),
                            OP.add)
                        nc.scalar.activation(
                            u2[:, r0:r0 + rr, :].rearrange("p a b -> p (a b)"),
                            tmp[:, :rr, :].rearrange("p a b -> p (a b)"),
                            AF.Prelu, bias=cw["pb"][:, 0:1],
                            alpha=cw["pa"][:, 0:1])
                    us = sb.tile([128, w], f32, name="p6us", tag="p6us")
                    nc.vector.tensor_reduce(us[:], u2[:], AX.X, OP.add)
                    uq = sb.tile([128, w, F], f32, name="p6uq", tag="p6uq")
                    nc.scalar.activation(uq[:], u2[:], AF.Square)
                    usq = sb.tile([128, w], f32, name="p6usq", tag="p6usq")
                    nc.vector.tensor_reduce(usq[:], uq[:], AX.X, OP.add)
                    st2 = pst.tile([2, 2 * w], f32, name="p6st2", tag="p6st2")
                    nc.tensor.matmul(st2[:, 0:w], cw["redq"][:], us[:],
                                     start=True, stop=True)
                    nc.tensor.matmul(st2[:, w:2 * w], cw["redq"][:], usq[:],
                                     start=True, stop=True)
                    ncnt = float(64 * F)
                    mu2 = sb.tile([2, w], f32, name="p6mu2", tag="p6mu2")
                    nc.vector.tensor_scalar(mu2[:], st2[:, 0:w], 1.0 / ncnt,
                                            None, OP.mult)
                    m22 = sb.tile([2, w], f32, name="p6m22", tag="p6m22")
                    nc.scalar.activation(m22[:], mu2[:], AF.Square)
                    var2 = sb.tile([2, w], f32, name="p6var2", tag="p6var2")
                    nc.vector.scalar_tensor_tensor(var2[:], st2[:, w:2 * w],
                                                   1.0 / ncnt, m22[:],
                                                   OP.mult, OP.subtract)
                    nc.vector.tensor_scalar(var2[:], var2[:], EPS, None, OP.add)
                    iv2 = sb.tile([2, w], f32, name="p6iv2", tag="p6iv2")
                    nc.scalar.activation(iv2[:], var2[:], AF.Abs_reciprocal_sqrt)
                    mb2 = pst.tile([128, 2 * w], f32, name="p6mb2", tag="p6mb2")
                    nc.tensor.matmul(mb2[:, 0:w], cw["ind2f"][:], mu2[:],
                                     start=True, stop=True)
                    nc.tensor.matmul(mb2[:, w:2 * w], cw["ind2f"][:], iv2[:],
                                     start=True, stop=True)
                    nc.vector.tensor_tensor(
                        u2[:], u2[:],
                        mb2[:, 0:w].unsqueeze(2).broadcast_to([128, w, F]),
                        OP.subtract)
                    nc.vector.tensor_tensor(
                        u2[:], u2[:],
                        mb2[:, w:2 * w].unsqueeze(2).broadcast_to([128, w, F]),
                        OP.mult)
                    nc.vector.tensor_tensor(
                        u2[:], u2[:],
                        cw["plng"][:].unsqueeze(1).broadcast_to([128, w, F]),
                        OP.mult)
                    rt = sb.tile([128, w, F], bf, name="p6rt", tag="p6rt")
                    nc.sync.dma_start(
                        rt[:], AP(out_d, t0 * F,
                                  [[w * F, 2], [TH * F, C], [F, w], [1, F]]))
                    r1 = sb.tile([128, w, F], f32, name="p6r1", tag="p6r1")
                    nc.vector.tensor_tensor(
                        r1[:], rt[:],
                        cw["plnb"][:].unsqueeze(1).broadcast_to([128, w, F]),
                        OP.add)
                    fint = sb.tile([128, w, F], bf, name="p6fin", tag="p6fin")
                    nc.vector.tensor_tensor(fint[:], u2[:], r1[:], OP.add)
                    nc.scalar.dma_start(
                        AP(fin, t0 * F,
                           [[w * F, 2], [TH * F, C], [F, w], [1, F]]), fint[:])
        if phases < 6:
            with tc.tile_pool(name="dummy", bufs=1) as dp:
                zt = dp.tile([C, 16], bf, name="zfin")
                nc.vector.memset(zt[:], 0.0)
                nc.sync.dma_start(AP(fin, 0, [[TH * F, C], [1, 16]]), zt[:])
        ctx.close()
    nc.compile()
    return nc


def _filt_fold(nc, sb, ps, per, cw, mean64, i, f32, bf, AF, OP, AX):
    u1 = sb.tile([64, 1], f32, name=f"u1_{i}", tag="ffu1")
    nc.vector.tensor_scalar(u1[:], mean64[:], cw[f"gtf_{i}"][:, 0:1],
                            cw[f"c64_{i}"][:, 0:1], OP.mult, OP.add)
    ftp = ps.tile([1, 12], f32, name=f"ftp_{i}", tag="ffftp")
    nc.tensor.matmul(ftp[:], u1[:], cw[f"lwT_{i}"][:], start=True, stop=True)
    ft = sb.tile([1, 12], f32, name=f"ft_{i}", tag="ffft")
    nc.scalar.activation(ft[:], ftp[:], AF.Tanh)
    ft4 = sb.tile([4, 3], f32, name=f"ft4_{i}", tag="ffft4")
    nc.sync.dma_start(ft4[:], ft[:].rearrange("o (g k) -> (o g) k", g=4))
    wcp = ps.tile([128, 3], f32, name=f"wcp_{i}", tag="ffwcp")
    nc.tensor.matmul(wcp[:], cw["grp4"][:], ft4[:], start=True, stop=True)
    atap = per.tile([128, 3], f32, name=f"atap_{i}")
    nc.vector.tensor_scalar_mul(atap[:], wcp[:], cw[f"gs_{i}"][:, 0:1])
    nc.vector.tensor_tensor(atap[:, 1:2], atap[:, 1:2],
                            cw[f"hc_{i}"][:, 0:1], OP.add)
    wcs = sb.tile([128, 1], f32, name=f"wcs_{i}", tag="ffwcs")
    nc.vector.tensor_reduce(wcs[:], wcp[:], AX.X, OP.add)
    kc = per.tile([128, 1], f32, name=f"kc_{i}")
    nc.vector.tensor_scalar(kc[:], wcs[:], cw[f"cs_{i}"][:, 0:1],
                            cw[f"cb_{i}"][:, 0:1], OP.mult, OP.add)
    nc.vector.tensor_tensor(kc[:], kc[:], cw[f"gc_{i}"][:, 0:1], OP.add)
    return atap, kc


# ---------------------------------------------------------------------------
# host entry
# ---------------------------------------------------------------------------

def _prep_inputs(inputs, fold):
    import ml_dtypes
    x = np.asarray(inputs["x"], np.float32)
    in_maps = []
    for c in range(8):
        s, hf = divmod(c, 2)
        xs = np.pad(x[s], ((0, 0), (PADR, PADR), (0, 0)), mode="reflect")
        xc = xs[:, hf * TH:hf * TH + RP, :]
        m = {"x_d": np.ascontiguousarray(xc).astype(ml_dtypes.bfloat16)}
        for n, (sh, isbf) in _fold_shapes().items():
            v = fold[n]
            m[n] = v.astype(ml_dtypes.bfloat16) if isbf else v
        for n in _dram_only_shapes():
            m[n] = fold[n]
        in_maps.append(m)
    return in_maps


def kernel(**inputs):
    import os
    from concourse.bass_utils import run_bass_kernel_spmd
    global LAST_EXEC_NS
    if "nc" not in _CACHE:
        _CACHE["nc"] = _build(dbg=_CACHE.get("dbg", ()),
                              phases=int(os.environ.get("KPHASES", "6")))
    nc = _CACHE["nc"]
    fold = _fold_weights(inputs)
    in_maps = _prep_inputs(inputs, fold)
    kw = {}
    if os.environ.get("KTRACE"):
        import tempfile
        base = os.environ.get("KTRACE_DIR",
                              os.path.join(os.getcwd(), "work"))
        os.makedirs(base, exist_ok=True)
        tdir = tempfile.mkdtemp(prefix="trace_", dir=base)
        with open(os.path.join(base, "last_trace_path.txt"), "w") as f:
            f.write(tdir)
        tc_ = os.environ.get("KTRACE_CORES", "0")
        kw = dict(trace=True, tmpdir=tdir,
                  trace_cores=[int(c) for c in tc_.split(",")])
    res = run_bass_kernel_spmd(nc, in_maps, core_ids=list(range(8)), **kw)
    _CACHE["last"] = res
    if getattr(res, "exec_time_ns", None):
        LAST_EXEC_NS = res.exec_time_ns
    out = np.zeros((B, C, T, F), np.float32)
    for c in range(8):
        s, hf = divmod(c, 2)
        out[s][:, hf * TH:(hf + 1) * TH, :] = \
            res.results[c]["fin"].astype(np.float32)
    return out



# revision 41
# speedup vs baseline: 1.4835x; 1.1760x over previous
"""GridNetBlock_Att Trainium2 kernel (Bass/Tile, 8 NeuronCores).

Core c handles sample s=c//2, T-half h=c%2 (rows [500h, 500h+500)).
Pre-attention is T-split per core (host supplies x with a reflect-padded
t-halo of 7 rows, which makes the SPMD program identical on all cores);
tiny pair AllGathers combine global stats, and one pair AllGather
exchanges the K/V halves before full-sequence attention.

Pre-attention tiles are "2-stack": 128 partitions = 2 consecutive
row-blocks x 64 channels.  LN-over-channels per psum chunk:
    w1  = (I - BO/64) @ x        (PE; BO = block-ones)    = x - mu
    sqw = Square(w1)             (ACT, psum->sbuf)
    s1  = BO @ sqw               (PE)                     = 64*var
    inv = AbsRecipSqrt(s1/64)    (ACT)                    = 1/sqrt(var)
    z   = w1 * inv               (DVE, psum x sbuf -> bf16)
"""
import time

import numpy as np

EPS = 1e-5
B, C, T, F = 4, 64, 1000, 129
H, E, D = 4, 4, 16
GROUP, KT = 4, 3
DILS = (3, 5, 7)
EF, DF = E * F, D * F      # 516, 2064
TH = T // 2                 # 500 local rows
PADR = 7
RP = TH + 2 * PADR          # 514
FP = F + 14                 # 143
TF = float(T * F)
RW = 3                      # rows per psum chunk
ISCALE = float(1.0 / np.sqrt(EF))
MB = 125                    # attention row block

_CACHE = {}
LAST_EXEC_NS = -1


def _tiles(total, w):
    out, t0 = [], 0
    while t0 < total:
        ww = min(2 * w, total - t0) // 2
        out.append((t0, ww))
        t0 += 2 * ww
    return out


def _chunks(w, rw=RW):
    return [(ci * rw, min(rw, w - ci * rw)) for ci in range((w + rw - 1) // rw)]


# ---------------------------------------------------------------------------
# host-side weight folding
# ---------------------------------------------------------------------------

def _fold_shapes():
    sh = {
        "bo_f": ((128, 128), False),
        "bo_bf": ((128, 128), True),
        "w1m_bf": ((128, 128), True),
        "ind2f": ((2, 128), False), "ident_bf": ((128, 128), True),
        "fold64": ((128, 64), False), "redq": ((128, 2), False),
        "grp4": ((4, 128), False),
        "wbbd": ((128, 128), True), "cbb": ((128, 1), False),
        "gn_g": ((128, 1), False), "gn_b": ((128, 1), False),
        "a0vec": ((128, 1), False),
        "wqkbd": ((128, 64), True), "wvbd": ((128, 128), True),
        "qkb": ((64, 1), False), "qka": ((64, 1), False),
        "vb": ((128, 1), False), "va": ((128, 1), False),
        "grp16": ((64, 16), False), "bc16": ((16, 64), False),
        "grp8": ((128, 8), False), "bc8": ((8, 128), False),
        "pwbd": ((128, 128), True), "pconst": ((128, F), False),
        "pb": ((128, 1), False), "pa": ((128, 1), False),
        "plng": ((128, F), False), "plnb": ((128, F), False),
    }
    for i in range(6):
        for nm in ("gs", "hc", "kg", "cs", "cb", "gc"):
            sh[f"{nm}_{i}"] = ((128, 1), False)
        sh[f"lwT_{i}"] = ((64, 12), False)
        sh[f"gtf_{i}"] = ((64, 1), False)
        sh[f"c64_{i}"] = ((64, 1), False)
    for i in range(3):
        sh[f"wgbd_{i}"] = ((128, 128), True)
    return sh


# loaded from DRAM on demand, not staged in SBUF constants
def _dram_only_shapes():
    return {
        "qlngT": (H, 640), "klngT": (H, 640),
        "qlnbT": (H, 640), "klnbT": (H, 640),
        "vlngT": (H, DF),
    }


def _fold_weights(w):
    f32 = np.float32
    g = {}
    ar = lambda a: np.ascontiguousarray(a, f32)
    dup = lambda v: np.tile(ar(v).reshape(64), 2).reshape(128, 1)

    bo = np.zeros((128, 128), f32)
    bo[:64, :64] = 1.0
    bo[64:, 64:] = 1.0
    g["bo_f"] = bo
    g["bo_bf"] = bo
    g["w1m_bf"] = np.eye(128, dtype=f32) - bo / 64.0
    ind2 = np.zeros((2, 128), f32)
    ind2[0, :64] = 1.0
    ind2[1, 64:] = 1.0
    g["ind2f"] = ind2
    g["ident_bf"] = np.eye(128, dtype=f32)
    fold2 = np.zeros((128, 64), f32)
    for p in range(128):
        fold2[p, p % 64] = 1.0
    g["fold64"] = fold2
    redq = np.zeros((128, 2), f32)
    redq[:64, 0] = 1.0
    redq[64:, 1] = 1.0
    g["redq"] = redq
    gi4 = np.zeros((4, 128), f32)
    for p in range(128):
        gi4[(p % 64) // 16, p] = 1.0
    g["grp4"] = gi4

    for i in range(6):
        gg = w["br_g"][i].astype(f32)
        cc = w["br_b"][i].astype(f32)
        ia = w["lisa_in"][i].astype(f32)
        ll = w["lisa_ll"][i].astype(f32)
        lh = w["lisa_lh"][i].astype(f32)
        s = (ia + 1.0) * ll
        gap_div = float(F) if i < 3 else float(T)
        g[f"gs_{i}"] = dup(gg * s)
        g[f"hc_{i}"] = dup(gg * (lh + 1.0))
        g[f"kg_{i}"] = dup((-ia * ll * gg) / gap_div)
        g[f"gc_{i}"] = dup(-ia * ll * cc)
        g[f"cs_{i}"] = dup(cc * s)
        g[f"cb_{i}"] = dup(cc * (lh + 1.0))
        g[f"lwT_{i}"] = ar(w["lisa_w"][i].T)
        g[f"gtf_{i}"] = ar((gg / TF).reshape(64, 1))
        g[f"c64_{i}"] = ar(cc.reshape(64, 1))

    cw_ = w["convb_w"].astype(f32)
    gam = w["mix_gamma"].astype(f32)
    bet = w["mix_beta"].astype(f32)

    def bd(m, no):
        z = np.zeros((128, 2 * no), f32)
        z[:64, :no] = m.T
        z[64:, no:] = m.T
        return z

    for i in range(3):
        g[f"wgbd_{i}"] = bd(cw_ * gam[i][None, :], 64)
    g["wbbd"] = bd(cw_ * bet.sum(0)[None, :], 64)
    g["cbb"] = dup(w["convb_b"])
    g["gn_g"] = dup(w["gn_g"])
    g["gn_b"] = dup(w["gn_b"])
    g["a0vec"] = np.full((128, 1), float(w["convb_a"]), f32)

    wqk = np.concatenate([w["q_w"].astype(f32).reshape(H * E, C),
                          w["k_w"].astype(f32).reshape(H * E, C)], 0)
    g["wqkbd"] = bd(wqk, 32)
    g["wvbd"] = bd(w["v_w"].astype(f32).reshape(H * D, C), 64)
    qkb = np.concatenate([w["q_b"].reshape(-1), w["k_b"].reshape(-1)])
    g["qkb"] = np.tile(ar(qkb), 2).reshape(64, 1)
    qka = np.concatenate([np.repeat(w["q_a"], E), np.repeat(w["k_a"], E)])
    g["qka"] = np.tile(ar(qka), 2).reshape(64, 1)
    g["vb"] = np.tile(ar(w["v_b"].reshape(-1)), 2).reshape(128, 1)
    g["va"] = np.tile(ar(np.repeat(w["v_a"], D)), 2).reshape(128, 1)
    g16 = np.zeros((64, 16), f32)
    for p in range(64):
        q, j = divmod(p, 32)
        g16[p, q * 8 + (j // 16) * 4 + (j % 16) // 4] = 1.0
    g["grp16"] = g16
    g["bc16"] = ar(g16.T)
    g8 = np.zeros((128, 8), f32)
    for p in range(128):
        q, j = divmod(p, 64)
        g8[p, q * 4 + j // 16] = 1.0
    g["grp8"] = g8
    g["bc8"] = ar(g8.T)
    for nm, src in (("qlngT", "q_lng"), ("klngT", "k_lng"),
                    ("qlnbT", "q_lnb"), ("klnbT", "k_lnb")):
        m = np.zeros((H, 640), f32)
        for h in range(H):
            m[h, :EF] = w[src][h].reshape(EF)
        g[nm] = m
    g["vlngT"] = ar(w["v_lng"].reshape(H, DF))
    pw = w["proj_w"].astype(f32)
    g["pwbd"] = bd(pw, 64)
    pconst = pw @ w["v_lnb"].reshape(H * D, F).astype(f32)
    g["pconst"] = np.tile(pconst, (2, 1)).reshape(128, F)
    g["pb"] = dup(w["proj_b"])
    g["pa"] = np.full((128, 1), float(w["proj_a"]), f32)
    g["plng"] = np.tile(w["proj_lng"].astype(f32), (2, 1)).reshape(128, F)
    g["plnb"] = np.tile(w["proj_lnb"].astype(f32), (2, 1)).reshape(128, F)
    return g


# ---------------------------------------------------------------------------
# device program
# ---------------------------------------------------------------------------

def _build(dbg=(), phases=6):
    import concourse.bass as bass
    import concourse.bacc as bacc
    import concourse.mybir as mybir
    from concourse import tile
    from contextlib import ExitStack

    f32 = mybir.dt.float32
    bf = mybir.dt.bfloat16
    AF = mybir.ActivationFunctionType
    OP = mybir.AluOpType
    AX = mybir.AxisListType

    nc = bacc.Bacc("TRN2", target_bir_lowering=False, debug=False,
                   num_devices=8)

    def AP(tensor, offset, dims):
        return bass.AP(tensor=tensor, offset=offset,
                       ap=[list(d) for d in dims])

    shapes = _fold_shapes()
    dshapes = _dram_only_shapes()
    x_d = nc.dram_tensor("x_d", [C, RP, F], bf, kind="ExternalInput")
    fw = {n: nc.dram_tensor(n, list(s), bf if b else f32,
                            kind="ExternalInput")
          for n, (s, b) in shapes.items()}
    for n, s in dshapes.items():
        fw[n] = nc.dram_tensor(n, list(s), f32, kind="ExternalInput")

    def idram(name, shape, dt_):
        kind = "ExternalOutput" if name in dbg else "Internal"
        return nc.dram_tensor(name, list(shape), dt_, kind=kind)

    n2_d = idram("n2_d", [3, C, RP, F], bf)
    y_d = idram("y_d", [C, TH, F], bf)
    out_d = idram("out_d", [C, TH, F], bf)
    qkvh_d = idram("qkvh_d", [96, TH, F], bf)
    qkvf_d = idram("qkvf_d", [2, 80, TH, F], bf)
    o_d = idram("o_d", [TH, H, D, F], bf)
    b1_d = idram("b1_d", [3, C, RP, F], bf) if "b1_d" in dbg else None
    dsm_d = idram("dsm_d", [16, 128], f32) if "dsm_d" in dbg else None
    ag1i = nc.dram_tensor("ag1i", [1, 128], f32)
    ag1o = nc.dram_tensor("ag1o", [2, 128], f32)
    ag2i = nc.dram_tensor("ag2i", [1, 3 * 128 * F], f32)
    ag2o = nc.dram_tensor("ag2o", [2, 3 * 128 * F], f32)
    ag3i = nc.dram_tensor("ag3i", [1, 256], f32)
    ag3o = nc.dram_tensor("ag3o", [2, 256], f32)
    fin = nc.dram_tensor("fin", [C, TH, F], bf, kind="ExternalOutput")

    groups = [[0, 1], [2, 3], [4, 5], [6, 7]]

    with nc.allow_low_precision(reason="bf16 pipeline, tol 2e-2"), \
         tile.TileContext(nc) as tc:
        ctx = ExitStack()
        cst = ctx.enter_context(tc.tile_pool(name="cst", bufs=1))
        per = ctx.enter_context(tc.tile_pool(name="per", bufs=1))

        def load_const(name):
            sh, isbf = shapes[name]
            t = cst.tile(list(sh), bf if isbf else f32, name=f"c_{name}",
                         tag=f"c_{name}")
            nc.sync.dma_start(t[:], fw[name].ap())
            return t

        cw = {n: load_const(n) for n in shapes}

        def x2_load(pool, t0, w, nm, eng=None):
            xt = pool.tile([128, w, F], bf, name=nm, tag=nm)
            src = AP(x_d, t0 * F,
                     [[w * F, 2], [RP * F, C], [F, w], [1, F]])
            (eng or nc.sync).dma_start(xt[:], src)
            return xt

        def ln_chunk(sb, ps, src_flat, n, nm):
            w1 = ps.tile([128, RW * FP], f32, name=f"{nm}w1", tag="Lw1")
            s1 = ps.tile([128, RW * FP], f32, name=f"{nm}s1", tag="Ls1")
            nc.tensor.matmul(w1[:, :n], cw["w1m_bf"][:], src_flat,
                             start=True, stop=True)
            sqw = sb.tile([128, RW * FP], bf, name=f"{nm}sqw", tag="Lsq")
            nc.scalar.activation(sqw[:, :n], w1[:, :n], AF.Square)
            nc.tensor.matmul(s1[:, :n], cw["bo_bf"][:], sqw[:, :n],
                             start=True, stop=True)
            inv = sb.tile([128, RW * FP], f32, name=f"{nm}inv", tag="Linv")
            nc.scalar.activation(inv[:, :n], s1[:, :n], AF.Abs_reciprocal_sqrt,
                                 scale=1.0 / 64.0)
            return w1, inv

        # persistent accumulators
        macc = per.tile([128, 1], f32, name="macc")
        nc.vector.memset(macc[:], 0.0)
        g2acc = [per.tile([128, F], f32, name=f"g2acc_{i}") for i in range(3)]
        for i in range(3):
            nc.vector.memset(g2acc[i][:], 0.0)
        ysacc = per.tile([128, 1], f32, name="ysacc")
        yqacc = per.tile([128, 1], f32, name="yqacc")
        nc.vector.memset(ysacc[:], 0.0)
        nc.vector.memset(yqacc[:], 0.0)

        # ============================ P1 ================================
        with tc.tile_pool(name="p1sb", bufs=3) as sb, \
             tc.tile_pool(name="p1ps", bufs=2, space="PSUM") as ps:
            for (t0, w) in _tiles(TH, 24):
                x2 = x2_load(sb, PADR + t0, w, "p1x")
                for (r0, rr) in _chunks(w):
                    n = rr * F
                    xc = x2[:, r0:r0 + rr, :].rearrange("p a b -> p (a b)")
                    w1, inv = ln_chunk(sb, ps, xc, n, "p1")
                    junk = sb.tile([128, RW * F], bf, name="p1junk",
                                   tag="p1junk")
                    acc = sb.tile([128, 1], f32, name="p1acc", tag="p1acc")
                    nc.vector.scalar_tensor_tensor(
                        junk[:, :n], w1[:, :n], 1.0, inv[:, :n],
                        OP.mult, OP.mult, accum_out=acc[:])
                    nc.vector.tensor_tensor(macc[:], macc[:], acc[:], OP.add)
        nc.sync.dma_start(AP(ag1i, 0, [[1, 128]]), macc[:])
        nc.gpsimd.collective_compute(
            "AllGather", OP.bypass, replica_groups=groups,
            ins=[ag1i.ap()], outs=[ag1o.ap()])
        m_a = per.tile([128, 2], f32, name="m_a")
        nc.sync.dma_start(m_a[:], AP(ag1o, 0, [[1, 128], [128, 2]]))
        mtot = per.tile([128, 1], f32, name="mtot")
        nc.vector.tensor_tensor(mtot[:], m_a[:, 0:1], m_a[:, 1:2], OP.add)

        # filt folds, horizontal stages
        ataps, kcv = [], []
        with tc.tile_pool(name="ffsb", bufs=2) as sb, \
             tc.tile_pool(name="ffps", bufs=2, space="PSUM") as ps:
            m64p = ps.tile([64, 1], f32, name="m64p")
            nc.tensor.matmul(m64p[:], cw["fold64"][:], mtot[:],
                             start=True, stop=True)
            m64 = per.tile([64, 1], f32, name="m64")
            nc.vector.tensor_copy(m64[:], m64p[:])
            for i in range(3):
                a_t, kc_t = _filt_fold(nc, sb, ps, per, cw, m64, i,
                                       f32, bf, AF, OP, AX)
                ataps.append(a_t)
                kcv.append(kc_t)
        if dsm_d is not None:
            nc.sync.dma_start(AP(dsm_d, 0, [[1, 128]]), mtot[:])
            for i in range(3):
                for k in range(KT):
                    nc.sync.dma_start(
                        AP(dsm_d, (1 + i * 3 + k) * 128, [[1, 128]]),
                        ataps[i][:, k:k + 1])
                nc.sync.dma_start(AP(dsm_d, (10 + i) * 128, [[1, 128]]),
                                  kcv[i][:, 0:1])
        dgh = []
        for i in range(3):
            for k in range(KT):
                # fused tap+LN-mean-removal stationary:
                # M_k^T = diag(a_k) (I - BO/64)  (w1m is symmetric)
                dt_ = per.tile([128, 128], bf, name=f"dgh_{i}_{k}")
                nc.vector.tensor_scalar_mul(dt_[:], cw["w1m_bf"][:],
                                            ataps[i][:, k:k + 1])
                dgh.append(dt_)

        if phases >= 2:
            # ============================ P2 ================================
            with tc.tile_pool(name="p2sb", bufs=3) as sb, \
                 tc.tile_pool(name="p2big", bufs=2) as big, \
                 tc.tile_pool(name="p2ps", bufs=1, space="PSUM") as ps, \
                 tc.tile_pool(name="p2pst", bufs=1, space="PSUM") as pst:
                for (t0, w) in _tiles(RP, 20):
                    x2 = x2_load(sb, t0, w, "p2x")
                    zzt = big.tile([128, 8 + w * FP + 8], bf, name="p2zzt",
                                   tag="p2zzt")
                    zz = zzt[:, 8:8 + w * FP].rearrange("p (a b) -> p a b", b=FP)
                    nc.vector.memset(zzt[:, 0:8], 0.0)
                    nc.vector.memset(zzt[:, 8 + w * FP:8 + w * FP + 8], 0.0)
                    for (r0, rr) in _chunks(w):
                        n = rr * F
                        xc = x2[:, r0:r0 + rr, :].rearrange("p a b -> p (a b)")
                        w1, inv = ln_chunk(sb, ps, xc, n, "p2a")
                        nc.vector.tensor_tensor(
                            zz[:, r0:r0 + rr, 7:7 + F],
                            w1[:, :n].rearrange("p (a b) -> p a b", b=F),
                            inv[:, :n].rearrange("p (a b) -> p a b", b=F),
                            OP.mult)
                    nc.vector.tensor_copy(zz[:, :, 0:7], zz[:, :, 14:7:-1])
                    nc.vector.tensor_copy(zz[:, :, 136:143], zz[:, :, 134:127:-1])
                    gpf = sb.tile([128, w], f32, name="p2gpf", tag="p2gpf")
                    nc.vector.tensor_reduce(gpf[:], zz[:, :, 7:7 + F], AX.X,
                                            OP.add)
                    for i in range(3):
                        d = DILS[i]
                        # grow' = grow - channel-block mean (fused tap
                        # matmuls M_k = (I-BO/64) diag(a_k) remove the mean
                        # of the tap sum; grow must match)
                        grow = sb.tile([128, w], f32, name="p2grow", tag="p2grow")
                        nc.vector.tensor_scalar(grow[:], gpf[:],
                                                cw[f"kg_{i}"][:, 0:1],
                                                kcv[i][:, 0:1], OP.mult, OP.add)
                        gmp = pst.tile([128, RW * F], f32, name="p2gmp",
                                       tag="p2gmp")
                        nc.tensor.matmul(gmp[:, :w], cw["bo_f"][:], grow[:],
                                         start=True, stop=True)
                        growp = sb.tile([128, w], f32, name="p2growp",
                                        tag="p2growp")
                        nc.vector.scalar_tensor_tensor(
                            growp[:], gmp[:, :w], -1.0 / 64.0, grow[:],
                            OP.mult, OP.add)
                        n2b = big.tile([128, w, F], bf, name="p2n2b",
                                       tag="p2n2b")
                        chs = _chunks(w)
                        for g0 in range(0, len(chs), 4):
                            grp = chs[g0:g0 + 4]
                            cps = [pst.tile([128, RW * F], f32,
                                            name=f"p2cp{cj}", tag=f"p2cp{cj}")
                                   for cj in range(len(grp))]
                            for k in range(KT):
                                a = 7 + (k - 1) * d
                                for cj, (r0, rr) in enumerate(grp):
                                    nc.tensor.matmul(
                                        cps[cj][:, :rr * F], dgh[i * KT + k][:],
                                        zz[:, r0:r0 + rr, a:a + F],
                                        start=(k == 0), stop=(k == 2))
                            for cj, (r0, rr) in enumerate(grp):
                                n = rr * F
                                w1f = sb.tile([128, RW * F], f32, name="p2w1f",
                                              tag="p2w1f")
                                nc.vector.tensor_tensor(
                                    w1f[:, :n].rearrange("p (a b) -> p a b",
                                                         b=F),
                                    cps[cj][:, :n].rearrange(
                                        "p (a b) -> p a b", b=F),
                                    growp[:, r0:r0 + rr].unsqueeze(2)
                                    .broadcast_to([128, rr, F]),
                                    OP.add)
                                sqw = sb.tile([128, RW * F], bf, name="p2sq",
                                              tag="p2sq")
                                nc.scalar.activation(sqw[:, :n], w1f[:, :n],
                                                     AF.Square)
                                s1 = ps.tile([128, RW * F], f32, name="p2s1b",
                                             tag="Ls1")
                                nc.tensor.matmul(s1[:, :n], cw["bo_bf"][:],
                                                 sqw[:, :n],
                                                 start=True, stop=True)
                                inv = sb.tile([128, RW * F], f32, name="p2inv",
                                              tag="Linv")
                                nc.scalar.activation(inv[:, :n], s1[:, :n],
                                                     AF.Abs_reciprocal_sqrt,
                                                     scale=1.0 / 64.0)
                                nc.vector.tensor_tensor(
                                    n2b[:, r0:r0 + rr, :].rearrange(
                                        "p a b -> p (a b)"),
                                    w1f[:, :n], inv[:, :n], OP.mult)
                        dst = AP(n2_d, i * C * RP * F + t0 * F,
                                 [[w * F, 2], [RP * F, C], [F, w], [1, F]])
                        nc.scalar.dma_start(dst, n2b[:])
                        # gap2 partials over strictly-local rows [PADR, PADR+TH)
                        rng = []
                        for q in range(2):
                            a = max(PADR - (t0 + q * w), 0)
                            bq = min(PADR + TH - (t0 + q * w), w)
                            rng.append((a, bq))
                        if rng[0] == (0, w) and rng[1] == (0, w):
                            red = sb.tile([128, F], f32, name="p2red",
                                          tag="p2red")
                            nc.vector.tensor_reduce(
                                red[:], n2b[:].transpose([0, 2, 1]),
                                AX.X, OP.add)
                            nc.vector.tensor_tensor(g2acc[i][:], g2acc[i][:],
                                                    red[:], OP.add)
                        else:
                            for q in range(2):
                                a, bq = rng[q]
                                if bq <= a:
                                    continue
                                p0, p1 = q * 64, q * 64 + 64
                                redh = sb.tile([128, F], f32, name="p2redh",
                                               tag="p2red")
                                nc.vector.tensor_reduce(
                                    redh[p0:p1],
                                    n2b[p0:p1, a:bq, :]
                                    .transpose([0, 2, 1]),
                                    AX.X, OP.add)
                                nc.vector.tensor_tensor(g2acc[i][p0:p1],
                                                        g2acc[i][p0:p1],
                                                        redh[p0:p1], OP.add)
            for i in range(3):
                nc.sync.dma_start(AP(ag2i, i * 128 * F, [[1, 128 * F]]),
                                  g2acc[i][:])
            nc.gpsimd.collective_compute(
                "AllGather", OP.bypass, replica_groups=groups,
                ins=[ag2i.ap()], outs=[ag2o.ap()])

            # filt folds vertical + gterm2
            gt2 = []
            with tc.tile_pool(name="f2sb", bufs=2) as sb, \
                 tc.tile_pool(name="f2ps", bufs=2, space="PSUM") as ps:
                for i in range(3):
                    ga = sb.tile([128, F], f32, name=f"f2ga_{i}", tag="f2ga")
                    gb = sb.tile([128, F], f32, name=f"f2gb_{i}", tag="f2gb")
                    nc.sync.dma_start(ga[:], AP(ag2o, i * 128 * F,
                                                [[F, 128], [1, F]]))
                    nc.sync.dma_start(gb[:], AP(ag2o, 3 * 128 * F + i * 128 * F,
                                                [[F, 128], [1, F]]))
                    gf = per.tile([128, F], f32, name=f"g2full_{i}")
                    nc.vector.tensor_tensor(gf[:], ga[:], gb[:], OP.add)
                    nsum = sb.tile([128, 1], f32, name=f"f2ns_{i}", tag="f2ns")
                    nc.vector.tensor_reduce(nsum[:], gf[:], AX.X, OP.add)
                    n64p = ps.tile([64, 1], f32, name=f"f2n64_{i}", tag="f2n64")
                    nc.tensor.matmul(n64p[:], cw["fold64"][:], nsum[:],
                                     start=True, stop=True)
                    n64 = sb.tile([64, 1], f32, name=f"f2n64s_{i}", tag="f2n64s")
                    nc.vector.tensor_copy(n64[:], n64p[:])
                    a_t, kc_t = _filt_fold(nc, sb, ps, per, cw, n64, i + 3,
                                           f32, bf, AF, OP, AX)
                    gt = per.tile([128, F], bf, name=f"gt2_{i}")
                    nc.vector.tensor_scalar(gt[:], gf[:],
                                            cw[f"kg_{i + 3}"][:, 0:1],
                                            kc_t[:, 0:1], OP.mult, OP.add)
                    gt2.append(gt)
                    ataps.append(a_t)
            # vertical taps folded into the mix conv stationaries:
            # (wg_i diag(av_ik))^T = diag(av_ik) wgbd_i
            dgv = []
            for i in range(3):
                for k in range(KT):
                    dt_ = per.tile([128, 128], bf, name=f"dgv_{i}_{k}")
                    nc.vector.tensor_scalar_mul(dt_[:], cw[f"wgbd_{i}"][:],
                                                ataps[3 + i][:, k:k + 1])
                    dgv.append(dt_)
            gt2mix = per.tile([128, F], f32, name="gt2mix")
            with tc.tile_pool(name="gtm", bufs=1, space="PSUM") as gps:
                gp = gps.tile([128, F], f32, name="gt2mp")
                for i in range(3):
                    nc.tensor.matmul(gp[:], cw[f"wgbd_{i}"][:], gt2[i][:],
                                     start=(i == 0), stop=(i == 2))
                nc.vector.tensor_copy(gt2mix[:], gp[:])

        if phases >= 3:
            # ============================ P3 ================================
            with tc.tile_pool(name="p3sb", bufs=3) as sb, \
                 tc.tile_pool(name="p3b2", bufs=2) as b2p, \
                 tc.tile_pool(name="p3ps", bufs=2, space="PSUM") as ps:
                for (t0, w) in _tiles(TH, 20):
                    x2 = x2_load(sb, PADR + t0, w, "p3x", eng=nc.scalar)
                    n2ws = []
                    for i in range(3):
                        n2w = b2p.tile([128, w + 14, F], bf, name=f"p3n2w_{i}",
                                       tag=f"p3n2w_{i}")
                        src = AP(n2_d, i * C * RP * F + t0 * F,
                                 [[w * F, 2], [RP * F, C], [F, w + 14], [1, F]])
                        (nc.gpsimd if i == 1 else nc.sync).dma_start(n2w[:],
                                                                     src)
                        n2ws.append(n2w[:].rearrange("p a b -> p (a b)"))
                    yt = sb.tile([128, w, F], bf, name="p3y", tag="p3y")
                    chs = _chunks(w)
                    for g0 in range(0, len(chs), 4):
                        grp = chs[g0:g0 + 4]
                        yps = [ps.tile([128, RW * F], f32, name=f"p3yp{cj}",
                                       tag=f"p3yp{cj}")
                               for cj in range(len(grp))]
                        for cj, (r0, rr) in enumerate(grp):
                            nc.tensor.matmul(
                                yps[cj][:, :rr * F], cw["wbbd"][:],
                                x2[:, r0:r0 + rr, :]
                                .rearrange("p a b -> p (a b)"),
                                start=True, stop=False)
                        for i in range(3):
                            d = DILS[i]
                            for k in range(KT):
                                last = (i == 2 and k == 2)
                                for cj, (r0, rr) in enumerate(grp):
                                    off = (PADR + r0 + (k - 1) * d) * F
                                    nc.tensor.matmul(
                                        yps[cj][:, :rr * F],
                                        dgv[i * KT + k][:],
                                        n2ws[i][:, off:off + rr * F],
                                        start=False, stop=last)
                        for cj, (r0, rr) in enumerate(grp):
                            n = rr * F
                            acc = sb.tile([128, 1], f32, name="p3acc",
                                          tag="p3acc")
                            nc.vector.scalar_tensor_tensor(
                                yt[:, r0:r0 + rr, :], yps[cj][:, :n]
                                .rearrange("p (a b) -> p a b", b=F),
                                1.0,
                                gt2mix[:].unsqueeze(1)
                                .broadcast_to([128, rr, F]),
                                OP.mult, OP.add, accum_out=acc[:])
                            nc.vector.tensor_tensor(ysacc[:], ysacc[:],
                                                    acc[:], OP.add)
                            sqy = sb.tile([128, RW * F], f32, name="p3sqy",
                                          tag="p3sqy")
                            acq = sb.tile([128, 1], f32, name="p3acq",
                                          tag="p3acq")
                            nc.scalar.activation(
                                sqy[:, :n],
                                yt[:, r0:r0 + rr, :]
                                .rearrange("p a b -> p (a b)"),
                                AF.Square, accum_out=acq[:])
                            nc.vector.tensor_tensor(yqacc[:], yqacc[:],
                                                    acq[:], OP.add)
                    dst = AP(y_d, t0 * F,
                             [[w * F, 2], [TH * F, C], [F, w], [1, F]])
                    nc.scalar.dma_start(dst, yt[:])
            pk = per.tile([128, 2], f32, name="pk")
            nc.vector.tensor_copy(pk[:, 0:1], ysacc[:])
            nc.vector.tensor_copy(pk[:, 1:2], yqacc[:])
            nc.sync.dma_start(AP(ag3i, 0, [[1, 256]]), pk[:])
            nc.gpsimd.collective_compute(
                "AllGather", OP.bypass, replica_groups=groups,
                ins=[ag3i.ap()], outs=[ag3o.ap()])

            # GN scalars (y in y_d excludes convb_b; fold it analytically)
            gnS = per.tile([128, 1], f32, name="gnS")
            gnB = per.tile([128, 1], f32, name="gnB")
            with tc.tile_pool(name="gnsb", bufs=2) as sb, \
                 tc.tile_pool(name="gnps", bufs=2, space="PSUM") as ps:
                pa_ = sb.tile([128, 4], f32, name="gn_pa")
                nc.sync.dma_start(pa_[:, 0:2], AP(ag3o, 0, [[2, 128], [1, 2]]))
                nc.sync.dma_start(pa_[:, 2:4], AP(ag3o, 256, [[2, 128], [1, 2]]))
                sy = sb.tile([128, 1], f32, name="gn_sy")
                sq = sb.tile([128, 1], f32, name="gn_sq")
                nc.vector.tensor_tensor(sy[:], pa_[:, 0:1], pa_[:, 2:3], OP.add)
                nc.vector.tensor_tensor(sq[:], pa_[:, 1:2], pa_[:, 3:4], OP.add)
                NcF = float(TH * F)
                t1 = sb.tile([128, 1], f32, name="gn_t1")
                nc.vector.scalar_tensor_tensor(t1[:], sy[:], 2.0,
                                               cw["cbb"][:], OP.mult, OP.mult)
                nc.vector.tensor_tensor(sq[:], sq[:], t1[:], OP.add)
                cb2 = sb.tile([128, 1], f32, name="gn_cb2")
                nc.scalar.activation(cb2[:], cw["cbb"][:], AF.Square,
                                     scale=1.0)
                nc.vector.tensor_scalar(cb2[:], cb2[:], NcF, None, OP.mult)
                nc.vector.tensor_tensor(sq[:], sq[:], cb2[:], OP.add)
                nc.vector.scalar_tensor_tensor(t1[:], cw["cbb"][:], NcF, sy[:],
                                               OP.mult, OP.add)
                on1 = sb.tile([128, 1], f32, name="gn_on1")
                nc.vector.memset(on1[:], 1.0)
                tots = ps.tile([1, 2], f32, name="gn_tots")
                nc.tensor.matmul(tots[0:1, 0:1], on1[:], t1[:],
                                 start=True, stop=True)
                nc.tensor.matmul(tots[0:1, 1:2], on1[:], sq[:],
                                 start=True, stop=True)
                Ntot = float(C * T * F)
                mg = sb.tile([1, 1], f32, name="gn_mg")
                nc.vector.tensor_scalar(mg[:], tots[0:1, 0:1], 1.0 / Ntot, None,
                                        OP.mult)
                m2g = sb.tile([1, 1], f32, name="gn_m2g")
                nc.scalar.activation(m2g[:], mg[:], AF.Square)
                vg = sb.tile([1, 1], f32, name="gn_vg")
                nc.vector.scalar_tensor_tensor(vg[:], tots[0:1, 1:2], 1.0 / Ntot,
                                               m2g[:], OP.mult, OP.subtract)
                nc.vector.tensor_scalar(vg[:], vg[:], EPS, None, OP.add)
                ig = sb.tile([1, 1], f32, name="gn_ig")
                nc.scalar.activation(ig[:], vg[:], AF.Abs_reciprocal_sqrt)
                igb = sb.tile([128, 1], f32, name="gn_igb")
                mgb = sb.tile([128, 1], f32, name="gn_mgb")
                nc.gpsimd.partition_broadcast(igb[:], ig[:])
                nc.gpsimd.partition_broadcast(mgb[:], mg[:])
                nc.vector.tensor_tensor(gnS[:], igb[:], cw["gn_g"][:], OP.mult)
                nc.vector.tensor_tensor(gnB[:], cw["cbb"][:], mgb[:],
                                        OP.subtract)
                nc.vector.tensor_tensor(gnB[:], gnB[:], gnS[:], OP.mult)
                nc.vector.tensor_tensor(gnB[:], gnB[:], cw["gn_b"][:], OP.add)

        if phases >= 4:
            # ============================ P4 ================================
            with tc.tile_pool(name="p4sb", bufs=2) as sb, \
                 tc.tile_pool(name="p4ps", bufs=2, space="PSUM") as ps, \
                 tc.tile_pool(name="p4st", bufs=1, space="PSUM") as pst:
                for (t0, w) in _tiles(TH, 20):
                    y2 = sb.tile([128, w, F], bf, name="p4y", tag="p4y")
                    nc.sync.dma_start(
                        y2[:], AP(y_d, t0 * F,
                                  [[w * F, 2], [TH * F, C], [F, w], [1, F]]))
                    ot = sb.tile([128, w, F], bf, name="p4o", tag="p4o")
                    nc.scalar.activation(ot[:], y2[:], AF.Prelu,
                                         bias=gnB[:, 0:1], scale=gnS[:, 0:1],
                                         alpha=cw["a0vec"][:, 0:1])
                    nc.gpsimd.dma_start(
                        AP(out_d, t0 * F,
                           [[w * F, 2], [TH * F, C], [F, w], [1, F]]), ot[:])
                    qk = sb.tile([64, w, F], bf, name="p4qk", tag="p4qk")
                    vt = sb.tile([128, w, F], bf, name="p4v", tag="p4v")
                    for (r0, rr) in _chunks(w):
                        n = rr * F
                        oc = ot[:, r0:r0 + rr, :].rearrange("p a b -> p (a b)")
                        qps = ps.tile([64, RW * F], f32, name="p4qps",
                                      tag="p4qps")
                        vps = ps.tile([128, RW * F], f32, name="p4vps",
                                      tag="p4vps")
                        nc.tensor.matmul(qps[:, :n], cw["wqkbd"][:], oc,
                                         start=True, stop=True)
                        nc.tensor.matmul(vps[:, :n], cw["wvbd"][:], oc,
                                         start=True, stop=True)
                        nc.scalar.activation(
                            qk[:, r0:r0 + rr, :].rearrange("p a b -> p (a b)"),
                            qps[:, :n], AF.Prelu, bias=cw["qkb"][:, 0:1],
                            alpha=cw["qka"][:, 0:1])
                        nc.scalar.activation(
                            vt[:, r0:r0 + rr, :].rearrange("p a b -> p (a b)"),
                            vps[:, :n], AF.Prelu, bias=cw["vb"][:, 0:1],
                            alpha=cw["va"][:, 0:1])
                    qs = sb.tile([64, w], f32, name="p4qs", tag="p4qs")
                    vs = sb.tile([128, w], f32, name="p4vs", tag="p4vs")
                    nc.vector.tensor_reduce(qs[:], qk[:], AX.X, OP.add)
                    nc.vector.tensor_reduce(vs[:], vt[:], AX.X, OP.add)
                    qq = sb.tile([64, w, F], f32, name="p4qq", tag="p4qq")
                    vv = sb.tile([128, w, F], f32, name="p4vv", tag="p4vv")
                    nc.scalar.activation(qq[:], qk[:], AF.Square)
                    nc.scalar.activation(vv[:], vt[:], AF.Square)
                    qsq = sb.tile([64, w], f32, name="p4qsq", tag="p4qsq")
                    vsq = sb.tile([128, w], f32, name="p4vsq", tag="p4vsq")
                    nc.vector.tensor_reduce(qsq[:], qq[:], AX.X, OP.add)
                    nc.vector.tensor_reduce(vsq[:], vv[:], AX.X, OP.add)
                    stq = pst.tile([16, 2 * w], f32, name="p4stq", tag="p4stq")
                    stv = pst.tile([8, 2 * w], f32, name="p4stv", tag="p4stv")
                    nc.tensor.matmul(stq[:, 0:w], cw["grp16"][:], qs[:],
                                     start=True, stop=True)
                    nc.tensor.matmul(stq[:, w:2 * w], cw["grp16"][:], qsq[:],
                                     start=True, stop=True)
                    nc.tensor.matmul(stv[:, 0:w], cw["grp8"][:], vs[:],
                                     start=True, stop=True)
                    nc.tensor.matmul(stv[:, w:2 * w], cw["grp8"][:], vsq[:],
                                     start=True, stop=True)
                    mivs = {}
                    for (st, npart, ncnt, nm) in ((stq, 16, 4 * F, "q"),
                                                  (stv, 8, 16 * F, "v")):
                        mu = sb.tile([npart, w], f32, name=f"p4mu{nm}",
                                     tag=f"p4mu{nm}")
                        nc.vector.tensor_scalar(mu[:], st[:, 0:w], 1.0 / ncnt,
                                                None, OP.mult)
                        m2_ = sb.tile([npart, w], f32, name=f"p4m2{nm}",
                                      tag=f"p4m2{nm}")
                        nc.scalar.activation(m2_[:], mu[:], AF.Square)
                        var = sb.tile([npart, w], f32, name=f"p4var{nm}",
                                      tag=f"p4var{nm}")
                        nc.vector.scalar_tensor_tensor(var[:], st[:, w:2 * w],
                                                       1.0 / ncnt, m2_[:],
                                                       OP.mult, OP.subtract)
                        nc.vector.tensor_scalar(var[:], var[:], EPS, None,
                                                OP.add)
                        iv = sb.tile([npart, w], f32, name=f"p4iv{nm}",
                                     tag=f"p4iv{nm}")
                        nc.scalar.activation(iv[:], var[:], AF.Abs_reciprocal_sqrt)
                        mivs[nm] = (mu, iv)
                    qb_ = pst.tile([64, 2 * w], f32, name="p4qb", tag="p4qb")
                    vb_ = pst.tile([128, 2 * w], f32, name="p4vb", tag="p4vb")
                    nc.tensor.matmul(qb_[:, 0:w], cw["bc16"][:], mivs["q"][0][:],
                                     start=True, stop=True)
                    nc.tensor.matmul(qb_[:, w:2 * w], cw["bc16"][:],
                                     mivs["q"][1][:], start=True, stop=True)
                    nc.tensor.matmul(vb_[:, 0:w], cw["bc8"][:], mivs["v"][0][:],
                                     start=True, stop=True)
                    nc.tensor.matmul(vb_[:, w:2 * w], cw["bc8"][:],
                                     mivs["v"][1][:], start=True, stop=True)
                    qkn = sb.tile([64, w, F], bf, name="p4qkn", tag="p4qkn")
                    vn = sb.tile([128, w, F], bf, name="p4vn", tag="p4vn")
                    nc.vector.tensor_tensor(
                        qkn[:], qk[:],
                        qb_[:, 0:w].unsqueeze(2).broadcast_to([64, w, F]),
                        OP.subtract)
                    nc.vector.tensor_tensor(
                        qkn[:], qkn[:],
                        qb_[:, w:2 * w].unsqueeze(2).broadcast_to([64, w, F]),
                        OP.mult)
                    nc.vector.tensor_tensor(
                        vn[:], vt[:],
                        vb_[:, 0:w].unsqueeze(2).broadcast_to([128, w, F]),
                        OP.subtract)
                    nc.vector.tensor_tensor(
                        vn[:], vn[:],
                        vb_[:, w:2 * w].unsqueeze(2).broadcast_to([128, w, F]),
                        OP.mult)
                    nc.sync.dma_start(
                        AP(qkvh_d, t0 * F,
                           [[w * F, 2], [TH * F, 32], [F, w], [1, F]]), qkn[:])
                    nc.gpsimd.dma_start(
                        AP(qkvh_d, 32 * TH * F + t0 * F,
                           [[w * F, 2], [TH * F, 64], [F, w], [1, F]]), vn[:])
            nc.gpsimd.collective_compute(
                "AllGather", OP.bypass, replica_groups=groups,
                ins=[AP(qkvh_d, 16 * TH * F, [[1, 80 * TH * F]])],
                outs=[qkvf_d.ap()])

        if phases >= 5:
            # ============================ P5 ================================
            KCH = [(0, 128), (128, 128), (256, 128), (384, 128), (512, 4)]
            NCH = [(0, 512), (512, 512), (1024, 512), (1536, 512), (2048, 16)]
            import contextlib as _cl
            for h in range(H):
                hx = _cl.ExitStack()
                sb = hx.enter_context(tc.tile_pool(name=f"a{h}sb", bufs=3))
                res = hx.enter_context(tc.tile_pool(name=f"a{h}res", bufs=1))
                ps = hx.enter_context(tc.tile_pool(name=f"a{h}ps", bufs=2,
                                                   space="PSUM"))
                pss = hx.enter_context(tc.tile_pool(name=f"a{h}pss", bufs=1,
                                                    space="PSUM"))
                pso = hx.enter_context(tc.tile_pool(name=f"a{h}pso", bufs=2,
                                                    space="PSUM"))
                ktr = [res.tile([kn, T], bf, name=f"ktr{h}_{ci}")
                       for ci, (k0, kn) in enumerate(KCH)]
                qtr = [res.tile([kn, TH], bf, name=f"qtr{h}_{ci}")
                       for ci, (k0, kn) in enumerate(KCH)]
                vres = [res.tile([MB, DF], bf, name=f"vres{h}_{b}")
                        for b in range(8)]
                for b in range(8):
                    seg, tl = divmod(b, 4)
                    km = sb.tile([MB, EF], bf, name=f"km{h}", tag="km")
                    nc.sync.dma_start(
                        km[:], AP(qkvf_d,
                                  seg * 80 * TH * F + (h * 4) * TH * F
                                  + tl * MB * F,
                                  [[F, MB], [TH * F, E], [1, F]]))
                    for ci, (k0, kn) in enumerate(KCH):
                        tp = ps.tile([128, MB], bf, name=f"tp{h}", tag="tp")
                        nc.tensor.transpose(tp[:kn, :], km[:, k0:k0 + kn],
                                            cw["ident_bf"][0:MB, 0:MB])
                        nc.vector.tensor_copy(ktr[ci][:, b * MB:(b + 1) * MB],
                                              tp[:kn, :])
                    nc.sync.dma_start(
                        vres[b][:], AP(qkvf_d,
                                       seg * 80 * TH * F + (16 + h * 16) * TH * F
                                       + tl * MB * F,
                                       [[F, MB], [TH * F, D], [1, F]]))
                for b in range(4):
                    km = sb.tile([MB, EF], bf, name=f"qm{h}", tag="km")
                    nc.sync.dma_start(
                        km[:], AP(qkvh_d, (h * 4) * TH * F + b * MB * F,
                                  [[F, MB], [TH * F, E], [1, F]]))
                    for ci, (k0, kn) in enumerate(KCH):
                        tp = ps.tile([128, MB], bf, name=f"tp{h}", tag="tp")
                        nc.tensor.transpose(tp[:kn, :], km[:, k0:k0 + kn],
                                            cw["ident_bf"][0:MB, 0:MB])
                        nc.vector.tensor_copy(qtr[ci][:, b * MB:(b + 1) * MB],
                                              tp[:kn, :])
                # lng/lnb per-partition post-transpose
                for ci, (k0, kn) in enumerate(KCH):
                    for (lg, lb, tt) in ((("klngT"), ("klnbT"), ktr),
                                         (("qlngT"), ("qlnbT"), qtr)):
                        vg_ = sb.tile([128, 1], f32, name=f"lg{h}", tag="lg")
                        vb2 = sb.tile([128, 1], f32, name=f"lb{h}", tag="lb")
                        nc.sync.dma_start(vg_[:kn, :],
                                          AP(fw[lg], h * 640 + k0, [[1, kn]]))
                        nc.sync.dma_start(vb2[:kn, :],
                                          AP(fw[lb], h * 640 + k0, [[1, kn]]))
                        nc.vector.tensor_scalar(tt[ci][:], tt[ci][:],
                                                vg_[:kn, 0:1], vb2[:kn, 0:1],
                                                OP.mult, OP.add)
                lngb = res.tile([MB, DF], f32, name=f"lngb{h}")
                lrow = sb.tile([1, DF], f32, name=f"lrow{h}", tag="lrow")
                nc.sync.dma_start(lrow[:], AP(fw["vlngT"], h * DF, [[DF, 1],
                                                                   [1, DF]]))
                nc.gpsimd.partition_broadcast(lngb[:], lrow[:])
                for mt in range(4):
                    spt = pss.tile([MB, 1024], f32, name=f"spt{h}", tag="spt")
                    for ci, (k0, kn) in enumerate(KCH):
                        lhs = qtr[ci][:, mt * MB:(mt + 1) * MB]
                        nc.tensor.matmul(spt[:, 0:500], lhs, ktr[ci][:, 0:500],
                                         start=(ci == 0), stop=(ci == 4))
                        nc.tensor.matmul(spt[:, 512:1012], lhs,
                                         ktr[ci][:, 500:1000],
                                         start=(ci == 0), stop=(ci == 4))
                    sview = spt[:].rearrange("p (a b) -> p a b", b=512)[:, :, 0:500]
                    mx = sb.tile([MB, 1], f32, name=f"mx{h}", tag="mx")
                    nc.vector.tensor_reduce(mx[:], sview, AX.XY, OP.max)
                    bias = sb.tile([MB, 1], f32, name=f"bias{h}", tag="bias")
                    nc.vector.tensor_scalar(bias[:], mx[:], -ISCALE, None,
                                            OP.mult)
                    pexp = sb.tile([MB, 1000], bf, name=f"pexp{h}", tag="pexp")
                    lsum = sb.tile([MB, 1], f32, name=f"lsum{h}", tag="lsum")
                    nc.scalar.activation(pexp[:], sview, AF.Exp,
                                         bias=bias[:, 0:1], scale=ISCALE,
                                         accum_out=lsum[:])
                    linv = sb.tile([MB, 1], f32, name=f"linv{h}", tag="linv")
                    nc.vector.reciprocal(linv[:], lsum[:])
                    ptr = []
                    for b in range(8):
                        tp2 = ps.tile([MB, MB], bf, name=f"tp2{h}", tag="tp2")
                        nc.tensor.transpose(tp2[:], pexp[:, b * MB:(b + 1) * MB],
                                            cw["ident_bf"][0:MB, 0:MB])
                        pb_ = sb.tile([MB, MB], bf, name=f"ptr{h}_{b}",
                                      tag=f"ptr{b}")
                        nc.vector.tensor_copy(pb_[:], tp2[:])
                        ptr.append(pb_)
                    ob = sb.tile([MB, DF], bf, name=f"ob{h}", tag="ob")
                    for (n0, nn) in NCH:
                        op_ = pso.tile([MB, 512], f32, name=f"op{h}", tag="op")
                        for b in range(8):
                            nc.tensor.matmul(op_[:, :nn], ptr[b][:],
                                             vres[b][:, n0:n0 + nn],
                                             start=(b == 0), stop=(b == 7))
                        nc.vector.scalar_tensor_tensor(
                            ob[:, n0:n0 + nn], op_[:, :nn], linv[:, 0:1],
                            lngb[:, n0:n0 + nn], OP.mult, OP.mult)
                    nc.scalar.dma_start(
                        AP(o_d, mt * MB * H * D * F + h * D * F,
                           [[H * D * F, MB], [1, D * F]]), ob[:])
                hx.close()

        if phases >= 6:
            # ============================ P6 ================================
            with tc.tile_pool(name="p6sb", bufs=2) as sb, \
                 tc.tile_pool(name="p6ps", bufs=2, space="PSUM") as ps, \
                 tc.tile_pool(name="p6st", bufs=1, space="PSUM") as pst:
                for (t0, w) in _tiles(TH, 20):
                    o2 = sb.tile([128, w, F], bf, name="p6o", tag="p6o")
                    for q, eng in ((0, nc.sync), (1, nc.gpsimd)):
                        eng.dma_start(
                            o2[q * 64:(q + 1) * 64],
                            AP(o_d, (t0 + q * w) * H * D * F,
                               [[F, 64], [H * D * F, w], [1, F]]))
                    u2 = sb.tile([128, w, F], f32, name="p6u2", tag="p6u2")
                    for (r0, rr) in _chunks(w):
                        n = rr * F
                        pps = ps.tile([128, RW * F], f32, name="p6pps",
                                      tag="p6pps")
                        nc.tensor.matmul(
                            pps[:, :n], cw["pwbd"][:],
                            o2[:, r0:r0 + rr, :].rearrange("p a b -> p (a b)"),
                            start=True, stop=True)
                        tmp = sb.tile([128, RW, F], f32, name="p6tmp",
                                      tag="p6tmp")
                        nc.vector.tensor_tensor(
                            tmp[:, :rr, :],
                            pps[:, :n].rearrange("p (a b) -> p a b", b=F),
                            cw["pconst"][:].unsqueeze(1)
                            .broadcast_to([128, rr, F]),
                            OP.add)
                        nc.scalar.activation(
                            u2[:, r0:r0 + rr, :].rearrange("p a b -> p (a b)"),
                            tmp[:, :rr, :].rearrange("p a b -> p (a b)"),
                            AF.Prelu, bias=cw["pb"][:, 0:1],
                            alpha=cw["pa"][:, 0:1])
                    us = sb.tile([128, w], f32, name="p6us", tag="p6us")
                    nc.vector.tensor_reduce(us[:], u2[:], AX.X, OP.add)
                    uq = sb.tile([128, w, F], f32, name="p6uq", tag="p6uq")
                    nc.scalar.activation(uq[:], u2[:], AF.Square)
                    usq = sb.tile([128, w], f32, name="p6usq", tag="p6usq")
                    nc.vector.tensor_reduce(usq[:], uq[:], AX.X, OP.add)
                    st2 = pst.tile([2, 2 * w], f32, name="p6st2", tag="p6st2")
                    nc.tensor.matmul(st2[:, 0:w], cw["redq"][:], us[:],
                                     start=True, stop=True)
                    nc.tensor.matmul(st2[:, w:2 * w], cw["redq"][:], usq[:],
                                     start=True, stop=True)
                    ncnt = float(64 * F)
                    mu2 = sb.tile([2, w], f32, name="p6mu2", tag="p6mu2")
                    nc.vector.tensor_scalar(mu2[:], st2[:, 0:w], 1.0 / ncnt,
                                            None, OP.mult)
                    m22 = sb.tile([2, w], f32, name="p6m22", tag="p6m22")
                    nc.scalar.activation(m22[:], mu2[:], AF.Square)
                    var2 = sb.tile([2, w], f32, name="p6var2", tag="p6var2")
                    nc.vector.scalar_tensor_tensor(var2[:], st2[:, w:2 * w],
                                                   1.0 / ncnt, m22[:],
                                                   OP.mult, OP.subtract)
                    nc.vector.tensor_scalar(var2[:], var2[:], EPS, None, OP.add)
                    iv2 = sb.tile([2, w], f32, name="p6iv2", tag="p6iv2")
                    nc.scalar.activation(iv2[:], var2[:], AF.Abs_reciprocal_sqrt)
                    mb2 = pst.tile([128, 2 * w], f32, name="p6mb2", tag="p6mb2")
                    nc.tensor.matmul(mb2[:, 0:w], cw["ind2f"][:], mu2[:],
                                     start=True, stop=True)
                    nc.tensor.matmul(mb2[:, w:2 * w], cw["ind2f"][:], iv2[:],
                                     start=True, stop=True)
                    nc.vector.tensor_tensor(
                        u2[:], u2[:],
                        mb2[:, 0:w].unsqueeze(2).broadcast_to([128, w, F]),
                        OP.subtract)
                    nc.vector.tensor_tensor(
                        u2[:], u2[:],
                        mb2[:, w:2 * w].unsqueeze(2).broadcast_to([128, w, F]),
                        OP.mult)
                    nc.vector.tensor_tensor(
                        u2[:], u2[:],
                        cw["plng"][:].unsqueeze(1).broadcast_to([128, w, F]),
                        OP.mult)
                    rt = sb.tile([128, w, F], bf, name="p6rt", tag="p6rt")
                    nc.sync.dma_start(
                        rt[:], AP(out_d, t0 * F,
                                  [[w * F, 2], [TH * F, C], [F, w], [1, F]]))
                    r1 = sb.tile([128, w, F], f32, name="p6r1", tag="p6r1")
                    nc.vector.tensor_tensor(
                        r1[:], rt[:],
                        cw["plnb"][:].unsqueeze(1).broadcast_to([128, w, F]),
                        OP.add)
                    fint = sb.tile([128, w, F], bf, name="p6fin", tag="p6fin")
                    nc.vector.tensor_tensor(fint[:], u2[:], r1[:], OP.add)
                    nc.scalar.dma_start(
                        AP(fin, t0 * F,
                           [[w * F, 2], [TH * F, C], [F, w], [1, F]]), fint[:])
        if phases < 6:
            with tc.tile_pool(name="dummy", bufs=1) as dp:
                zt = dp.tile([C, 16], bf, name="zfin")
                nc.vector.memset(zt[:], 0.0)
                nc.sync.dma_start(AP(fin, 0, [[TH * F, C], [1, 16]]), zt[:])
        ctx.close()
    nc.compile()
    return nc


def _filt_fold(nc, sb, ps, per, cw, mean64, i, f32, bf, AF, OP, AX):
    u1 = sb.tile([64, 1], f32, name=f"u1_{i}", tag="ffu1")
    nc.vector.tensor_scalar(u1[:], mean64[:], cw[f"gtf_{i}"][:, 0:1],
                            cw[f"c64_{i}"][:, 0:1], OP.mult, OP.add)
    ftp = ps.tile([1, 12], f32, name=f"ftp_{i}", tag="ffftp")
    nc.tensor.matmul(ftp[:], u1[:], cw[f"lwT_{i}"][:], start=True, stop=True)
    ft = sb.tile([1, 12], f32, name=f"ft_{i}", tag="ffft")
    nc.scalar.activation(ft[:], ftp[:], AF.Tanh)
    ft4 = sb.tile([4, 3], f32, name=f"ft4_{i}", tag="ffft4")
    nc.sync.dma_start(ft4[:], ft[:].rearrange("o (g k) -> (o g) k", g=4))
    wcp = ps.tile([128, 3], f32, name=f"wcp_{i}", tag="ffwcp")
    nc.tensor.matmul(wcp[:], cw["grp4"][:], ft4[:], start=True, stop=True)
    atap = per.tile([128, 3], f32, name=f"atap_{i}")
    nc.vector.tensor_scalar_mul(atap[:], wcp[:], cw[f"gs_{i}"][:, 0:1])
    nc.vector.tensor_tensor(atap[:, 1:2], atap[:, 1:2],
                            cw[f"hc_{i}"][:, 0:1], OP.add)
    wcs = sb.tile([128, 1], f32, name=f"wcs_{i}", tag="ffwcs")
    nc.vector.tensor_reduce(wcs[:], wcp[:], AX.X, OP.add)
    kc = per.tile([128, 1], f32, name=f"kc_{i}")
    nc.vector.tensor_scalar(kc[:], wcs[:], cw[f"cs_{i}"][:, 0:1],
                            cw[f"cb_{i}"][:, 0:1], OP.mult, OP.add)
    nc.vector.tensor_tensor(kc[:], kc[:], cw[f"gc_{i}"][:, 0:1], OP.add)
    return atap, kc


# ---------------------------------------------------------------------------
# host entry
# ---------------------------------------------------------------------------

def _prep_inputs(inputs, fold):
    import ml_dtypes
    x = np.asarray(inputs["x"], np.float32)
    in_maps = []
    for c in range(8):
        s, hf = divmod(c, 2)
        xs = np.pad(x[s], ((0, 0), (PADR, PADR), (0, 0)), mode="reflect")
        xc = xs[:, hf * TH:hf * TH + RP, :]
        m = {"x_d": np.ascontiguousarray(xc).astype(ml_dtypes.bfloat16)}
        for n, (sh, isbf) in _fold_shapes().items():
            v = fold[n]
            m[n] = v.astype(ml_dtypes.bfloat16) if isbf else v
        for n in _dram_only_shapes():
            m[n] = fold[n]
        in_maps.append(m)
    return in_maps


def kernel(**inputs):
    import os
    from concourse.bass_utils import run_bass_kernel_spmd
    global LAST_EXEC_NS
    if "nc" not in _CACHE:
        _CACHE["nc"] = _build(dbg=_CACHE.get("dbg", ()),
                              phases=int(os.environ.get("KPHASES", "6")))
    nc = _CACHE["nc"]
    fold = _fold_weights(inputs)
    in_maps = _prep_inputs(inputs, fold)
    kw = {}
    if os.environ.get("KTRACE"):
        import tempfile
        base = os.environ.get("KTRACE_DIR",
                              os.path.join(os.getcwd(), "work"))
        os.makedirs(base, exist_ok=True)
        tdir = tempfile.mkdtemp(prefix="trace_", dir=base)
        with open(os.path.join(base, "last_trace_path.txt"), "w") as f:
            f.write(tdir)
        tc_ = os.environ.get("KTRACE_CORES", "0")
        kw = dict(trace=True, tmpdir=tdir,
                  trace_cores=[int(c) for c in tc_.split(",")])
    res = run_bass_kernel_spmd(nc, in_maps, core_ids=list(range(8)), **kw)
    _CACHE["last"] = res
    if getattr(res, "exec_time_ns", None):
        LAST_EXEC_NS = res.exec_time_ns
    out = np.zeros((B, C, T, F), np.float32)
    for c in range(8):
        s, hf = divmod(c, 2)
        out[s][:, hf * TH:(hf + 1) * TH, :] = \
            res.results[c]["fin"].astype(np.float32)
    return out



# revision 49
# speedup vs baseline: 1.7049x; 1.1492x over previous
"""GridNetBlock_Att Trainium2 kernel (Bass/Tile, 8 NeuronCores).

Core c handles sample s=c//2, T-half h=c%2 (rows [500h, 500h+500)).
Pre-attention is T-split per core (host supplies x with a reflect-padded
t-halo of 7 rows, which makes the SPMD program identical on all cores);
tiny pair AllGathers combine global stats, and one pair AllGather
exchanges the K/V halves before full-sequence attention.

Pre-attention tiles are "2-stack": 128 partitions = 2 consecutive
row-blocks x 64 channels.  LN-over-channels per psum chunk:
    w1  = (I - BO/64) @ x        (PE; BO = block-ones)    = x - mu
    sqw = Square(w1)             (ACT, psum->sbuf)
    s1  = BO @ sqw               (PE)                     = 64*var
    inv = AbsRecipSqrt(s1/64)    (ACT)                    = 1/sqrt(var)
    z   = w1 * inv               (DVE, psum x sbuf -> bf16)
"""
import time

import numpy as np

EPS = 1e-5
B, C, T, F = 4, 64, 1000, 129
H, E, D = 4, 4, 16
GROUP, KT = 4, 3
DILS = (3, 5, 7)
EF, DF = E * F, D * F      # 516, 2064
TH = T // 2                 # 500 local rows
PADR = 7
RP = TH + 2 * PADR          # 514
FP = F + 14                 # 143
TF = float(T * F)
RW = 3                      # rows per psum chunk
ISCALE = float(1.0 / np.sqrt(EF))
MB = 125                    # attention row block

_CACHE = {}
LAST_EXEC_NS = -1


def _tiles(total, w):
    out, t0 = [], 0
    while t0 < total:
        ww = min(2 * w, total - t0) // 2
        out.append((t0, ww))
        t0 += 2 * ww
    return out


def _chunks(w, rw=RW):
    return [(ci * rw, min(rw, w - ci * rw)) for ci in range((w + rw - 1) // rw)]


# ---------------------------------------------------------------------------
# host-side weight folding
# ---------------------------------------------------------------------------

def _fold_shapes():
    sh = {
        "bo_f": ((128, 128), False),
        "bo_bf": ((128, 128), True),
        "w1m_bf": ((128, 128), True),
        "ind2f": ((2, 128), False), "ident_bf": ((128, 128), True),
        "fold64": ((128, 64), False), "redq": ((128, 2), False),
        "grp4": ((4, 128), False),
        "wbbd": ((128, 128), True), "cbb": ((128, 1), False),
        "gn_g": ((128, 1), False), "gn_b": ((128, 1), False),
        "a0vec": ((128, 1), False),
        "wqkbd": ((128, 64), True), "wvbd": ((128, 128), True),
        "qkb": ((64, 1), False), "qka": ((64, 1), False),
        "vb": ((128, 1), False), "va": ((128, 1), False),
        "grp16": ((64, 16), False), "bc16": ((16, 64), False),
        "grp8": ((128, 8), False), "bc8": ((8, 128), False),
        "pwbd": ((128, 128), True), "pconst": ((128, F), False),
        "pb": ((128, 1), False), "pa": ((128, 1), False),
        "plng": ((128, F), False), "plnb": ((128, F), False),
    }
    for i in range(6):
        for nm in ("gs", "hc", "kg", "cs", "cb", "gc"):
            sh[f"{nm}_{i}"] = ((128, 1), False)
        sh[f"lwT_{i}"] = ((64, 12), False)
        sh[f"gtf_{i}"] = ((64, 1), False)
        sh[f"c64_{i}"] = ((64, 1), False)
    for i in range(3):
        sh[f"wgbd_{i}"] = ((128, 128), True)
    return sh


# loaded from DRAM on demand, not staged in SBUF constants
def _dram_only_shapes():
    return {
        "qlngT": (H, 640), "klngT": (H, 640),
        "qlnbT": (H, 640), "klnbT": (H, 640),
        "vlngT": (H, DF),
    }


def _fold_weights(w):
    f32 = np.float32
    g = {}
    ar = lambda a: np.ascontiguousarray(a, f32)
    dup = lambda v: np.tile(ar(v).reshape(64), 2).reshape(128, 1)

    bo = np.zeros((128, 128), f32)
    bo[:64, :64] = 1.0
    bo[64:, 64:] = 1.0
    g["bo_f"] = bo
    g["bo_bf"] = bo
    g["w1m_bf"] = np.eye(128, dtype=f32) - bo / 64.0
    ind2 = np.zeros((2, 128), f32)
    ind2[0, :64] = 1.0
    ind2[1, 64:] = 1.0
    g["ind2f"] = ind2
    g["ident_bf"] = np.eye(128, dtype=f32)
    fold2 = np.zeros((128, 64), f32)
    for p in range(128):
        fold2[p, p % 64] = 1.0
    g["fold64"] = fold2
    redq = np.zeros((128, 2), f32)
    redq[:64, 0] = 1.0
    redq[64:, 1] = 1.0
    g["redq"] = redq
    gi4 = np.zeros((4, 128), f32)
    for p in range(128):
        gi4[(p % 64) // 16, p] = 1.0
    g["grp4"] = gi4

    for i in range(6):
        gg = w["br_g"][i].astype(f32)
        cc = w["br_b"][i].astype(f32)
        ia = w["lisa_in"][i].astype(f32)
        ll = w["lisa_ll"][i].astype(f32)
        lh = w["lisa_lh"][i].astype(f32)
        s = (ia + 1.0) * ll
        gap_div = float(F) if i < 3 else float(T)
        g[f"gs_{i}"] = dup(gg * s)
        g[f"hc_{i}"] = dup(gg * (lh + 1.0))
        g[f"kg_{i}"] = dup((-ia * ll * gg) / gap_div)
        g[f"gc_{i}"] = dup(-ia * ll * cc)
        g[f"cs_{i}"] = dup(cc * s)
        g[f"cb_{i}"] = dup(cc * (lh + 1.0))
        g[f"lwT_{i}"] = ar(w["lisa_w"][i].T)
        g[f"gtf_{i}"] = ar((gg / TF).reshape(64, 1))
        g[f"c64_{i}"] = ar(cc.reshape(64, 1))

    cw_ = w["convb_w"].astype(f32)
    gam = w["mix_gamma"].astype(f32)
    bet = w["mix_beta"].astype(f32)

    def bd(m, no):
        z = np.zeros((128, 2 * no), f32)
        z[:64, :no] = m.T
        z[64:, no:] = m.T
        return z

    for i in range(3):
        g[f"wgbd_{i}"] = bd(cw_ * gam[i][None, :], 64)
    g["wbbd"] = bd(cw_ * bet.sum(0)[None, :], 64)
    g["cbb"] = dup(w["convb_b"])
    g["gn_g"] = dup(w["gn_g"])
    g["gn_b"] = dup(w["gn_b"])
    g["a0vec"] = np.full((128, 1), float(w["convb_a"]), f32)

    wqk = np.concatenate([w["q_w"].astype(f32).reshape(H * E, C),
                          w["k_w"].astype(f32).reshape(H * E, C)], 0)
    g["wqkbd"] = bd(wqk, 32)
    g["wvbd"] = bd(w["v_w"].astype(f32).reshape(H * D, C), 64)
    qkb = np.concatenate([w["q_b"].reshape(-1), w["k_b"].reshape(-1)])
    g["qkb"] = np.tile(ar(qkb), 2).reshape(64, 1)
    qka = np.concatenate([np.repeat(w["q_a"], E), np.repeat(w["k_a"], E)])
    g["qka"] = np.tile(ar(qka), 2).reshape(64, 1)
    g["vb"] = np.tile(ar(w["v_b"].reshape(-1)), 2).reshape(128, 1)
    g["va"] = np.tile(ar(np.repeat(w["v_a"], D)), 2).reshape(128, 1)
    g16 = np.zeros((64, 16), f32)
    for p in range(64):
        q, j = divmod(p, 32)
        g16[p, q * 8 + (j // 16) * 4 + (j % 16) // 4] = 1.0
    g["grp16"] = g16
    g["bc16"] = ar(g16.T)
    g8 = np.zeros((128, 8), f32)
    for p in range(128):
        q, j = divmod(p, 64)
        g8[p, q * 4 + j // 16] = 1.0
    g["grp8"] = g8
    g["bc8"] = ar(g8.T)
    for nm, src in (("qlngT", "q_lng"), ("klngT", "k_lng"),
                    ("qlnbT", "q_lnb"), ("klnbT", "k_lnb")):
        m = np.zeros((H, 640), f32)
        for h in range(H):
            m[h, :EF] = w[src][h].reshape(EF)
        g[nm] = m
    g["vlngT"] = ar(w["v_lng"].reshape(H, DF))
    pw = w["proj_w"].astype(f32)
    g["pwbd"] = bd(pw, 64)
    pconst = pw @ w["v_lnb"].reshape(H * D, F).astype(f32)
    g["pconst"] = np.tile(pconst, (2, 1)).reshape(128, F)
    g["pb"] = dup(w["proj_b"])
    g["pa"] = np.full((128, 1), float(w["proj_a"]), f32)
    g["plng"] = np.tile(w["proj_lng"].astype(f32), (2, 1)).reshape(128, F)
    g["plnb"] = np.tile(w["proj_lnb"].astype(f32), (2, 1)).reshape(128, F)
    return g


# ---------------------------------------------------------------------------
# device program
# ---------------------------------------------------------------------------

def _build(dbg=(), phases=6):
    import concourse.bass as bass
    import concourse.bacc as bacc
    import concourse.mybir as mybir
    from concourse import tile
    from contextlib import ExitStack

    f32 = mybir.dt.float32
    bf = mybir.dt.bfloat16
    AF = mybir.ActivationFunctionType
    OP = mybir.AluOpType
    AX = mybir.AxisListType

    nc = bacc.Bacc("TRN2", target_bir_lowering=False, debug=False,
                   num_devices=8)

    def AP(tensor, offset, dims):
        return bass.AP(tensor=tensor, offset=offset,
                       ap=[list(d) for d in dims])

    shapes = _fold_shapes()
    dshapes = _dram_only_shapes()
    x_d = nc.dram_tensor("x_d", [C, RP, F], bf, kind="ExternalInput")
    fw = {n: nc.dram_tensor(n, list(s), bf if b else f32,
                            kind="ExternalInput")
          for n, (s, b) in shapes.items()}
    for n, s in dshapes.items():
        fw[n] = nc.dram_tensor(n, list(s), f32, kind="ExternalInput")

    def idram(name, shape, dt_):
        kind = "ExternalOutput" if name in dbg else "Internal"
        return nc.dram_tensor(name, list(shape), dt_, kind=kind)

    n2_d = idram("n2_d", [3, C, RP, F], bf)
    y_d = idram("y_d", [C, TH, F], bf)
    out_d = idram("out_d", [C, TH, F], bf)
    qkvh_d = idram("qkvh_d", [96, TH, F], bf)
    qkvf_d = idram("qkvf_d", [2, 80, TH, F], bf)
    o_d = idram("o_d", [TH, H, D, F], bf)
    b1_d = idram("b1_d", [3, C, RP, F], bf) if "b1_d" in dbg else None
    dsm_d = idram("dsm_d", [16, 128], f32) if "dsm_d" in dbg else None
    ag1i = nc.dram_tensor("ag1i", [1, 128], f32)
    ag1o = nc.dram_tensor("ag1o", [2, 128], f32)
    ag2i = nc.dram_tensor("ag2i", [1, 3 * 128 * F], f32)
    ag2o = nc.dram_tensor("ag2o", [2, 3 * 128 * F], f32)
    ag3i = nc.dram_tensor("ag3i", [1, 256], f32)
    ag3o = nc.dram_tensor("ag3o", [2, 256], f32)
    fin = nc.dram_tensor("fin", [C, TH, F], bf, kind="ExternalOutput")

    groups = [[0, 1], [2, 3], [4, 5], [6, 7]]

    with nc.allow_low_precision(reason="bf16 pipeline, tol 2e-2"), \
         tile.TileContext(nc) as tc:
        ctx = ExitStack()
        cst = ctx.enter_context(tc.tile_pool(name="cst", bufs=1))
        per = ctx.enter_context(tc.tile_pool(name="per", bufs=1))

        def load_const(name):
            sh, isbf = shapes[name]
            t = cst.tile(list(sh), bf if isbf else f32, name=f"c_{name}",
                         tag=f"c_{name}")
            nc.sync.dma_start(t[:], fw[name].ap())
            return t

        cw = {n: load_const(n) for n in shapes}

        def x2_load(pool, t0, w, nm, eng=None):
            # SWDGE (gpsimd) sprays descriptors over all 16 SDMA engines;
            # HWDGE fans out per 64-descriptor block (128-descr tile DMAs
            # land on only 2 engines), so bulk tile loads go SWDGE.
            xt = pool.tile([128, w, F], bf, name=nm, tag=nm)
            src = AP(x_d, t0 * F,
                     [[w * F, 2], [RP * F, C], [F, w], [1, F]])
            (eng or nc.gpsimd).dma_start(xt[:], src)
            return xt

        def ln_chunk(sb, ps, src_flat, n, nm):
            w1 = ps.tile([128, RW * FP], f32, name=f"{nm}w1", tag="Lw1")
            s1 = ps.tile([128, RW * FP], f32, name=f"{nm}s1", tag="Ls1")
            nc.tensor.matmul(w1[:, :n], cw["w1m_bf"][:], src_flat,
                             start=True, stop=True)
            sqw = sb.tile([128, RW * FP], bf, name=f"{nm}sqw", tag="Lsq")
            nc.scalar.activation(sqw[:, :n], w1[:, :n], AF.Square)
            nc.tensor.matmul(s1[:, :n], cw["bo_bf"][:], sqw[:, :n],
                             start=True, stop=True)
            inv = sb.tile([128, RW * FP], f32, name=f"{nm}inv", tag="Linv")
            nc.scalar.activation(inv[:, :n], s1[:, :n], AF.Abs_reciprocal_sqrt,
                                 scale=1.0 / 64.0)
            return w1, inv

        # persistent accumulators
        macc = per.tile([128, 1], f32, name="macc")
        nc.vector.memset(macc[:], 0.0)
        g2acc = [per.tile([128, F], f32, name=f"g2acc_{i}") for i in range(3)]
        for i in range(3):
            nc.vector.memset(g2acc[i][:], 0.0)
        ysacc = per.tile([128, 1], f32, name="ysacc")
        yqacc = per.tile([128, 1], f32, name="yqacc")
        nc.vector.memset(ysacc[:], 0.0)
        nc.vector.memset(yqacc[:], 0.0)

        # ============================ P1 ================================
        with tc.tile_pool(name="p1sb", bufs=3) as sb, \
             tc.tile_pool(name="p1ps", bufs=2, space="PSUM") as ps:
            for (t0, w) in _tiles(TH, 24):
                x2 = x2_load(sb, PADR + t0, w, "p1x")
                for (r0, rr) in _chunks(w):
                    n = rr * F
                    xc = x2[:, r0:r0 + rr, :].rearrange("p a b -> p (a b)")
                    w1, inv = ln_chunk(sb, ps, xc, n, "p1")
                    junk = sb.tile([128, RW * F], bf, name="p1junk",
                                   tag="p1junk")
                    acc = sb.tile([128, 1], f32, name="p1acc", tag="p1acc")
                    nc.vector.scalar_tensor_tensor(
                        junk[:, :n], w1[:, :n], 1.0, inv[:, :n],
                        OP.mult, OP.mult, accum_out=acc[:])
                    nc.vector.tensor_tensor(macc[:], macc[:], acc[:], OP.add)
        nc.sync.dma_start(AP(ag1i, 0, [[1, 128]]), macc[:])
        nc.gpsimd.collective_compute(
            "AllGather", OP.bypass, replica_groups=groups,
            ins=[ag1i.ap()], outs=[ag1o.ap()])
        m_a = per.tile([128, 2], f32, name="m_a")
        nc.sync.dma_start(m_a[:], AP(ag1o, 0, [[1, 128], [128, 2]]))
        mtot = per.tile([128, 1], f32, name="mtot")
        nc.vector.tensor_tensor(mtot[:], m_a[:, 0:1], m_a[:, 1:2], OP.add)

        # filt folds, horizontal stages
        ataps, kcv = [], []
        with tc.tile_pool(name="ffsb", bufs=2) as sb, \
             tc.tile_pool(name="ffps", bufs=2, space="PSUM") as ps:
            m64p = ps.tile([64, 1], f32, name="m64p")
            nc.tensor.matmul(m64p[:], cw["fold64"][:], mtot[:],
                             start=True, stop=True)
            m64 = per.tile([64, 1], f32, name="m64")
            nc.vector.tensor_copy(m64[:], m64p[:])
            for i in range(3):
                a_t, kc_t = _filt_fold(nc, sb, ps, per, cw, m64, i,
                                       f32, bf, AF, OP, AX)
                ataps.append(a_t)
                kcv.append(kc_t)
        if dsm_d is not None:
            nc.sync.dma_start(AP(dsm_d, 0, [[1, 128]]), mtot[:])
            for i in range(3):
                for k in range(KT):
                    nc.sync.dma_start(
                        AP(dsm_d, (1 + i * 3 + k) * 128, [[1, 128]]),
                        ataps[i][:, k:k + 1])
                nc.sync.dma_start(AP(dsm_d, (10 + i) * 128, [[1, 128]]),
                                  kcv[i][:, 0:1])
        dgh = []
        for i in range(3):
            for k in range(KT):
                # fused tap+LN-mean-removal stationary:
                # M_k^T = diag(a_k) (I - BO/64)  (w1m is symmetric)
                dt_ = per.tile([128, 128], bf, name=f"dgh_{i}_{k}")
                nc.vector.tensor_scalar_mul(dt_[:], cw["w1m_bf"][:],
                                            ataps[i][:, k:k + 1])
                dgh.append(dt_)

        if phases >= 2:
            # ============================ P2 ================================
            with tc.tile_pool(name="p2sb", bufs=3) as sb, \
                 tc.tile_pool(name="p2big", bufs=2) as big, \
                 tc.tile_pool(name="p2ps", bufs=1, space="PSUM") as ps, \
                 tc.tile_pool(name="p2pst", bufs=1, space="PSUM") as pst:
                for (t0, w) in _tiles(RP, 20):
                    x2 = x2_load(sb, t0, w, "p2x")
                    zzt = big.tile([128, 8 + w * FP + 8], bf, name="p2zzt",
                                   tag="p2zzt")
                    zz = zzt[:, 8:8 + w * FP].rearrange("p (a b) -> p a b", b=FP)
                    nc.vector.memset(zzt[:, 0:8], 0.0)
                    nc.vector.memset(zzt[:, 8 + w * FP:8 + w * FP + 8], 0.0)
                    for (r0, rr) in _chunks(w):
                        n = rr * F
                        xc = x2[:, r0:r0 + rr, :].rearrange("p a b -> p (a b)")
                        w1, inv = ln_chunk(sb, ps, xc, n, "p2a")
                        nc.vector.tensor_tensor(
                            zz[:, r0:r0 + rr, 7:7 + F],
                            w1[:, :n].rearrange("p (a b) -> p a b", b=F),
                            inv[:, :n].rearrange("p (a b) -> p a b", b=F),
                            OP.mult)
                    nc.vector.tensor_copy(zz[:, :, 0:7], zz[:, :, 14:7:-1])
                    nc.vector.tensor_copy(zz[:, :, 136:143], zz[:, :, 134:127:-1])
                    gpf = sb.tile([128, w], f32, name="p2gpf", tag="p2gpf")
                    nc.vector.tensor_reduce(gpf[:], zz[:, :, 7:7 + F], AX.X,
                                            OP.add)
                    for i in range(3):
                        d = DILS[i]
                        # grow' = grow - channel-block mean (fused tap
                        # matmuls M_k = (I-BO/64) diag(a_k) remove the mean
                        # of the tap sum; grow must match)
                        grow = sb.tile([128, w], f32, name="p2grow", tag="p2grow")
                        nc.vector.tensor_scalar(grow[:], gpf[:],
                                                cw[f"kg_{i}"][:, 0:1],
                                                kcv[i][:, 0:1], OP.mult, OP.add)
                        gmp = pst.tile([128, RW * F], f32, name="p2gmp",
                                       tag="p2gmp")
                        nc.tensor.matmul(gmp[:, :w], cw["bo_f"][:], grow[:],
                                         start=True, stop=True)
                        growp = sb.tile([128, w], f32, name="p2growp",
                                        tag="p2growp")
                        nc.vector.scalar_tensor_tensor(
                            growp[:], gmp[:, :w], -1.0 / 64.0, grow[:],
                            OP.mult, OP.add)
                        n2b = big.tile([128, w, F], bf, name="p2n2b",
                                       tag="p2n2b")
                        chs = _chunks(w)
                        for g0 in range(0, len(chs), 4):
                            grp = chs[g0:g0 + 4]
                            cps = [pst.tile([128, RW * F], f32,
                                            name=f"p2cp{cj}", tag=f"p2cp{cj}")
                                   for cj in range(len(grp))]
                            for k in range(KT):
                                a = 7 + (k - 1) * d
                                for cj, (r0, rr) in enumerate(grp):
                                    nc.tensor.matmul(
                                        cps[cj][:, :rr * F], dgh[i * KT + k][:],
                                        zz[:, r0:r0 + rr, a:a + F],
                                        start=(k == 0), stop=(k == 2))
                            for cj, (r0, rr) in enumerate(grp):
                                n = rr * F
                                w1f = sb.tile([128, RW * F], f32, name="p2w1f",
                                              tag="p2w1f")
                                nc.vector.tensor_tensor(
                                    w1f[:, :n].rearrange("p (a b) -> p a b",
                                                         b=F),
                                    cps[cj][:, :n].rearrange(
                                        "p (a b) -> p a b", b=F),
                                    growp[:, r0:r0 + rr].unsqueeze(2)
                                    .broadcast_to([128, rr, F]),
                                    OP.add)
                                sqw = sb.tile([128, RW * F], bf, name="p2sq",
                                              tag="p2sq")
                                nc.scalar.activation(sqw[:, :n], w1f[:, :n],
                                                     AF.Square)
                                s1 = ps.tile([128, RW * F], f32, name="p2s1b",
                                             tag="Ls1")
                                nc.tensor.matmul(s1[:, :n], cw["bo_bf"][:],
                                                 sqw[:, :n],
                                                 start=True, stop=True)
                                inv = sb.tile([128, RW * F], f32, name="p2inv",
                                              tag="Linv")
                                nc.scalar.activation(inv[:, :n], s1[:, :n],
                                                     AF.Abs_reciprocal_sqrt,
                                                     scale=1.0 / 64.0)
                                nc.vector.tensor_tensor(
                                    n2b[:, r0:r0 + rr, :].rearrange(
                                        "p a b -> p (a b)"),
                                    w1f[:, :n], inv[:, :n], OP.mult)
                        dst = AP(n2_d, i * C * RP * F + t0 * F,
                                 [[w * F, 2], [RP * F, C], [F, w], [1, F]])
                        nc.gpsimd.dma_start(dst, n2b[:])
                        # gap2 partials over strictly-local rows [PADR, PADR+TH)
                        rng = []
                        for q in range(2):
                            a = max(PADR - (t0 + q * w), 0)
                            bq = min(PADR + TH - (t0 + q * w), w)
                            rng.append((a, bq))
                        if rng[0] == (0, w) and rng[1] == (0, w):
                            red = sb.tile([128, F], f32, name="p2red",
                                          tag="p2red")
                            nc.vector.tensor_reduce(
                                red[:], n2b[:].transpose([0, 2, 1]),
                                AX.X, OP.add)
                            nc.vector.tensor_tensor(g2acc[i][:], g2acc[i][:],
                                                    red[:], OP.add)
                        else:
                            for q in range(2):
                                a, bq = rng[q]
                                if bq <= a:
                                    continue
                                p0, p1 = q * 64, q * 64 + 64
                                redh = sb.tile([128, F], f32, name="p2redh",
                                               tag="p2red")
                                nc.vector.tensor_reduce(
                                    redh[p0:p1],
                                    n2b[p0:p1, a:bq, :]
                                    .transpose([0, 2, 1]),
                                    AX.X, OP.add)
                                nc.vector.tensor_tensor(g2acc[i][p0:p1],
                                                        g2acc[i][p0:p1],
                                                        redh[p0:p1], OP.add)
            for i in range(3):
                nc.sync.dma_start(AP(ag2i, i * 128 * F, [[1, 128 * F]]),
                                  g2acc[i][:])
            nc.gpsimd.collective_compute(
                "AllGather", OP.bypass, replica_groups=groups,
                ins=[ag2i.ap()], outs=[ag2o.ap()])

            # filt folds vertical + gterm2
            gt2 = []
            with tc.tile_pool(name="f2sb", bufs=2) as sb, \
                 tc.tile_pool(name="f2ps", bufs=2, space="PSUM") as ps:
                for i in range(3):
                    ga = sb.tile([128, F], f32, name=f"f2ga_{i}", tag="f2ga")
                    gb = sb.tile([128, F], f32, name=f"f2gb_{i}", tag="f2gb")
                    nc.sync.dma_start(ga[:], AP(ag2o, i * 128 * F,
                                                [[F, 128], [1, F]]))
                    nc.sync.dma_start(gb[:], AP(ag2o, 3 * 128 * F + i * 128 * F,
                                                [[F, 128], [1, F]]))
                    gf = per.tile([128, F], f32, name=f"g2full_{i}")
                    nc.vector.tensor_tensor(gf[:], ga[:], gb[:], OP.add)
                    nsum = sb.tile([128, 1], f32, name=f"f2ns_{i}", tag="f2ns")
                    nc.vector.tensor_reduce(nsum[:], gf[:], AX.X, OP.add)
                    n64p = ps.tile([64, 1], f32, name=f"f2n64_{i}", tag="f2n64")
                    nc.tensor.matmul(n64p[:], cw["fold64"][:], nsum[:],
                                     start=True, stop=True)
                    n64 = sb.tile([64, 1], f32, name=f"f2n64s_{i}", tag="f2n64s")
                    nc.vector.tensor_copy(n64[:], n64p[:])
                    a_t, kc_t = _filt_fold(nc, sb, ps, per, cw, n64, i + 3,
                                           f32, bf, AF, OP, AX)
                    gt = per.tile([128, F], bf, name=f"gt2_{i}")
                    nc.vector.tensor_scalar(gt[:], gf[:],
                                            cw[f"kg_{i + 3}"][:, 0:1],
                                            kc_t[:, 0:1], OP.mult, OP.add)
                    gt2.append(gt)
                    ataps.append(a_t)
            # vertical taps folded into the mix conv stationaries:
            # (wg_i diag(av_ik))^T = diag(av_ik) wgbd_i
            dgv = []
            for i in range(3):
                for k in range(KT):
                    dt_ = per.tile([128, 128], bf, name=f"dgv_{i}_{k}")
                    nc.vector.tensor_scalar_mul(dt_[:], cw[f"wgbd_{i}"][:],
                                                ataps[3 + i][:, k:k + 1])
                    dgv.append(dt_)
            gt2mix = per.tile([128, F], f32, name="gt2mix")
            with tc.tile_pool(name="gtm", bufs=1, space="PSUM") as gps:
                gp = gps.tile([128, F], f32, name="gt2mp")
                for i in range(3):
                    nc.tensor.matmul(gp[:], cw[f"wgbd_{i}"][:], gt2[i][:],
                                     start=(i == 0), stop=(i == 2))
                nc.vector.tensor_copy(gt2mix[:], gp[:])

        if phases >= 3:
            # ============================ P3 ================================
            with tc.tile_pool(name="p3sb", bufs=3) as sb, \
                 tc.tile_pool(name="p3b2", bufs=2) as b2p, \
                 tc.tile_pool(name="p3ps", bufs=2, space="PSUM") as ps:
                for (t0, w) in _tiles(TH, 20):
                    x2 = x2_load(sb, PADR + t0, w, "p3x")
                    n2ws = []
                    for i in range(3):
                        n2w = b2p.tile([128, w + 14, F], bf, name=f"p3n2w_{i}",
                                       tag=f"p3n2w_{i}")
                        src = AP(n2_d, i * C * RP * F + t0 * F,
                                 [[w * F, 2], [RP * F, C], [F, w + 14], [1, F]])
                        nc.gpsimd.dma_start(n2w[:], src)
                        n2ws.append(n2w[:].rearrange("p a b -> p (a b)"))
                    yt = sb.tile([128, w, F], bf, name="p3y", tag="p3y")
                    chs = _chunks(w)
                    for g0 in range(0, len(chs), 4):
                        grp = chs[g0:g0 + 4]
                        yps = [ps.tile([128, RW * F], f32, name=f"p3yp{cj}",
                                       tag=f"p3yp{cj}")
                               for cj in range(len(grp))]
                        for cj, (r0, rr) in enumerate(grp):
                            nc.tensor.matmul(
                                yps[cj][:, :rr * F], cw["wbbd"][:],
                                x2[:, r0:r0 + rr, :]
                                .rearrange("p a b -> p (a b)"),
                                start=True, stop=False)
                        for i in range(3):
                            d = DILS[i]
                            for k in range(KT):
                                last = (i == 2 and k == 2)
                                for cj, (r0, rr) in enumerate(grp):
                                    off = (PADR + r0 + (k - 1) * d) * F
                                    nc.tensor.matmul(
                                        yps[cj][:, :rr * F],
                                        dgv[i * KT + k][:],
                                        n2ws[i][:, off:off + rr * F],
                                        start=False, stop=last)
                        for cj, (r0, rr) in enumerate(grp):
                            n = rr * F
                            acc = sb.tile([128, 1], f32, name="p3acc",
                                          tag="p3acc")
                            nc.vector.scalar_tensor_tensor(
                                yt[:, r0:r0 + rr, :], yps[cj][:, :n]
                                .rearrange("p (a b) -> p a b", b=F),
                                1.0,
                                gt2mix[:].unsqueeze(1)
                                .broadcast_to([128, rr, F]),
                                OP.mult, OP.add, accum_out=acc[:])
                            nc.vector.tensor_tensor(ysacc[:], ysacc[:],
                                                    acc[:], OP.add)
                            sqy = sb.tile([128, RW * F], f32, name="p3sqy",
                                          tag="p3sqy")
                            acq = sb.tile([128, 1], f32, name="p3acq",
                                          tag="p3acq")
                            nc.scalar.activation(
                                sqy[:, :n],
                                yt[:, r0:r0 + rr, :]
                                .rearrange("p a b -> p (a b)"),
                                AF.Square, accum_out=acq[:])
                            nc.vector.tensor_tensor(yqacc[:], yqacc[:],
                                                    acq[:], OP.add)
                    dst = AP(y_d, t0 * F,
                             [[w * F, 2], [TH * F, C], [F, w], [1, F]])
                    nc.gpsimd.dma_start(dst, yt[:])
            pk = per.tile([128, 2], f32, name="pk")
            nc.vector.tensor_copy(pk[:, 0:1], ysacc[:])
            nc.vector.tensor_copy(pk[:, 1:2], yqacc[:])
            nc.sync.dma_start(AP(ag3i, 0, [[1, 256]]), pk[:])
            nc.gpsimd.collective_compute(
                "AllGather", OP.bypass, replica_groups=groups,
                ins=[ag3i.ap()], outs=[ag3o.ap()])

            # GN scalars (y in y_d excludes convb_b; fold it analytically)
            gnS = per.tile([128, 1], f32, name="gnS")
            gnB = per.tile([128, 1], f32, name="gnB")
            with tc.tile_pool(name="gnsb", bufs=2) as sb, \
                 tc.tile_pool(name="gnps", bufs=2, space="PSUM") as ps:
                pa_ = sb.tile([128, 4], f32, name="gn_pa")
                nc.sync.dma_start(pa_[:, 0:2], AP(ag3o, 0, [[2, 128], [1, 2]]))
                nc.sync.dma_start(pa_[:, 2:4], AP(ag3o, 256, [[2, 128], [1, 2]]))
                sy = sb.tile([128, 1], f32, name="gn_sy")
                sq = sb.tile([128, 1], f32, name="gn_sq")
                nc.vector.tensor_tensor(sy[:], pa_[:, 0:1], pa_[:, 2:3], OP.add)
                nc.vector.tensor_tensor(sq[:], pa_[:, 1:2], pa_[:, 3:4], OP.add)
                NcF = float(TH * F)
                t1 = sb.tile([128, 1], f32, name="gn_t1")
                nc.vector.scalar_tensor_tensor(t1[:], sy[:], 2.0,
                                               cw["cbb"][:], OP.mult, OP.mult)
                nc.vector.tensor_tensor(sq[:], sq[:], t1[:], OP.add)
                cb2 = sb.tile([128, 1], f32, name="gn_cb2")
                nc.scalar.activation(cb2[:], cw["cbb"][:], AF.Square,
                                     scale=1.0)
                nc.vector.tensor_scalar(cb2[:], cb2[:], NcF, None, OP.mult)
                nc.vector.tensor_tensor(sq[:], sq[:], cb2[:], OP.add)
                nc.vector.scalar_tensor_tensor(t1[:], cw["cbb"][:], NcF, sy[:],
                                               OP.mult, OP.add)
                on1 = sb.tile([128, 1], f32, name="gn_on1")
                nc.vector.memset(on1[:], 1.0)
                tots = ps.tile([1, 2], f32, name="gn_tots")
                nc.tensor.matmul(tots[0:1, 0:1], on1[:], t1[:],
                                 start=True, stop=True)
                nc.tensor.matmul(tots[0:1, 1:2], on1[:], sq[:],
                                 start=True, stop=True)
                Ntot = float(C * T * F)
                mg = sb.tile([1, 1], f32, name="gn_mg")
                nc.vector.tensor_scalar(mg[:], tots[0:1, 0:1], 1.0 / Ntot, None,
                                        OP.mult)
                m2g = sb.tile([1, 1], f32, name="gn_m2g")
                nc.scalar.activation(m2g[:], mg[:], AF.Square)
                vg = sb.tile([1, 1], f32, name="gn_vg")
                nc.vector.scalar_tensor_tensor(vg[:], tots[0:1, 1:2], 1.0 / Ntot,
                                               m2g[:], OP.mult, OP.subtract)
                nc.vector.tensor_scalar(vg[:], vg[:], EPS, None, OP.add)
                ig = sb.tile([1, 1], f32, name="gn_ig")
                nc.scalar.activation(ig[:], vg[:], AF.Abs_reciprocal_sqrt)
                igb = sb.tile([128, 1], f32, name="gn_igb")
                mgb = sb.tile([128, 1], f32, name="gn_mgb")
                nc.gpsimd.partition_broadcast(igb[:], ig[:])
                nc.gpsimd.partition_broadcast(mgb[:], mg[:])
                nc.vector.tensor_tensor(gnS[:], igb[:], cw["gn_g"][:], OP.mult)
                nc.vector.tensor_tensor(gnB[:], cw["cbb"][:], mgb[:],
                                        OP.subtract)
                nc.vector.tensor_tensor(gnB[:], gnB[:], gnS[:], OP.mult)
                nc.vector.tensor_tensor(gnB[:], gnB[:], cw["gn_b"][:], OP.add)

        if phases >= 4:
            # ============================ P4 ================================
            with tc.tile_pool(name="p4sb", bufs=2) as sb, \
                 tc.tile_pool(name="p4ps", bufs=2, space="PSUM") as ps, \
                 tc.tile_pool(name="p4st", bufs=1, space="PSUM") as pst:
                for (t0, w) in _tiles(TH, 20):
                    y2 = sb.tile([128, w, F], bf, name="p4y", tag="p4y")
                    nc.gpsimd.dma_start(
                        y2[:], AP(y_d, t0 * F,
                                  [[w * F, 2], [TH * F, C], [F, w], [1, F]]))
                    ot = sb.tile([128, w, F], bf, name="p4o", tag="p4o")
                    nc.scalar.activation(ot[:], y2[:], AF.Prelu,
                                         bias=gnB[:, 0:1], scale=gnS[:, 0:1],
                                         alpha=cw["a0vec"][:, 0:1])
                    nc.gpsimd.dma_start(
                        AP(out_d, t0 * F,
                           [[w * F, 2], [TH * F, C], [F, w], [1, F]]), ot[:])
                    qk = sb.tile([64, w, F], bf, name="p4qk", tag="p4qk")
                    vt = sb.tile([128, w, F], bf, name="p4v", tag="p4v")
                    for (r0, rr) in _chunks(w):
                        n = rr * F
                        oc = ot[:, r0:r0 + rr, :].rearrange("p a b -> p (a b)")
                        qps = ps.tile([64, RW * F], f32, name="p4qps",
                                      tag="p4qps")
                        vps = ps.tile([128, RW * F], f32, name="p4vps",
                                      tag="p4vps")
                        nc.tensor.matmul(qps[:, :n], cw["wqkbd"][:], oc,
                                         start=True, stop=True)
                        nc.tensor.matmul(vps[:, :n], cw["wvbd"][:], oc,
                                         start=True, stop=True)
                        nc.scalar.activation(
                            qk[:, r0:r0 + rr, :].rearrange("p a b -> p (a b)"),
                            qps[:, :n], AF.Prelu, bias=cw["qkb"][:, 0:1],
                            alpha=cw["qka"][:, 0:1])
                        nc.scalar.activation(
                            vt[:, r0:r0 + rr, :].rearrange("p a b -> p (a b)"),
                            vps[:, :n], AF.Prelu, bias=cw["vb"][:, 0:1],
                            alpha=cw["va"][:, 0:1])
                    qs = sb.tile([64, w], f32, name="p4qs", tag="p4qs")
                    vs = sb.tile([128, w], f32, name="p4vs", tag="p4vs")
                    nc.vector.tensor_reduce(qs[:], qk[:], AX.X, OP.add)
                    nc.vector.tensor_reduce(vs[:], vt[:], AX.X, OP.add)
                    qq = sb.tile([64, w, F], f32, name="p4qq", tag="p4qq")
                    vv = sb.tile([128, w, F], f32, name="p4vv", tag="p4vv")
                    nc.scalar.activation(qq[:], qk[:], AF.Square)
                    nc.scalar.activation(vv[:], vt[:], AF.Square)
                    qsq = sb.tile([64, w], f32, name="p4qsq", tag="p4qsq")
                    vsq = sb.tile([128, w], f32, name="p4vsq", tag="p4vsq")
                    nc.vector.tensor_reduce(qsq[:], qq[:], AX.X, OP.add)
                    nc.vector.tensor_reduce(vsq[:], vv[:], AX.X, OP.add)
                    stq = pst.tile([16, 2 * w], f32, name="p4stq", tag="p4stq")
                    stv = pst.tile([8, 2 * w], f32, name="p4stv", tag="p4stv")
                    nc.tensor.matmul(stq[:, 0:w], cw["grp16"][:], qs[:],
                                     start=True, stop=True)
                    nc.tensor.matmul(stq[:, w:2 * w], cw["grp16"][:], qsq[:],
                                     start=True, stop=True)
                    nc.tensor.matmul(stv[:, 0:w], cw["grp8"][:], vs[:],
                                     start=True, stop=True)
                    nc.tensor.matmul(stv[:, w:2 * w], cw["grp8"][:], vsq[:],
                                     start=True, stop=True)
                    mivs = {}
                    for (st, npart, ncnt, nm) in ((stq, 16, 4 * F, "q"),
                                                  (stv, 8, 16 * F, "v")):
                        mu = sb.tile([npart, w], f32, name=f"p4mu{nm}",
                                     tag=f"p4mu{nm}")
                        nc.vector.tensor_scalar(mu[:], st[:, 0:w], 1.0 / ncnt,
                                                None, OP.mult)
                        m2_ = sb.tile([npart, w], f32, name=f"p4m2{nm}",
                                      tag=f"p4m2{nm}")
                        nc.scalar.activation(m2_[:], mu[:], AF.Square)
                        var = sb.tile([npart, w], f32, name=f"p4var{nm}",
                                      tag=f"p4var{nm}")
                        nc.vector.scalar_tensor_tensor(var[:], st[:, w:2 * w],
                                                       1.0 / ncnt, m2_[:],
                                                       OP.mult, OP.subtract)
                        nc.vector.tensor_scalar(var[:], var[:], EPS, None,
                                                OP.add)
                        iv = sb.tile([npart, w], f32, name=f"p4iv{nm}",
                                     tag=f"p4iv{nm}")
                        nc.scalar.activation(iv[:], var[:], AF.Abs_reciprocal_sqrt)
                        mivs[nm] = (mu, iv)
                    qb_ = pst.tile([64, 2 * w], f32, name="p4qb", tag="p4qb")
                    vb_ = pst.tile([128, 2 * w], f32, name="p4vb", tag="p4vb")
                    nc.tensor.matmul(qb_[:, 0:w], cw["bc16"][:], mivs["q"][0][:],
                                     start=True, stop=True)
                    nc.tensor.matmul(qb_[:, w:2 * w], cw["bc16"][:],
                                     mivs["q"][1][:], start=True, stop=True)
                    nc.tensor.matmul(vb_[:, 0:w], cw["bc8"][:], mivs["v"][0][:],
                                     start=True, stop=True)
                    nc.tensor.matmul(vb_[:, w:2 * w], cw["bc8"][:],
                                     mivs["v"][1][:], start=True, stop=True)
                    qkn = sb.tile([64, w, F], bf, name="p4qkn", tag="p4qkn")
                    vn = sb.tile([128, w, F], bf, name="p4vn", tag="p4vn")
                    nc.vector.tensor_tensor(
                        qkn[:], qk[:],
                        qb_[:, 0:w].unsqueeze(2).broadcast_to([64, w, F]),
                        OP.subtract)
                    nc.vector.tensor_tensor(
                        qkn[:], qkn[:],
                        qb_[:, w:2 * w].unsqueeze(2).broadcast_to([64, w, F]),
                        OP.mult)
                    nc.vector.tensor_tensor(
                        vn[:], vt[:],
                        vb_[:, 0:w].unsqueeze(2).broadcast_to([128, w, F]),
                        OP.subtract)
                    nc.vector.tensor_tensor(
                        vn[:], vn[:],
                        vb_[:, w:2 * w].unsqueeze(2).broadcast_to([128, w, F]),
                        OP.mult)
                    nc.sync.dma_start(
                        AP(qkvh_d, t0 * F,
                           [[w * F, 2], [TH * F, 32], [F, w], [1, F]]), qkn[:])
                    nc.sync.dma_start(
                        AP(qkvh_d, 32 * TH * F + t0 * F,
                           [[w * F, 2], [TH * F, 64], [F, w], [1, F]]), vn[:])
            nc.gpsimd.collective_compute(
                "AllGather", OP.bypass, replica_groups=groups,
                ins=[AP(qkvh_d, 16 * TH * F, [[1, 80 * TH * F]])],
                outs=[qkvf_d.ap()])

        if phases >= 5:
            # ============================ P5 ================================
            KCH = [(0, 128), (128, 128), (256, 128), (384, 128), (512, 4)]
            NCH = [(0, 512), (512, 512), (1024, 512), (1536, 512), (2048, 16)]
            import contextlib as _cl
            for h in range(H):
                hx = _cl.ExitStack()
                sb = hx.enter_context(tc.tile_pool(name=f"a{h}sb", bufs=3))
                res = hx.enter_context(tc.tile_pool(name=f"a{h}res", bufs=1))
                ps = hx.enter_context(tc.tile_pool(name=f"a{h}ps", bufs=2,
                                                   space="PSUM"))
                pss = hx.enter_context(tc.tile_pool(name=f"a{h}pss", bufs=1,
                                                    space="PSUM"))
                pso = hx.enter_context(tc.tile_pool(name=f"a{h}pso", bufs=2,
                                                    space="PSUM"))
                ktr = [res.tile([kn, T], bf, name=f"ktr{h}_{ci}")
                       for ci, (k0, kn) in enumerate(KCH)]
                qtr = [res.tile([kn, TH], bf, name=f"qtr{h}_{ci}")
                       for ci, (k0, kn) in enumerate(KCH)]
                vres = [res.tile([MB, DF], bf, name=f"vres{h}_{b}")
                        for b in range(8)]
                for b in range(8):
                    seg, tl = divmod(b, 4)
                    km = sb.tile([MB, EF], bf, name=f"km{h}", tag="km")
                    nc.sync.dma_start(
                        km[:], AP(qkvf_d,
                                  seg * 80 * TH * F + (h * 4) * TH * F
                                  + tl * MB * F,
                                  [[F, MB], [TH * F, E], [1, F]]))
                    for ci, (k0, kn) in enumerate(KCH):
                        tp = ps.tile([128, MB], bf, name=f"tp{h}", tag="tp")
                        nc.tensor.transpose(tp[:kn, :], km[:, k0:k0 + kn],
                                            cw["ident_bf"][0:MB, 0:MB])
                        nc.vector.tensor_copy(ktr[ci][:, b * MB:(b + 1) * MB],
                                              tp[:kn, :])
                    nc.sync.dma_start(
                        vres[b][:], AP(qkvf_d,
                                       seg * 80 * TH * F + (16 + h * 16) * TH * F
                                       + tl * MB * F,
                                       [[F, MB], [TH * F, D], [1, F]]))
                for b in range(4):
                    km = sb.tile([MB, EF], bf, name=f"qm{h}", tag="km")
                    nc.sync.dma_start(
                        km[:], AP(qkvh_d, (h * 4) * TH * F + b * MB * F,
                                  [[F, MB], [TH * F, E], [1, F]]))
                    for ci, (k0, kn) in enumerate(KCH):
                        tp = ps.tile([128, MB], bf, name=f"tp{h}", tag="tp")
                        nc.tensor.transpose(tp[:kn, :], km[:, k0:k0 + kn],
                                            cw["ident_bf"][0:MB, 0:MB])
                        nc.vector.tensor_copy(qtr[ci][:, b * MB:(b + 1) * MB],
                                              tp[:kn, :])
                # lng/lnb per-partition post-transpose
                for ci, (k0, kn) in enumerate(KCH):
                    for (lg, lb, tt) in ((("klngT"), ("klnbT"), ktr),
                                         (("qlngT"), ("qlnbT"), qtr)):
                        vg_ = sb.tile([128, 1], f32, name=f"lg{h}", tag="lg")
                        vb2 = sb.tile([128, 1], f32, name=f"lb{h}", tag="lb")
                        nc.sync.dma_start(vg_[:kn, :],
                                          AP(fw[lg], h * 640 + k0, [[1, kn]]))
                        nc.sync.dma_start(vb2[:kn, :],
                                          AP(fw[lb], h * 640 + k0, [[1, kn]]))
                        nc.vector.tensor_scalar(tt[ci][:], tt[ci][:],
                                                vg_[:kn, 0:1], vb2[:kn, 0:1],
                                                OP.mult, OP.add)
                lngb = res.tile([MB, DF], f32, name=f"lngb{h}")
                lrow = sb.tile([1, DF], f32, name=f"lrow{h}", tag="lrow")
                nc.sync.dma_start(lrow[:], AP(fw["vlngT"], h * DF, [[DF, 1],
                                                                   [1, DF]]))
                nc.gpsimd.partition_broadcast(lngb[:], lrow[:])
                for mt in range(4):
                    spt = pss.tile([MB, 1024], f32, name=f"spt{h}", tag="spt")
                    for ci, (k0, kn) in enumerate(KCH):
                        lhs = qtr[ci][:, mt * MB:(mt + 1) * MB]
                        nc.tensor.matmul(spt[:, 0:500], lhs, ktr[ci][:, 0:500],
                                         start=(ci == 0), stop=(ci == 4))
                        nc.tensor.matmul(spt[:, 512:1012], lhs,
                                         ktr[ci][:, 500:1000],
                                         start=(ci == 0), stop=(ci == 4))
                    sview = spt[:].rearrange("p (a b) -> p a b", b=512)[:, :, 0:500]
                    mx = sb.tile([MB, 1], f32, name=f"mx{h}", tag="mx")
                    nc.vector.tensor_reduce(mx[:], sview, AX.XY, OP.max)
                    bias = sb.tile([MB, 1], f32, name=f"bias{h}", tag="bias")
                    nc.vector.tensor_scalar(bias[:], mx[:], -ISCALE, None,
                                            OP.mult)
                    pexp = sb.tile([MB, 1000], bf, name=f"pexp{h}", tag="pexp")
                    lsum = sb.tile([MB, 1], f32, name=f"lsum{h}", tag="lsum")
                    nc.scalar.activation(pexp[:], sview, AF.Exp,
                                         bias=bias[:, 0:1], scale=ISCALE,
                                         accum_out=lsum[:])
                    linv = sb.tile([MB, 1], f32, name=f"linv{h}", tag="linv")
                    nc.vector.reciprocal(linv[:], lsum[:])
                    ptr = []
                    for b in range(8):
                        tp2 = ps.tile([MB, MB], bf, name=f"tp2{h}", tag="tp2")
                        nc.tensor.transpose(tp2[:], pexp[:, b * MB:(b + 1) * MB],
                                            cw["ident_bf"][0:MB, 0:MB])
                        pb_ = sb.tile([MB, MB], bf, name=f"ptr{h}_{b}",
                                      tag=f"ptr{b}")
                        nc.vector.tensor_copy(pb_[:], tp2[:])
                        ptr.append(pb_)
                    ob = sb.tile([MB, DF], bf, name=f"ob{h}", tag="ob")
                    for (n0, nn) in NCH:
                        op_ = pso.tile([MB, 512], f32, name=f"op{h}", tag="op")
                        for b in range(8):
                            nc.tensor.matmul(op_[:, :nn], ptr[b][:],
                                             vres[b][:, n0:n0 + nn],
                                             start=(b == 0), stop=(b == 7))
                        nc.vector.scalar_tensor_tensor(
                            ob[:, n0:n0 + nn], op_[:, :nn], linv[:, 0:1],
                            lngb[:, n0:n0 + nn], OP.mult, OP.mult)
                    nc.gpsimd.dma_start(
                        AP(o_d, mt * MB * H * D * F + h * D * F,
                           [[H * D * F, MB], [1, D * F]]), ob[:])
                hx.close()

        if phases >= 6:
            # ============================ P6 ================================
            with tc.tile_pool(name="p6sb", bufs=2) as sb, \
                 tc.tile_pool(name="p6ps", bufs=2, space="PSUM") as ps, \
                 tc.tile_pool(name="p6st", bufs=1, space="PSUM") as pst:
                for (t0, w) in _tiles(TH, 20):
                    o2 = sb.tile([128, w, F], bf, name="p6o", tag="p6o")
                    for q in range(2):
                        nc.gpsimd.dma_start(
                            o2[q * 64:(q + 1) * 64],
                            AP(o_d, (t0 + q * w) * H * D * F,
                               [[F, 64], [H * D * F, w], [1, F]]))
                    u2 = sb.tile([128, w, F], f32, name="p6u2", tag="p6u2")
                    for (r0, rr) in _chunks(w):
                        n = rr * F
                        pps = ps.tile([128, RW * F], f32, name="p6pps",
                                      tag="p6pps")
                        nc.tensor.matmul(
                            pps[:, :n], cw["pwbd"][:],
                            o2[:, r0:r0 + rr, :].rearrange("p a b -> p (a b)"),
                            start=True, stop=True)
                        tmp = sb.tile([128, RW, F], f32, name="p6tmp",
                                      tag="p6tmp")
                        nc.vector.tensor_tensor(
                            tmp[:, :rr, :],
                            pps[:, :n].rearrange("p (a b) -> p a b", b=F),
                            cw["pconst"][:].unsqueeze(1)
                            .broadcast_to([128, rr, F]),
                            OP.add)
                        nc.scalar.activation(
                            u2[:, r0:r0 + rr, :].rearrange("p a b -> p (a b)"),
                            tmp[:, :rr, :].rearrange("p a b -> p (a b)"),
                            AF.Prelu, bias=cw["pb"][:, 0:1],
                            alpha=cw["pa"][:, 0:1])
                    us = sb.tile([128, w], f32, name="p6us", tag="p6us")
                    nc.vector.tensor_reduce(us[:], u2[:], AX.X, OP.add)
                    uq = sb.tile([128, w, F], f32, name="p6uq", tag="p6uq")
                    nc.scalar.activation(uq[:], u2[:], AF.Square)
                    usq = sb.tile([128, w], f32, name="p6usq", tag="p6usq")
                    nc.vector.tensor_reduce(usq[:], uq[:], AX.X, OP.add)
                    st2 = pst.tile([2, 2 * w], f32, name="p6st2", tag="p6st2")
                    nc.tensor.matmul(st2[:, 0:w], cw["redq"][:], us[:],
                                     start=True, stop=True)
                    nc.tensor.matmul(st2[:, w:2 * w], cw["redq"][:], usq[:],
                                     start=True, stop=True)
                    ncnt = float(64 * F)
                    mu2 = sb.tile([2, w], f32, name="p6mu2", tag="p6mu2")
                    nc.vector.tensor_scalar(mu2[:], st2[:, 0:w], 1.0 / ncnt,
                                            None, OP.mult)
                    m22 = sb.tile([2, w], f32, name="p6m22", tag="p6m22")
                    nc.scalar.activation(m22[:], mu2[:], AF.Square)
                    var2 = sb.tile([2, w], f32, name="p6var2", tag="p6var2")
                    nc.vector.scalar_tensor_tensor(var2[:], st2[:, w:2 * w],
                                                   1.0 / ncnt, m22[:],
                                                   OP.mult, OP.subtract)
                    nc.vector.tensor_scalar(var2[:], var2[:], EPS, None, OP.add)
                    iv2 = sb.tile([2, w], f32, name="p6iv2", tag="p6iv2")
                    nc.scalar.activation(iv2[:], var2[:], AF.Abs_reciprocal_sqrt)
                    mb2 = pst.tile([128, 2 * w], f32, name="p6mb2", tag="p6mb2")
                    nc.tensor.matmul(mb2[:, 0:w], cw["ind2f"][:], mu2[:],
                                     start=True, stop=True)
                    nc.tensor.matmul(mb2[:, w:2 * w], cw["ind2f"][:], iv2[:],
                                     start=True, stop=True)
                    nc.vector.tensor_tensor(
                        u2[:], u2[:],
                        mb2[:, 0:w].unsqueeze(2).broadcast_to([128, w, F]),
                        OP.subtract)
                    nc.vector.tensor_tensor(
                        u2[:], u2[:],
                        mb2[:, w:2 * w].unsqueeze(2).broadcast_to([128, w, F]),
                        OP.mult)
                    nc.vector.tensor_tensor(
                        u2[:], u2[:],
                        cw["plng"][:].unsqueeze(1).broadcast_to([128, w, F]),
                        OP.mult)
                    rt = sb.tile([128, w, F], bf, name="p6rt", tag="p6rt")
                    nc.gpsimd.dma_start(
                        rt[:], AP(out_d, t0 * F,
                                  [[w * F, 2], [TH * F, C], [F, w], [1, F]]))
                    r1 = sb.tile([128, w, F], f32, name="p6r1", tag="p6r1")
                    nc.vector.tensor_tensor(
                        r1[:], rt[:],
                        cw["plnb"][:].unsqueeze(1).broadcast_to([128, w, F]),
                        OP.add)
                    fint = sb.tile([128, w, F], bf, name="p6fin", tag="p6fin")
                    nc.vector.tensor_tensor(fint[:], u2[:], r1[:], OP.add)
                    nc.scalar.dma_start(
                        AP(fin, t0 * F,
                           [[w * F, 2], [TH * F, C], [F, w], [1, F]]), fint[:])
        if phases < 6:
            with tc.tile_pool(name="dummy", bufs=1) as dp:
                zt = dp.tile([C, 16], bf, name="zfin")
                nc.vector.memset(zt[:], 0.0)
                nc.sync.dma_start(AP(fin, 0, [[TH * F, C], [1, 16]]), zt[:])
        ctx.close()
    nc.compile()
    return nc


def _filt_fold(nc, sb, ps, per, cw, mean64, i, f32, bf, AF, OP, AX):
    u1 = sb.tile([64, 1], f32, name=f"u1_{i}", tag="ffu1")
    nc.vector.tensor_scalar(u1[:], mean64[:], cw[f"gtf_{i}"][:, 0:1],
                            cw[f"c64_{i}"][:, 0:1], OP.mult, OP.add)
    ftp = ps.tile([1, 12], f32, name=f"ftp_{i}", tag="ffftp")
    nc.tensor.matmul(ftp[:], u1[:], cw[f"lwT_{i}"][:], start=True, stop=True)
    ft = sb.tile([1, 12], f32, name=f"ft_{i}", tag="ffft")
    nc.scalar.activation(ft[:], ftp[:], AF.Tanh)
    ft4 = sb.tile([4, 3], f32, name=f"ft4_{i}", tag="ffft4")
    nc.sync.dma_start(ft4[:], ft[:].rearrange("o (g k) -> (o g) k", g=4))
    wcp = ps.tile([128, 3], f32, name=f"wcp_{i}", tag="ffwcp")
    nc.tensor.matmul(wcp[:], cw["grp4"][:], ft4[:], start=True, stop=True)
    atap = per.tile([128, 3], f32, name=f"atap_{i}")
    nc.vector.tensor_scalar_mul(atap[:], wcp[:], cw[f"gs_{i}"][:, 0:1])
    nc.vector.tensor_tensor(atap[:, 1:2], atap[:, 1:2],
                            cw[f"hc_{i}"][:, 0:1], OP.add)
    wcs = sb.tile([128, 1], f32, name=f"wcs_{i}", tag="ffwcs")
    nc.vector.tensor_reduce(wcs[:], wcp[:], AX.X, OP.add)
    kc = per.tile([128, 1], f32, name=f"kc_{i}")
    nc.vector.tensor_scalar(kc[:], wcs[:], cw[f"cs_{i}"][:, 0:1],
                            cw[f"cb_{i}"][:, 0:1], OP.mult, OP.add)
    nc.vector.tensor_tensor(kc[:], kc[:], cw[f"gc_{i}"][:, 0:1], OP.add)
    return atap, kc


# ---------------------------------------------------------------------------
# host entry
# ---------------------------------------------------------------------------

def _prep_inputs(inputs, fold):
    import ml_dtypes
    x = np.asarray(inputs["x"], np.float32)
    in_maps = []
    for c in range(8):
        s, hf = divmod(c, 2)
        xs = np.pad(x[s], ((0, 0), (PADR, PADR), (0, 0)), mode="reflect")
        xc = xs[:, hf * TH:hf * TH + RP, :]
        m = {"x_d": np.ascontiguousarray(xc).astype(ml_dtypes.bfloat16)}
        for n, (sh, isbf) in _fold_shapes().items():
            v = fold[n]
            m[n] = v.astype(ml_dtypes.bfloat16) if isbf else v
        for n in _dram_only_shapes():
            m[n] = fold[n]
        in_maps.append(m)
    return in_maps


def kernel(**inputs):
    import os
    from concourse.bass_utils import run_bass_kernel_spmd
    global LAST_EXEC_NS
    if "nc" not in _CACHE:
        _CACHE["nc"] = _build(dbg=_CACHE.get("dbg", ()),
                              phases=int(os.environ.get("KPHASES", "6")))
    nc = _CACHE["nc"]
    fold = _fold_weights(inputs)
    in_maps = _prep_inputs(inputs, fold)
    kw = {}
    if os.environ.get("KTRACE"):
        import tempfile
        base = os.environ.get("KTRACE_DIR",
                              os.path.join(os.getcwd(), "work"))
        os.makedirs(base, exist_ok=True)
        tdir = tempfile.mkdtemp(prefix="trace_", dir=base)
        with open(os.path.join(base, "last_trace_path.txt"), "w") as f:
            f.write(tdir)
        tc_ = os.environ.get("KTRACE_CORES", "0")
        kw = dict(trace=True, tmpdir=tdir,
                  trace_cores=[int(c) for c in tc_.split(",")])
    res = run_bass_kernel_spmd(nc, in_maps, core_ids=list(range(8)), **kw)
    _CACHE["last"] = res
    if getattr(res, "exec_time_ns", None):
        LAST_EXEC_NS = res.exec_time_ns
    out = np.zeros((B, C, T, F), np.float32)
    for c in range(8):
        s, hf = divmod(c, 2)
        out[s][:, hf * TH:(hf + 1) * TH, :] = \
            res.results[c]["fin"].astype(np.float32)
    return out



# revision 57
# speedup vs baseline: 1.8066x; 1.0597x over previous
"""GridNetBlock_Att Trainium2 kernel (Bass/Tile, 8 NeuronCores).

Core c handles sample s=c//2, T-half h=c%2 (rows [500h, 500h+500)).
Pre-attention is T-split per core (host supplies x with a reflect-padded
t-halo of 7 rows, which makes the SPMD program identical on all cores);
tiny pair AllGathers combine global stats, and one pair AllGather
exchanges the K/V halves before full-sequence attention.

Pre-attention tiles are "2-stack": 128 partitions = 2 consecutive
row-blocks x 64 channels.  LN-over-channels per psum chunk:
    w1  = (I - BO/64) @ x        (PE; BO = block-ones)    = x - mu
    sqw = Square(w1)             (ACT, psum->sbuf)
    s1  = BO @ sqw               (PE)                     = 64*var
    inv = AbsRecipSqrt(s1/64)    (ACT)                    = 1/sqrt(var)
    z   = w1 * inv               (DVE, psum x sbuf -> bf16)
"""
import time

import numpy as np

EPS = 1e-5
B, C, T, F = 4, 64, 1000, 129
H, E, D = 4, 4, 16
GROUP, KT = 4, 3
DILS = (3, 5, 7)
EF, DF = E * F, D * F      # 516, 2064
TH = T // 2                 # 500 local rows
PADR = 7
RP = TH + 2 * PADR          # 514
FP = F + 14                 # 143
TF = float(T * F)
RW = 3                      # rows per psum chunk
ISCALE = float(1.0 / np.sqrt(EF))
MB = 125                    # attention row block

_CACHE = {}
LAST_EXEC_NS = -1


def _tiles(total, w):
    out, t0 = [], 0
    while t0 < total:
        ww = min(2 * w, total - t0) // 2
        out.append((t0, ww))
        t0 += 2 * ww
    return out


def _chunks(w, rw=RW):
    return [(ci * rw, min(rw, w - ci * rw)) for ci in range((w + rw - 1) // rw)]


# ---------------------------------------------------------------------------
# host-side weight folding
# ---------------------------------------------------------------------------

def _fold_shapes():
    sh = {
        "bo_f": ((128, 128), False),
        "bo_bf": ((128, 128), True),
        "w1m_bf": ((128, 128), True),
        "ind2f": ((2, 128), False), "ident_bf": ((128, 128), True),
        "fold64": ((128, 64), False), "redq": ((128, 2), False),
        "grp4": ((4, 128), False),
        "wbbd": ((128, 128), True), "cbb": ((128, 1), False),
        "gn_g": ((128, 1), False), "gn_b": ((128, 1), False),
        "a0vec": ((128, 1), False),
        "wqkbd": ((128, 64), True), "wvbd": ((128, 128), True),
        "qkb": ((64, 1), False), "qka": ((64, 1), False),
        "vb": ((128, 1), False), "va": ((128, 1), False),
        "grp16": ((64, 16), False), "bc16": ((16, 64), False),
        "grp8": ((128, 8), False), "bc8": ((8, 128), False),
        "pwbd": ((128, 128), True), "pconst": ((128, F), False),
        "pb": ((128, 1), False), "pa": ((128, 1), False),
        "plng": ((128, F), False), "plnb": ((128, F), False),
    }
    for i in range(6):
        for nm in ("gs", "hc", "kg", "cs", "cb", "gc"):
            sh[f"{nm}_{i}"] = ((128, 1), False)
        sh[f"lwT_{i}"] = ((64, 12), False)
        sh[f"gtf_{i}"] = ((64, 1), False)
        sh[f"c64_{i}"] = ((64, 1), False)
    for i in range(3):
        sh[f"wgbd_{i}"] = ((128, 128), True)
    return sh


# loaded from DRAM on demand, not staged in SBUF constants
def _dram_only_shapes():
    return {
        "qlngT": (H, 640), "klngT": (H, 640),
        "qlnbT": (H, 640), "klnbT": (H, 640),
        "vlngT": (H, DF),
    }


def _fold_weights(w):
    f32 = np.float32
    g = {}
    ar = lambda a: np.ascontiguousarray(a, f32)
    dup = lambda v: np.tile(ar(v).reshape(64), 2).reshape(128, 1)

    bo = np.zeros((128, 128), f32)
    bo[:64, :64] = 1.0
    bo[64:, 64:] = 1.0
    g["bo_f"] = bo
    g["bo_bf"] = bo
    g["w1m_bf"] = np.eye(128, dtype=f32) - bo / 64.0
    ind2 = np.zeros((2, 128), f32)
    ind2[0, :64] = 1.0
    ind2[1, 64:] = 1.0
    g["ind2f"] = ind2
    g["ident_bf"] = np.eye(128, dtype=f32)
    fold2 = np.zeros((128, 64), f32)
    for p in range(128):
        fold2[p, p % 64] = 1.0
    g["fold64"] = fold2
    redq = np.zeros((128, 2), f32)
    redq[:64, 0] = 1.0
    redq[64:, 1] = 1.0
    g["redq"] = redq
    gi4 = np.zeros((4, 128), f32)
    for p in range(128):
        gi4[(p % 64) // 16, p] = 1.0
    g["grp4"] = gi4

    for i in range(6):
        gg = w["br_g"][i].astype(f32)
        cc = w["br_b"][i].astype(f32)
        ia = w["lisa_in"][i].astype(f32)
        ll = w["lisa_ll"][i].astype(f32)
        lh = w["lisa_lh"][i].astype(f32)
        s = (ia + 1.0) * ll
        gap_div = float(F) if i < 3 else float(T)
        g[f"gs_{i}"] = dup(gg * s)
        g[f"hc_{i}"] = dup(gg * (lh + 1.0))
        g[f"kg_{i}"] = dup((-ia * ll * gg) / gap_div)
        g[f"gc_{i}"] = dup(-ia * ll * cc)
        g[f"cs_{i}"] = dup(cc * s)
        g[f"cb_{i}"] = dup(cc * (lh + 1.0))
        g[f"lwT_{i}"] = ar(w["lisa_w"][i].T)
        g[f"gtf_{i}"] = ar((gg / TF).reshape(64, 1))
        g[f"c64_{i}"] = ar(cc.reshape(64, 1))

    cw_ = w["convb_w"].astype(f32)
    gam = w["mix_gamma"].astype(f32)
    bet = w["mix_beta"].astype(f32)

    def bd(m, no):
        z = np.zeros((128, 2 * no), f32)
        z[:64, :no] = m.T
        z[64:, no:] = m.T
        return z

    for i in range(3):
        g[f"wgbd_{i}"] = bd(cw_ * gam[i][None, :], 64)
    g["wbbd"] = bd(cw_ * bet.sum(0)[None, :], 64)
    g["cbb"] = dup(w["convb_b"])
    g["gn_g"] = dup(w["gn_g"])
    g["gn_b"] = dup(w["gn_b"])
    g["a0vec"] = np.full((128, 1), float(w["convb_a"]), f32)

    wqk = np.concatenate([w["q_w"].astype(f32).reshape(H * E, C),
                          w["k_w"].astype(f32).reshape(H * E, C)], 0)
    g["wqkbd"] = bd(wqk, 32)
    g["wvbd"] = bd(w["v_w"].astype(f32).reshape(H * D, C), 64)
    qkb = np.concatenate([w["q_b"].reshape(-1), w["k_b"].reshape(-1)])
    g["qkb"] = np.tile(ar(qkb), 2).reshape(64, 1)
    qka = np.concatenate([np.repeat(w["q_a"], E), np.repeat(w["k_a"], E)])
    g["qka"] = np.tile(ar(qka), 2).reshape(64, 1)
    g["vb"] = np.tile(ar(w["v_b"].reshape(-1)), 2).reshape(128, 1)
    g["va"] = np.tile(ar(np.repeat(w["v_a"], D)), 2).reshape(128, 1)
    g16 = np.zeros((64, 16), f32)
    for p in range(64):
        q, j = divmod(p, 32)
        g16[p, q * 8 + (j // 16) * 4 + (j % 16) // 4] = 1.0
    g["grp16"] = g16
    g["bc16"] = ar(g16.T)
    g8 = np.zeros((128, 8), f32)
    for p in range(128):
        q, j = divmod(p, 64)
        g8[p, q * 4 + j // 16] = 1.0
    g["grp8"] = g8
    g["bc8"] = ar(g8.T)
    for nm, src in (("qlngT", "q_lng"), ("klngT", "k_lng"),
                    ("qlnbT", "q_lnb"), ("klnbT", "k_lnb")):
        m = np.zeros((H, 640), f32)
        for h in range(H):
            m[h, :EF] = w[src][h].reshape(EF)
        g[nm] = m
    g["vlngT"] = ar(w["v_lng"].reshape(H, DF))
    pw = w["proj_w"].astype(f32)
    g["pwbd"] = bd(pw, 64)
    pconst = pw @ w["v_lnb"].reshape(H * D, F).astype(f32)
    g["pconst"] = np.tile(pconst, (2, 1)).reshape(128, F)
    g["pb"] = dup(w["proj_b"])
    g["pa"] = np.full((128, 1), float(w["proj_a"]), f32)
    g["plng"] = np.tile(w["proj_lng"].astype(f32), (2, 1)).reshape(128, F)
    g["plnb"] = np.tile(w["proj_lnb"].astype(f32), (2, 1)).reshape(128, F)
    return g


# ---------------------------------------------------------------------------
# device program
# ---------------------------------------------------------------------------

def _build(dbg=(), phases=6):
    import concourse.bass as bass
    import concourse.bacc as bacc
    import concourse.mybir as mybir
    from concourse import tile
    from contextlib import ExitStack

    f32 = mybir.dt.float32
    bf = mybir.dt.bfloat16
    AF = mybir.ActivationFunctionType
    OP = mybir.AluOpType
    AX = mybir.AxisListType

    nc = bacc.Bacc("TRN2", target_bir_lowering=False, debug=False,
                   num_devices=8)

    def AP(tensor, offset, dims):
        return bass.AP(tensor=tensor, offset=offset,
                       ap=[list(d) for d in dims])

    shapes = _fold_shapes()
    dshapes = _dram_only_shapes()
    x_d = nc.dram_tensor("x_d", [C, RP, F], bf, kind="ExternalInput")
    fw = {n: nc.dram_tensor(n, list(s), bf if b else f32,
                            kind="ExternalInput")
          for n, (s, b) in shapes.items()}
    for n, s in dshapes.items():
        fw[n] = nc.dram_tensor(n, list(s), f32, kind="ExternalInput")

    def idram(name, shape, dt_):
        kind = "ExternalOutput" if name in dbg else "Internal"
        return nc.dram_tensor(name, list(shape), dt_, kind=kind)

    n2_d = idram("n2_d", [3, C, RP, F], bf)
    y_d = idram("y_d", [C, TH, F], bf)
    out_d = idram("out_d", [C, TH, F], bf)
    qh_d = idram("qh_d", [16, TH, F], bf)
    kvh_d = [idram(f"kvh{h}_d", [20, TH, F], bf) for h in range(H)]
    kvf_d = [idram(f"kvf{h}_d", [2, 20, TH, F], bf) for h in range(H)]
    o_d = idram("o_d", [TH, H, D, F], bf)
    b1_d = idram("b1_d", [3, C, RP, F], bf) if "b1_d" in dbg else None
    dsm_d = idram("dsm_d", [16, 128], f32) if "dsm_d" in dbg else None
    ag1i = nc.dram_tensor("ag1i", [1, 128], f32)
    ag1o = nc.dram_tensor("ag1o", [2, 128], f32)
    ag2i = nc.dram_tensor("ag2i", [1, 3 * 128 * F], f32)
    ag2o = nc.dram_tensor("ag2o", [2, 3 * 128 * F], f32)
    ag3i = nc.dram_tensor("ag3i", [1, 256], f32)
    ag3o = nc.dram_tensor("ag3o", [2, 256], f32)
    fin = nc.dram_tensor("fin", [C, TH, F], bf, kind="ExternalOutput")

    groups = [[0, 1], [2, 3], [4, 5], [6, 7]]

    with nc.allow_low_precision(reason="bf16 pipeline, tol 2e-2"), \
         tile.TileContext(nc) as tc:
        ctx = ExitStack()
        cst = ctx.enter_context(tc.tile_pool(name="cst", bufs=1))
        per = ctx.enter_context(tc.tile_pool(name="per", bufs=1))

        def load_const(name):
            sh, isbf = shapes[name]
            t = cst.tile(list(sh), bf if isbf else f32, name=f"c_{name}",
                         tag=f"c_{name}")
            nc.sync.dma_start(t[:], fw[name].ap())
            return t

        cw = {n: load_const(n) for n in shapes}

        def x2_load(pool, t0, w, nm, eng=None):
            # SWDGE (gpsimd) sprays descriptors over all 16 SDMA engines;
            # HWDGE fans out per 64-descriptor block (128-descr tile DMAs
            # land on only 2 engines), so bulk tile loads go SWDGE.
            xt = pool.tile([128, w, F], bf, name=nm, tag=nm)
            src = AP(x_d, t0 * F,
                     [[w * F, 2], [RP * F, C], [F, w], [1, F]])
            (eng or nc.gpsimd).dma_start(xt[:], src)
            return xt

        def ln_chunk(sb, ps, src_flat, n, nm):
            w1 = ps.tile([128, RW * FP], f32, name=f"{nm}w1", tag="Lw1")
            s1 = ps.tile([128, RW * FP], f32, name=f"{nm}s1", tag="Ls1")
            nc.tensor.matmul(w1[:, :n], cw["w1m_bf"][:], src_flat,
                             start=True, stop=True)
            sqw = sb.tile([128, RW * FP], bf, name=f"{nm}sqw", tag="Lsq")
            nc.scalar.activation(sqw[:, :n], w1[:, :n], AF.Square)
            nc.tensor.matmul(s1[:, :n], cw["bo_bf"][:], sqw[:, :n],
                             start=True, stop=True)
            inv = sb.tile([128, RW * FP], f32, name=f"{nm}inv", tag="Linv")
            nc.scalar.activation(inv[:, :n], s1[:, :n], AF.Abs_reciprocal_sqrt,
                                 scale=1.0 / 64.0)
            return w1, inv

        # persistent accumulators
        macc = per.tile([128, 1], f32, name="macc")
        nc.vector.memset(macc[:], 0.0)
        g2acc = [per.tile([128, F], f32, name=f"g2acc_{i}") for i in range(3)]
        for i in range(3):
            nc.vector.memset(g2acc[i][:], 0.0)
        ysacc = per.tile([128, 1], f32, name="ysacc")
        yqacc = per.tile([128, 1], f32, name="yqacc")
        nc.vector.memset(ysacc[:], 0.0)
        nc.vector.memset(yqacc[:], 0.0)

        # ============================ P1 ================================
        with tc.tile_pool(name="p1sb", bufs=3) as sb, \
             tc.tile_pool(name="p1ps", bufs=2, space="PSUM") as ps:
            for (t0, w) in _tiles(TH, 24):
                x2 = x2_load(sb, PADR + t0, w, "p1x")
                for (r0, rr) in _chunks(w):
                    n = rr * F
                    xc = x2[:, r0:r0 + rr, :].rearrange("p a b -> p (a b)")
                    w1, inv = ln_chunk(sb, ps, xc, n, "p1")
                    junk = sb.tile([128, RW * F], bf, name="p1junk",
                                   tag="p1junk")
                    acc = sb.tile([128, 1], f32, name="p1acc", tag="p1acc")
                    nc.vector.scalar_tensor_tensor(
                        junk[:, :n], w1[:, :n], 1.0, inv[:, :n],
                        OP.mult, OP.mult, accum_out=acc[:])
                    nc.vector.tensor_tensor(macc[:], macc[:], acc[:], OP.add)
        nc.sync.dma_start(AP(ag1i, 0, [[1, 128]]), macc[:])
        nc.gpsimd.collective_compute(
            "AllGather", OP.bypass, replica_groups=groups,
            ins=[ag1i.ap()], outs=[ag1o.ap()])
        m_a = per.tile([128, 2], f32, name="m_a")
        nc.sync.dma_start(m_a[:], AP(ag1o, 0, [[1, 128], [128, 2]]))
        mtot = per.tile([128, 1], f32, name="mtot")
        nc.vector.tensor_tensor(mtot[:], m_a[:, 0:1], m_a[:, 1:2], OP.add)

        # filt folds, horizontal stages
        ataps, kcv = [], []
        with tc.tile_pool(name="ffsb", bufs=2) as sb, \
             tc.tile_pool(name="ffps", bufs=2, space="PSUM") as ps:
            m64p = ps.tile([64, 1], f32, name="m64p")
            nc.tensor.matmul(m64p[:], cw["fold64"][:], mtot[:],
                             start=True, stop=True)
            m64 = per.tile([64, 1], f32, name="m64")
            nc.vector.tensor_copy(m64[:], m64p[:])
            for i in range(3):
                a_t, kc_t = _filt_fold(nc, sb, ps, per, cw, m64, i,
                                       f32, bf, AF, OP, AX)
                ataps.append(a_t)
                kcv.append(kc_t)
        if dsm_d is not None:
            nc.sync.dma_start(AP(dsm_d, 0, [[1, 128]]), mtot[:])
            for i in range(3):
                for k in range(KT):
                    nc.sync.dma_start(
                        AP(dsm_d, (1 + i * 3 + k) * 128, [[1, 128]]),
                        ataps[i][:, k:k + 1])
                nc.sync.dma_start(AP(dsm_d, (10 + i) * 128, [[1, 128]]),
                                  kcv[i][:, 0:1])
        dgh = []
        for i in range(3):
            for k in range(KT):
                # fused tap+LN-mean-removal stationary:
                # M_k^T = diag(a_k) (I - BO/64)  (w1m is symmetric)
                dt_ = per.tile([128, 128], bf, name=f"dgh_{i}_{k}")
                nc.vector.tensor_scalar_mul(dt_[:], cw["w1m_bf"][:],
                                            ataps[i][:, k:k + 1])
                dgh.append(dt_)

        if phases >= 2:
            # ============================ P2 ================================
            with tc.tile_pool(name="p2sb", bufs=3) as sb, \
                 tc.tile_pool(name="p2big", bufs=2) as big, \
                 tc.tile_pool(name="p2ps", bufs=1, space="PSUM") as ps, \
                 tc.tile_pool(name="p2pst", bufs=1, space="PSUM") as pst:
                for (t0, w) in _tiles(RP, 20):
                    x2 = x2_load(sb, t0, w, "p2x")
                    zzt = big.tile([128, 8 + w * FP + 8], bf, name="p2zzt",
                                   tag="p2zzt")
                    zz = zzt[:, 8:8 + w * FP].rearrange("p (a b) -> p a b", b=FP)
                    nc.vector.memset(zzt[:, 0:8], 0.0)
                    nc.vector.memset(zzt[:, 8 + w * FP:8 + w * FP + 8], 0.0)
                    for (r0, rr) in _chunks(w):
                        n = rr * F
                        xc = x2[:, r0:r0 + rr, :].rearrange("p a b -> p (a b)")
                        w1, inv = ln_chunk(sb, ps, xc, n, "p2a")
                        nc.vector.tensor_tensor(
                            zz[:, r0:r0 + rr, 7:7 + F],
                            w1[:, :n].rearrange("p (a b) -> p a b", b=F),
                            inv[:, :n].rearrange("p (a b) -> p a b", b=F),
                            OP.mult)
                    nc.vector.tensor_copy(zz[:, :, 0:7], zz[:, :, 14:7:-1])
                    nc.vector.tensor_copy(zz[:, :, 136:143], zz[:, :, 134:127:-1])
                    gpf = sb.tile([128, w], f32, name="p2gpf", tag="p2gpf")
                    nc.vector.tensor_reduce(gpf[:], zz[:, :, 7:7 + F], AX.X,
                                            OP.add)
                    for i in range(3):
                        d = DILS[i]
                        # grow' = grow - channel-block mean (fused tap
                        # matmuls M_k = (I-BO/64) diag(a_k) remove the mean
                        # of the tap sum; grow must match)
                        grow = sb.tile([128, w], f32, name="p2grow", tag="p2grow")
                        nc.vector.tensor_scalar(grow[:], gpf[:],
                                                cw[f"kg_{i}"][:, 0:1],
                                                kcv[i][:, 0:1], OP.mult, OP.add)
                        gmp = pst.tile([128, RW * F], f32, name="p2gmp",
                                       tag="p2gmp")
                        nc.tensor.matmul(gmp[:, :w], cw["bo_f"][:], grow[:],
                                         start=True, stop=True)
                        growp = sb.tile([128, w], f32, name="p2growp",
                                        tag="p2growp")
                        nc.vector.scalar_tensor_tensor(
                            growp[:], gmp[:, :w], -1.0 / 64.0, grow[:],
                            OP.mult, OP.add)
                        n2b = big.tile([128, w, F], bf, name="p2n2b",
                                       tag="p2n2b")
                        chs = _chunks(w)
                        for g0 in range(0, len(chs), 4):
                            grp = chs[g0:g0 + 4]
                            cps = [pst.tile([128, RW * F], f32,
                                            name=f"p2cp{cj}", tag=f"p2cp{cj}")
                                   for cj in range(len(grp))]
                            for k in range(KT):
                                a = 7 + (k - 1) * d
                                for cj, (r0, rr) in enumerate(grp):
                                    nc.tensor.matmul(
                                        cps[cj][:, :rr * F], dgh[i * KT + k][:],
                                        zz[:, r0:r0 + rr, a:a + F],
                                        start=(k == 0), stop=(k == 2))
                            for cj, (r0, rr) in enumerate(grp):
                                n = rr * F
                                w1f = sb.tile([128, RW * F], f32, name="p2w1f",
                                              tag="p2w1f")
                                nc.vector.tensor_tensor(
                                    w1f[:, :n].rearrange("p (a b) -> p a b",
                                                         b=F),
                                    cps[cj][:, :n].rearrange(
                                        "p (a b) -> p a b", b=F),
                                    growp[:, r0:r0 + rr].unsqueeze(2)
                                    .broadcast_to([128, rr, F]),
                                    OP.add)
                                sqw = sb.tile([128, RW * F], bf, name="p2sq",
                                              tag="p2sq")
                                nc.scalar.activation(sqw[:, :n], w1f[:, :n],
                                                     AF.Square)
                                s1 = ps.tile([128, RW * F], f32, name="p2s1b",
                                             tag="Ls1")
                                nc.tensor.matmul(s1[:, :n], cw["bo_bf"][:],
                                                 sqw[:, :n],
                                                 start=True, stop=True)
                                inv = sb.tile([128, RW * F], f32, name="p2inv",
                                              tag="Linv")
                                nc.scalar.activation(inv[:, :n], s1[:, :n],
                                                     AF.Abs_reciprocal_sqrt,
                                                     scale=1.0 / 64.0)
                                nc.vector.tensor_tensor(
                                    n2b[:, r0:r0 + rr, :].rearrange(
                                        "p a b -> p (a b)"),
                                    w1f[:, :n], inv[:, :n], OP.mult)
                        dst = AP(n2_d, i * C * RP * F + t0 * F,
                                 [[w * F, 2], [RP * F, C], [F, w], [1, F]])
                        nc.gpsimd.dma_start(dst, n2b[:])
                        # gap2 partials over strictly-local rows [PADR, PADR+TH)
                        rng = []
                        for q in range(2):
                            a = max(PADR - (t0 + q * w), 0)
                            bq = min(PADR + TH - (t0 + q * w), w)
                            rng.append((a, bq))
                        if rng[0] == (0, w) and rng[1] == (0, w):
                            red = sb.tile([128, F], f32, name="p2red",
                                          tag="p2red")
                            nc.vector.tensor_reduce(
                                red[:], n2b[:].transpose([0, 2, 1]),
                                AX.X, OP.add)
                            nc.vector.tensor_tensor(g2acc[i][:], g2acc[i][:],
                                                    red[:], OP.add)
                        else:
                            for q in range(2):
                                a, bq = rng[q]
                                if bq <= a:
                                    continue
                                p0, p1 = q * 64, q * 64 + 64
                                redh = sb.tile([128, F], f32, name="p2redh",
                                               tag="p2red")
                                nc.vector.tensor_reduce(
                                    redh[p0:p1],
                                    n2b[p0:p1, a:bq, :]
                                    .transpose([0, 2, 1]),
                                    AX.X, OP.add)
                                nc.vector.tensor_tensor(g2acc[i][p0:p1],
                                                        g2acc[i][p0:p1],
                                                        redh[p0:p1], OP.add)
            for i in range(3):
                nc.sync.dma_start(AP(ag2i, i * 128 * F, [[1, 128 * F]]),
                                  g2acc[i][:])
            nc.gpsimd.collective_compute(
                "AllGather", OP.bypass, replica_groups=groups,
                ins=[ag2i.ap()], outs=[ag2o.ap()])

            # filt folds vertical + gterm2
            gt2 = []
            with tc.tile_pool(name="f2sb", bufs=2) as sb, \
                 tc.tile_pool(name="f2ps", bufs=2, space="PSUM") as ps:
                for i in range(3):
                    ga = sb.tile([128, F], f32, name=f"f2ga_{i}", tag="f2ga")
                    gb = sb.tile([128, F], f32, name=f"f2gb_{i}", tag="f2gb")
                    nc.sync.dma_start(ga[:], AP(ag2o, i * 128 * F,
                                                [[F, 128], [1, F]]))
                    nc.sync.dma_start(gb[:], AP(ag2o, 3 * 128 * F + i * 128 * F,
                                                [[F, 128], [1, F]]))
                    gf = per.tile([128, F], f32, name=f"g2full_{i}")
                    nc.vector.tensor_tensor(gf[:], ga[:], gb[:], OP.add)
                    nsum = sb.tile([128, 1], f32, name=f"f2ns_{i}", tag="f2ns")
                    nc.vector.tensor_reduce(nsum[:], gf[:], AX.X, OP.add)
                    n64p = ps.tile([64, 1], f32, name=f"f2n64_{i}", tag="f2n64")
                    nc.tensor.matmul(n64p[:], cw["fold64"][:], nsum[:],
                                     start=True, stop=True)
                    n64 = sb.tile([64, 1], f32, name=f"f2n64s_{i}", tag="f2n64s")
                    nc.vector.tensor_copy(n64[:], n64p[:])
                    a_t, kc_t = _filt_fold(nc, sb, ps, per, cw, n64, i + 3,
                                           f32, bf, AF, OP, AX)
                    gt = per.tile([128, F], bf, name=f"gt2_{i}")
                    nc.vector.tensor_scalar(gt[:], gf[:],
                                            cw[f"kg_{i + 3}"][:, 0:1],
                                            kc_t[:, 0:1], OP.mult, OP.add)
                    gt2.append(gt)
                    ataps.append(a_t)
            # vertical taps folded into the mix conv stationaries:
            # (wg_i diag(av_ik))^T = diag(av_ik) wgbd_i
            dgv = []
            for i in range(3):
                for k in range(KT):
                    dt_ = per.tile([128, 128], bf, name=f"dgv_{i}_{k}")
                    nc.vector.tensor_scalar_mul(dt_[:], cw[f"wgbd_{i}"][:],
                                                ataps[3 + i][:, k:k + 1])
                    dgv.append(dt_)
            gt2mix = per.tile([128, F], f32, name="gt2mix")
            with tc.tile_pool(name="gtm", bufs=1, space="PSUM") as gps:
                gp = gps.tile([128, F], f32, name="gt2mp")
                for i in range(3):
                    nc.tensor.matmul(gp[:], cw[f"wgbd_{i}"][:], gt2[i][:],
                                     start=(i == 0), stop=(i == 2))
                nc.vector.tensor_copy(gt2mix[:], gp[:])

        if phases >= 3:
            # ============================ P3 ================================
            with tc.tile_pool(name="p3sb", bufs=3) as sb, \
                 tc.tile_pool(name="p3b2", bufs=2) as b2p, \
                 tc.tile_pool(name="p3ps", bufs=2, space="PSUM") as ps:
                for (t0, w) in _tiles(TH, 20):
                    x2 = x2_load(sb, PADR + t0, w, "p3x")
                    n2ws = []
                    for i in range(3):
                        n2w = b2p.tile([128, w + 14, F], bf, name=f"p3n2w_{i}",
                                       tag=f"p3n2w_{i}")
                        src = AP(n2_d, i * C * RP * F + t0 * F,
                                 [[w * F, 2], [RP * F, C], [F, w + 14], [1, F]])
                        nc.gpsimd.dma_start(n2w[:], src)
                        n2ws.append(n2w[:].rearrange("p a b -> p (a b)"))
                    yt = sb.tile([128, w, F], bf, name="p3y", tag="p3y")
                    chs = _chunks(w)
                    for g0 in range(0, len(chs), 4):
                        grp = chs[g0:g0 + 4]
                        yps = [ps.tile([128, RW * F], f32, name=f"p3yp{cj}",
                                       tag=f"p3yp{cj}")
                               for cj in range(len(grp))]
                        for cj, (r0, rr) in enumerate(grp):
                            nc.tensor.matmul(
                                yps[cj][:, :rr * F], cw["wbbd"][:],
                                x2[:, r0:r0 + rr, :]
                                .rearrange("p a b -> p (a b)"),
                                start=True, stop=False)
                        for i in range(3):
                            d = DILS[i]
                            for k in range(KT):
                                last = (i == 2 and k == 2)
                                for cj, (r0, rr) in enumerate(grp):
                                    off = (PADR + r0 + (k - 1) * d) * F
                                    nc.tensor.matmul(
                                        yps[cj][:, :rr * F],
                                        dgv[i * KT + k][:],
                                        n2ws[i][:, off:off + rr * F],
                                        start=False, stop=last)
                        for cj, (r0, rr) in enumerate(grp):
                            n = rr * F
                            acc = sb.tile([128, 1], f32, name="p3acc",
                                          tag="p3acc")
                            nc.vector.scalar_tensor_tensor(
                                yt[:, r0:r0 + rr, :], yps[cj][:, :n]
                                .rearrange("p (a b) -> p a b", b=F),
                                1.0,
                                gt2mix[:].unsqueeze(1)
                                .broadcast_to([128, rr, F]),
                                OP.mult, OP.add, accum_out=acc[:])
                            nc.vector.tensor_tensor(ysacc[:], ysacc[:],
                                                    acc[:], OP.add)
                            sqy = sb.tile([128, RW * F], f32, name="p3sqy",
                                          tag="p3sqy")
                            acq = sb.tile([128, 1], f32, name="p3acq",
                                          tag="p3acq")
                            nc.scalar.activation(
                                sqy[:, :n],
                                yt[:, r0:r0 + rr, :]
                                .rearrange("p a b -> p (a b)"),
                                AF.Square, accum_out=acq[:])
                            nc.vector.tensor_tensor(yqacc[:], yqacc[:],
                                                    acq[:], OP.add)
                    dst = AP(y_d, t0 * F,
                             [[w * F, 2], [TH * F, C], [F, w], [1, F]])
                    nc.gpsimd.dma_start(dst, yt[:])
            pk = per.tile([128, 2], f32, name="pk")
            nc.vector.tensor_copy(pk[:, 0:1], ysacc[:])
            nc.vector.tensor_copy(pk[:, 1:2], yqacc[:])
            nc.sync.dma_start(AP(ag3i, 0, [[1, 256]]), pk[:])
            nc.gpsimd.collective_compute(
                "AllGather", OP.bypass, replica_groups=groups,
                ins=[ag3i.ap()], outs=[ag3o.ap()])

            # GN scalars (y in y_d excludes convb_b; fold it analytically)
            gnS = per.tile([128, 1], f32, name="gnS")
            gnB = per.tile([128, 1], f32, name="gnB")
            with tc.tile_pool(name="gnsb", bufs=2) as sb, \
                 tc.tile_pool(name="gnps", bufs=2, space="PSUM") as ps:
                pa_ = sb.tile([128, 4], f32, name="gn_pa")
                nc.sync.dma_start(pa_[:, 0:2], AP(ag3o, 0, [[2, 128], [1, 2]]))
                nc.sync.dma_start(pa_[:, 2:4], AP(ag3o, 256, [[2, 128], [1, 2]]))
                sy = sb.tile([128, 1], f32, name="gn_sy")
                sq = sb.tile([128, 1], f32, name="gn_sq")
                nc.vector.tensor_tensor(sy[:], pa_[:, 0:1], pa_[:, 2:3], OP.add)
                nc.vector.tensor_tensor(sq[:], pa_[:, 1:2], pa_[:, 3:4], OP.add)
                NcF = float(TH * F)
                t1 = sb.tile([128, 1], f32, name="gn_t1")
                nc.vector.scalar_tensor_tensor(t1[:], sy[:], 2.0,
                                               cw["cbb"][:], OP.mult, OP.mult)
                nc.vector.tensor_tensor(sq[:], sq[:], t1[:], OP.add)
                cb2 = sb.tile([128, 1], f32, name="gn_cb2")
                nc.scalar.activation(cb2[:], cw["cbb"][:], AF.Square,
                                     scale=1.0)
                nc.vector.tensor_scalar(cb2[:], cb2[:], NcF, None, OP.mult)
                nc.vector.tensor_tensor(sq[:], sq[:], cb2[:], OP.add)
                nc.vector.scalar_tensor_tensor(t1[:], cw["cbb"][:], NcF, sy[:],
                                               OP.mult, OP.add)
                on1 = sb.tile([128, 1], f32, name="gn_on1")
                nc.vector.memset(on1[:], 1.0)
                tots = ps.tile([1, 2], f32, name="gn_tots")
                nc.tensor.matmul(tots[0:1, 0:1], on1[:], t1[:],
                                 start=True, stop=True)
                nc.tensor.matmul(tots[0:1, 1:2], on1[:], sq[:],
                                 start=True, stop=True)
                Ntot = float(C * T * F)
                mg = sb.tile([1, 1], f32, name="gn_mg")
                nc.vector.tensor_scalar(mg[:], tots[0:1, 0:1], 1.0 / Ntot, None,
                                        OP.mult)
                m2g = sb.tile([1, 1], f32, name="gn_m2g")
                nc.scalar.activation(m2g[:], mg[:], AF.Square)
                vg = sb.tile([1, 1], f32, name="gn_vg")
                nc.vector.scalar_tensor_tensor(vg[:], tots[0:1, 1:2], 1.0 / Ntot,
                                               m2g[:], OP.mult, OP.subtract)
                nc.vector.tensor_scalar(vg[:], vg[:], EPS, None, OP.add)
                ig = sb.tile([1, 1], f32, name="gn_ig")
                nc.scalar.activation(ig[:], vg[:], AF.Abs_reciprocal_sqrt)
                igb = sb.tile([128, 1], f32, name="gn_igb")
                mgb = sb.tile([128, 1], f32, name="gn_mgb")
                nc.gpsimd.partition_broadcast(igb[:], ig[:])
                nc.gpsimd.partition_broadcast(mgb[:], mg[:])
                nc.vector.tensor_tensor(gnS[:], igb[:], cw["gn_g"][:], OP.mult)
                nc.vector.tensor_tensor(gnB[:], cw["cbb"][:], mgb[:],
                                        OP.subtract)
                nc.vector.tensor_tensor(gnB[:], gnB[:], gnS[:], OP.mult)
                nc.vector.tensor_tensor(gnB[:], gnB[:], cw["gn_b"][:], OP.add)

        if phases >= 4:
            # ============================ P4 ================================
            with tc.tile_pool(name="p4sb", bufs=2) as sb, \
                 tc.tile_pool(name="p4ps", bufs=2, space="PSUM") as ps, \
                 tc.tile_pool(name="p4st", bufs=1, space="PSUM") as pst:
                for (t0, w) in _tiles(TH, 20):
                    y2 = sb.tile([128, w, F], bf, name="p4y", tag="p4y")
                    nc.gpsimd.dma_start(
                        y2[:], AP(y_d, t0 * F,
                                  [[w * F, 2], [TH * F, C], [F, w], [1, F]]))
                    ot = sb.tile([128, w, F], bf, name="p4o", tag="p4o")
                    nc.scalar.activation(ot[:], y2[:], AF.Prelu,
                                         bias=gnB[:, 0:1], scale=gnS[:, 0:1],
                                         alpha=cw["a0vec"][:, 0:1])
                    nc.gpsimd.dma_start(
                        AP(out_d, t0 * F,
                           [[w * F, 2], [TH * F, C], [F, w], [1, F]]), ot[:])
                    qk = sb.tile([64, w, F], bf, name="p4qk", tag="p4qk")
                    vt = sb.tile([128, w, F], bf, name="p4v", tag="p4v")
                    for (r0, rr) in _chunks(w):
                        n = rr * F
                        oc = ot[:, r0:r0 + rr, :].rearrange("p a b -> p (a b)")
                        qps = ps.tile([64, RW * F], f32, name="p4qps",
                                      tag="p4qps")
                        vps = ps.tile([128, RW * F], f32, name="p4vps",
                                      tag="p4vps")
                        nc.tensor.matmul(qps[:, :n], cw["wqkbd"][:], oc,
                                         start=True, stop=True)
                        nc.tensor.matmul(vps[:, :n], cw["wvbd"][:], oc,
                                         start=True, stop=True)
                        nc.scalar.activation(
                            qk[:, r0:r0 + rr, :].rearrange("p a b -> p (a b)"),
                            qps[:, :n], AF.Prelu, bias=cw["qkb"][:, 0:1],
                            alpha=cw["qka"][:, 0:1])
                        nc.scalar.activation(
                            vt[:, r0:r0 + rr, :].rearrange("p a b -> p (a b)"),
                            vps[:, :n], AF.Prelu, bias=cw["vb"][:, 0:1],
                            alpha=cw["va"][:, 0:1])
                    qs = sb.tile([64, w], f32, name="p4qs", tag="p4qs")
                    vs = sb.tile([128, w], f32, name="p4vs", tag="p4vs")
                    nc.vector.tensor_reduce(qs[:], qk[:], AX.X, OP.add)
                    nc.vector.tensor_reduce(vs[:], vt[:], AX.X, OP.add)
                    qq = sb.tile([64, w, F], f32, name="p4qq", tag="p4qq")
                    vv = sb.tile([128, w, F], f32, name="p4vv", tag="p4vv")
                    nc.scalar.activation(qq[:], qk[:], AF.Square)
                    nc.scalar.activation(vv[:], vt[:], AF.Square)
                    qsq = sb.tile([64, w], f32, name="p4qsq", tag="p4qsq")
                    vsq = sb.tile([128, w], f32, name="p4vsq", tag="p4vsq")
                    nc.vector.tensor_reduce(qsq[:], qq[:], AX.X, OP.add)
                    nc.vector.tensor_reduce(vsq[:], vv[:], AX.X, OP.add)
                    stq = pst.tile([16, 2 * w], f32, name="p4stq", tag="p4stq")
                    stv = pst.tile([8, 2 * w], f32, name="p4stv", tag="p4stv")
                    nc.tensor.matmul(stq[:, 0:w], cw["grp16"][:], qs[:],
                                     start=True, stop=True)
                    nc.tensor.matmul(stq[:, w:2 * w], cw["grp16"][:], qsq[:],
                                     start=True, stop=True)
                    nc.tensor.matmul(stv[:, 0:w], cw["grp8"][:], vs[:],
                                     start=True, stop=True)
                    nc.tensor.matmul(stv[:, w:2 * w], cw["grp8"][:], vsq[:],
                                     start=True, stop=True)
                    mivs = {}
                    for (st, npart, ncnt, nm) in ((stq, 16, 4 * F, "q"),
                                                  (stv, 8, 16 * F, "v")):
                        mu = sb.tile([npart, w], f32, name=f"p4mu{nm}",
                                     tag=f"p4mu{nm}")
                        nc.vector.tensor_scalar(mu[:], st[:, 0:w], 1.0 / ncnt,
                                                None, OP.mult)
                        m2_ = sb.tile([npart, w], f32, name=f"p4m2{nm}",
                                      tag=f"p4m2{nm}")
                        nc.scalar.activation(m2_[:], mu[:], AF.Square)
                        var = sb.tile([npart, w], f32, name=f"p4var{nm}",
                                      tag=f"p4var{nm}")
                        nc.vector.scalar_tensor_tensor(var[:], st[:, w:2 * w],
                                                       1.0 / ncnt, m2_[:],
                                                       OP.mult, OP.subtract)
                        nc.vector.tensor_scalar(var[:], var[:], EPS, None,
                                                OP.add)
                        iv = sb.tile([npart, w], f32, name=f"p4iv{nm}",
                                     tag=f"p4iv{nm}")
                        nc.scalar.activation(iv[:], var[:], AF.Abs_reciprocal_sqrt)
                        mivs[nm] = (mu, iv)
                    qb_ = pst.tile([64, 2 * w], f32, name="p4qb", tag="p4qb")
                    vb_ = pst.tile([128, 2 * w], f32, name="p4vb", tag="p4vb")
                    nc.tensor.matmul(qb_[:, 0:w], cw["bc16"][:], mivs["q"][0][:],
                                     start=True, stop=True)
                    nc.tensor.matmul(qb_[:, w:2 * w], cw["bc16"][:],
                                     mivs["q"][1][:], start=True, stop=True)
                    nc.tensor.matmul(vb_[:, 0:w], cw["bc8"][:], mivs["v"][0][:],
                                     start=True, stop=True)
                    nc.tensor.matmul(vb_[:, w:2 * w], cw["bc8"][:],
                                     mivs["v"][1][:], start=True, stop=True)
                    qkn = sb.tile([64, w, F], bf, name="p4qkn", tag="p4qkn")
                    vn = sb.tile([128, w, F], bf, name="p4vn", tag="p4vn")
                    qbs = sb.tile([64, 2 * w], f32, name="p4qbs", tag="p4qbs")
                    nc.vector.tensor_copy(qbs[:], qb_[:])
                    nc.vector.tensor_tensor(
                        qkn[:], qk[:],
                        qbs[:, 0:w].unsqueeze(2).broadcast_to([64, w, F]),
                        OP.subtract)
                    nc.vector.tensor_tensor(
                        qkn[:], qkn[:],
                        qbs[:, w:2 * w].unsqueeze(2).broadcast_to([64, w, F]),
                        OP.mult)
                    nc.vector.tensor_tensor(
                        vn[:], vt[:],
                        vb_[:, 0:w].unsqueeze(2).broadcast_to([128, w, F]),
                        OP.subtract)
                    nc.vector.tensor_tensor(
                        vn[:], vn[:],
                        vb_[:, w:2 * w].unsqueeze(2).broadcast_to([128, w, F]),
                        OP.mult)
                    # per-head contiguous K/V staging so the AllGather can be
                    # split per head and overlapped with P5 attention
                    for q in range(2):
                        ro = (t0 + q * w) * F
                        nc.sync.dma_start(
                            AP(qh_d, ro, [[TH * F, 16], [F, w], [1, F]]),
                            qkn[q * 32:q * 32 + 16])
                        for h in range(H):
                            nc.sync.dma_start(
                                AP(kvh_d[h], ro,
                                   [[TH * F, 4], [F, w], [1, F]]),
                                qkn[q * 32 + 16 + 4 * h:q * 32 + 20 + 4 * h])
                            nc.sync.dma_start(
                                AP(kvh_d[h], 4 * TH * F + ro,
                                   [[TH * F, 16], [F, w], [1, F]]),
                                vn[q * 64 + 16 * h:q * 64 + 16 * (h + 1)])
            for h in range(H):
                nc.gpsimd.collective_compute(
                    "AllGather", OP.bypass, replica_groups=groups,
                    ins=[kvh_d[h].ap()], outs=[kvf_d[h].ap()])

        if phases >= 5:
            # ============================ P5 ================================
            KCH = [(0, 128), (128, 128), (256, 128), (384, 128), (512, 4)]
            NCH = [(0, 512), (512, 512), (1024, 512), (1536, 512), (2048, 16)]
            import contextlib as _cl
            for h in range(H):
                hx = _cl.ExitStack()
                sb = hx.enter_context(tc.tile_pool(name=f"a{h}sb", bufs=3))
                res = hx.enter_context(tc.tile_pool(name=f"a{h}res", bufs=1))
                ps = hx.enter_context(tc.tile_pool(name=f"a{h}ps", bufs=2,
                                                   space="PSUM"))
                pss = hx.enter_context(tc.tile_pool(name=f"a{h}pss", bufs=1,
                                                    space="PSUM"))
                pso = hx.enter_context(tc.tile_pool(name=f"a{h}pso", bufs=2,
                                                    space="PSUM"))
                ktr = [res.tile([kn, T], bf, name=f"ktr{h}_{ci}")
                       for ci, (k0, kn) in enumerate(KCH)]
                qtr = [res.tile([kn, TH], bf, name=f"qtr{h}_{ci}")
                       for ci, (k0, kn) in enumerate(KCH)]
                vres = [res.tile([MB, DF], bf, name=f"vres{h}_{b}")
                        for b in range(8)]
                for b in range(8):
                    seg, tl = divmod(b, 4)
                    km = sb.tile([MB, EF], bf, name=f"km{h}", tag="km")
                    nc.sync.dma_start(
                        km[:], AP(kvf_d[h],
                                  seg * 20 * TH * F + tl * MB * F,
                                  [[F, MB], [TH * F, E], [1, F]]))
                    for ci, (k0, kn) in enumerate(KCH):
                        tp = ps.tile([128, MB], bf, name=f"tp{h}", tag="tp")
                        nc.tensor.transpose(tp[:kn, :], km[:, k0:k0 + kn],
                                            cw["ident_bf"][0:MB, 0:MB])
                        nc.vector.tensor_copy(ktr[ci][:, b * MB:(b + 1) * MB],
                                              tp[:kn, :])
                    nc.sync.dma_start(
                        vres[b][:], AP(kvf_d[h],
                                       seg * 20 * TH * F + 4 * TH * F
                                       + tl * MB * F,
                                       [[F, MB], [TH * F, D], [1, F]]))
                for b in range(4):
                    km = sb.tile([MB, EF], bf, name=f"qm{h}", tag="km")
                    nc.sync.dma_start(
                        km[:], AP(qh_d, (h * 4) * TH * F + b * MB * F,
                                  [[F, MB], [TH * F, E], [1, F]]))
                    for ci, (k0, kn) in enumerate(KCH):
                        tp = ps.tile([128, MB], bf, name=f"tp{h}", tag="tp")
                        nc.tensor.transpose(tp[:kn, :], km[:, k0:k0 + kn],
                                            cw["ident_bf"][0:MB, 0:MB])
                        nc.vector.tensor_copy(qtr[ci][:, b * MB:(b + 1) * MB],
                                              tp[:kn, :])
                # lng/lnb per-partition post-transpose
                for ci, (k0, kn) in enumerate(KCH):
                    for (lg, lb, tt) in ((("klngT"), ("klnbT"), ktr),
                                         (("qlngT"), ("qlnbT"), qtr)):
                        vg_ = sb.tile([128, 1], f32, name=f"lg{h}", tag="lg")
                        vb2 = sb.tile([128, 1], f32, name=f"lb{h}", tag="lb")
                        nc.sync.dma_start(vg_[:kn, :],
                                          AP(fw[lg], h * 640 + k0, [[1, kn]]))
                        nc.sync.dma_start(vb2[:kn, :],
                                          AP(fw[lb], h * 640 + k0, [[1, kn]]))
                        nc.vector.tensor_scalar(tt[ci][:], tt[ci][:],
                                                vg_[:kn, 0:1], vb2[:kn, 0:1],
                                                OP.mult, OP.add)
                lngb = res.tile([MB, DF], f32, name=f"lngb{h}")
                lrow = sb.tile([1, DF], f32, name=f"lrow{h}", tag="lrow")
                nc.sync.dma_start(lrow[:], AP(fw["vlngT"], h * DF, [[DF, 1],
                                                                   [1, DF]]))
                nc.gpsimd.partition_broadcast(lngb[:], lrow[:])
                for mt in range(4):
                    spt = pss.tile([MB, 1024], f32, name=f"spt{h}", tag="spt")
                    for ci, (k0, kn) in enumerate(KCH):
                        lhs = qtr[ci][:, mt * MB:(mt + 1) * MB]
                        nc.tensor.matmul(spt[:, 0:500], lhs, ktr[ci][:, 0:500],
                                         start=(ci == 0), stop=(ci == 4))
                        nc.tensor.matmul(spt[:, 512:1012], lhs,
                                         ktr[ci][:, 500:1000],
                                         start=(ci == 0), stop=(ci == 4))
                    sview = spt[:].rearrange("p (a b) -> p a b", b=512)[:, :, 0:500]
                    mx = sb.tile([MB, 1], f32, name=f"mx{h}", tag="mx")
                    nc.vector.tensor_reduce(mx[:], sview, AX.XY, OP.max)
                    bias = sb.tile([MB, 1], f32, name=f"bias{h}", tag="bias")
                    nc.vector.tensor_scalar(bias[:], mx[:], -ISCALE, None,
                                            OP.mult)
                    pexp = sb.tile([MB, 1000], bf, name=f"pexp{h}", tag="pexp")
                    lsum = sb.tile([MB, 1], f32, name=f"lsum{h}", tag="lsum")
                    nc.scalar.activation(pexp[:], sview, AF.Exp,
                                         bias=bias[:, 0:1], scale=ISCALE,
                                         accum_out=lsum[:])
                    linv = sb.tile([MB, 1], f32, name=f"linv{h}", tag="linv")
                    nc.vector.reciprocal(linv[:], lsum[:])
                    ptr = []
                    for b in range(8):
                        tp2 = ps.tile([MB, MB], bf, name=f"tp2{h}", tag="tp2")
                        nc.tensor.transpose(tp2[:], pexp[:, b * MB:(b + 1) * MB],
                                            cw["ident_bf"][0:MB, 0:MB])
                        pb_ = sb.tile([MB, MB], bf, name=f"ptr{h}_{b}",
                                      tag=f"ptr{b}")
                        nc.vector.tensor_copy(pb_[:], tp2[:])
                        ptr.append(pb_)
                    ob = sb.tile([MB, DF], bf, name=f"ob{h}", tag="ob")
                    for (n0, nn) in NCH:
                        op_ = pso.tile([MB, 512], f32, name=f"op{h}", tag="op")
                        for b in range(8):
                            nc.tensor.matmul(op_[:, :nn], ptr[b][:],
                                             vres[b][:, n0:n0 + nn],
                                             start=(b == 0), stop=(b == 7))
                        nc.vector.scalar_tensor_tensor(
                            ob[:, n0:n0 + nn], op_[:, :nn], linv[:, 0:1],
                            lngb[:, n0:n0 + nn], OP.mult, OP.mult)
                    nc.gpsimd.dma_start(
                        AP(o_d, mt * MB * H * D * F + h * D * F,
                           [[H * D * F, MB], [1, D * F]]), ob[:])
                hx.close()

        if phases >= 6:
            # ============================ P6 ================================
            with tc.tile_pool(name="p6sb", bufs=2) as sb, \
                 tc.tile_pool(name="p6ps", bufs=2, space="PSUM") as ps, \
                 tc.tile_pool(name="p6st", bufs=1, space="PSUM") as pst:
                for (t0, w) in _tiles(TH, 20):
                    o2 = sb.tile([128, w, F], bf, name="p6o", tag="p6o")
                    for q in range(2):
                        nc.gpsimd.dma_start(
                            o2[q * 64:(q + 1) * 64],
                            AP(o_d, (t0 + q * w) * H * D * F,
                               [[F, 64], [H * D * F, w], [1, F]]))
                    u2 = sb.tile([128, w, F], f32, name="p6u2", tag="p6u2")
                    for (r0, rr) in _chunks(w):
                        n = rr * F
                        pps = ps.tile([128, RW * F], f32, name="p6pps",
                                      tag="p6pps")
                        nc.tensor.matmul(
                            pps[:, :n], cw["pwbd"][:],
                            o2[:, r0:r0 + rr, :].rearrange("p a b -> p (a b)"),
                            start=True, stop=True)
                        tmp = sb.tile([128, RW, F], f32, name="p6tmp",
                                      tag="p6tmp")
                        nc.vector.tensor_tensor(
                            tmp[:, :rr, :],
                            pps[:, :n].rearrange("p (a b) -> p a b", b=F),
                            cw["pconst"][:].unsqueeze(1)
                            .broadcast_to([128, rr, F]),
                            OP.add)
                        nc.scalar.activation(
                            u2[:, r0:r0 + rr, :].rearrange("p a b -> p (a b)"),
                            tmp[:, :rr, :].rearrange("p a b -> p (a b)"),
                            AF.Prelu, bias=cw["pb"][:, 0:1],
                            alpha=cw["pa"][:, 0:1])
                    us = sb.tile([128, w], f32, name="p6us", tag="p6us")
                    nc.vector.tensor_reduce(us[:], u2[:], AX.X, OP.add)
                    uq = sb.tile([128, w, F], f32, name="p6uq", tag="p6uq")
                    nc.scalar.activation(uq[:], u2[:], AF.Square)
                    usq = sb.tile([128, w], f32, name="p6usq", tag="p6usq")
                    nc.vector.tensor_reduce(usq[:], uq[:], AX.X, OP.add)
                    st2 = pst.tile([2, 2 * w], f32, name="p6st2", tag="p6st2")
                    nc.tensor.matmul(st2[:, 0:w], cw["redq"][:], us[:],
                                     start=True, stop=True)
                    nc.tensor.matmul(st2[:, w:2 * w], cw["redq"][:], usq[:],
                                     start=True, stop=True)
                    ncnt = float(64 * F)
                    mu2 = sb.tile([2, w], f32, name="p6mu2", tag="p6mu2")
                    nc.vector.tensor_scalar(mu2[:], st2[:, 0:w], 1.0 / ncnt,
                                            None, OP.mult)
                    m22 = sb.tile([2, w], f32, name="p6m22", tag="p6m22")
                    nc.scalar.activation(m22[:], mu2[:], AF.Square)
                    var2 = sb.tile([2, w], f32, name="p6var2", tag="p6var2")
                    nc.vector.scalar_tensor_tensor(var2[:], st2[:, w:2 * w],
                                                   1.0 / ncnt, m22[:],
                                                   OP.mult, OP.subtract)
                    nc.vector.tensor_scalar(var2[:], var2[:], EPS, None, OP.add)
                    iv2 = sb.tile([2, w], f32, name="p6iv2", tag="p6iv2")
                    nc.scalar.activation(iv2[:], var2[:], AF.Abs_reciprocal_sqrt)
                    mb2 = pst.tile([128, 2 * w], f32, name="p6mb2", tag="p6mb2")
                    nc.tensor.matmul(mb2[:, 0:w], cw["ind2f"][:], mu2[:],
                                     start=True, stop=True)
                    nc.tensor.matmul(mb2[:, w:2 * w], cw["ind2f"][:], iv2[:],
                                     start=True, stop=True)
                    nc.vector.tensor_tensor(
                        u2[:], u2[:],
                        mb2[:, 0:w].unsqueeze(2).broadcast_to([128, w, F]),
                        OP.subtract)
                    nc.vector.tensor_tensor(
                        u2[:], u2[:],
                        mb2[:, w:2 * w].unsqueeze(2).broadcast_to([128, w, F]),
                        OP.mult)
                    nc.vector.tensor_tensor(
                        u2[:], u2[:],
                        cw["plng"][:].unsqueeze(1).broadcast_to([128, w, F]),
                        OP.mult)
                    rt = sb.tile([128, w, F], bf, name="p6rt", tag="p6rt")
                    nc.gpsimd.dma_start(
                        rt[:], AP(out_d, t0 * F,
                                  [[w * F, 2], [TH * F, C], [F, w], [1, F]]))
                    r1 = sb.tile([128, w, F], f32, name="p6r1", tag="p6r1")
                    nc.vector.tensor_tensor(
                        r1[:], rt[:],
                        cw["plnb"][:].unsqueeze(1).broadcast_to([128, w, F]),
                        OP.add)
                    fint = sb.tile([128, w, F], bf, name="p6fin", tag="p6fin")
                    nc.gpsimd.tensor_tensor(fint[:], u2[:], r1[:], OP.add)
                    nc.scalar.dma_start(
                        AP(fin, t0 * F,
                           [[w * F, 2], [TH * F, C], [F, w], [1, F]]), fint[:])
        if phases < 6:
            with tc.tile_pool(name="dummy", bufs=1) as dp:
                zt = dp.tile([C, 16], bf, name="zfin")
                nc.vector.memset(zt[:], 0.0)
                nc.sync.dma_start(AP(fin, 0, [[TH * F, C], [1, 16]]), zt[:])
        ctx.close()
    nc.compile()
    return nc


def _filt_fold(nc, sb, ps, per, cw, mean64, i, f32, bf, AF, OP, AX):
    u1 = sb.tile([64, 1], f32, name=f"u1_{i}", tag="ffu1")
    nc.vector.tensor_scalar(u1[:], mean64[:], cw[f"gtf_{i}"][:, 0:1],
                            cw[f"c64_{i}"][:, 0:1], OP.mult, OP.add)
    ftp = ps.tile([1, 12], f32, name=f"ftp_{i}", tag="ffftp")
    nc.tensor.matmul(ftp[:], u1[:], cw[f"lwT_{i}"][:], start=True, stop=True)
    ft = sb.tile([1, 12], f32, name=f"ft_{i}", tag="ffft")
    nc.scalar.activation(ft[:], ftp[:], AF.Tanh)
    ft4 = sb.tile([4, 3], f32, name=f"ft4_{i}", tag="ffft4")
    nc.sync.dma_start(ft4[:], ft[:].rearrange("o (g k) -> (o g) k", g=4))
    wcp = ps.tile([128, 3], f32, name=f"wcp_{i}", tag="ffwcp")
    nc.tensor.matmul(wcp[:], cw["grp4"][:], ft4[:], start=True, stop=True)
    atap = per.tile([128, 3], f32, name=f"atap_{i}")
    nc.vector.tensor_scalar_mul(atap[:], wcp[:], cw[f"gs_{i}"][:, 0:1])
    nc.vector.tensor_tensor(atap[:, 1:2], atap[:, 1:2],
                            cw[f"hc_{i}"][:, 0:1], OP.add)
    wcs = sb.tile([128, 1], f32, name=f"wcs_{i}", tag="ffwcs")
    nc.vector.tensor_reduce(wcs[:], wcp[:], AX.X, OP.add)
    kc = per.tile([128, 1], f32, name=f"kc_{i}")
    nc.vector.tensor_scalar(kc[:], wcs[:], cw[f"cs_{i}"][:, 0:1],
                            cw[f"cb_{i}"][:, 0:1], OP.mult, OP.add)
    nc.vector.tensor_tensor(kc[:], kc[:], cw[f"gc_{i}"][:, 0:1], OP.add)
    return atap, kc


# ---------------------------------------------------------------------------
# host entry
# ---------------------------------------------------------------------------

def _prep_inputs(inputs, fold):
    import ml_dtypes
    x = np.asarray(inputs["x"], np.float32)
    in_maps = []
    for c in range(8):
        s, hf = divmod(c, 2)
        xs = np.pad(x[s], ((0, 0), (PADR, PADR), (0, 0)), mode="reflect")
        xc = xs[:, hf * TH:hf * TH + RP, :]
        m = {"x_d": np.ascontiguousarray(xc).astype(ml_dtypes.bfloat16)}
        for n, (sh, isbf) in _fold_shapes().items():
            v = fold[n]
            m[n] = v.astype(ml_dtypes.bfloat16) if isbf else v
        for n in _dram_only_shapes():
            m[n] = fold[n]
        in_maps.append(m)
    return in_maps


def kernel(**inputs):
    import os
    from concourse.bass_utils import run_bass_kernel_spmd
    global LAST_EXEC_NS
    if "nc" not in _CACHE:
        _CACHE["nc"] = _build(dbg=_CACHE.get("dbg", ()),
                              phases=int(os.environ.get("KPHASES", "6")))
    nc = _CACHE["nc"]
    fold = _fold_weights(inputs)
    in_maps = _prep_inputs(inputs, fold)
    kw = {}
    if os.environ.get("KTRACE"):
        import tempfile
        base = os.environ.get("KTRACE_DIR",
                              os.path.join(os.getcwd(), "work"))
        os.makedirs(base, exist_ok=True)
        tdir = tempfile.mkdtemp(prefix="trace_", dir=base)
        with open(os.path.join(base, "last_trace_path.txt"), "w") as f:
            f.write(tdir)
        tc_ = os.environ.get("KTRACE_CORES", "0")
        kw = dict(trace=True, tmpdir=tdir,
                  trace_cores=[int(c) for c in tc_.split(",")])
    res = run_bass_kernel_spmd(nc, in_maps, core_ids=list(range(8)), **kw)
    _CACHE["last"] = res
    if getattr(res, "exec_time_ns", None):
        LAST_EXEC_NS = res.exec_time_ns
    out = np.zeros((B, C, T, F), np.float32)
    for c in range(8):
        s, hf = divmod(c, 2)
        out[s][:, hf * TH:(hf + 1) * TH, :] = \
            res.results[c]["fin"].astype(np.float32)
    return out

